# revision 1
# baseline (speedup 1.0000x reference)
# Trainium2 Bass kernel for nn_CrossAttention_noise (B=4, T1=T2=1024, D=1024,
# H=16, DK=64, K=13, FF=4096), SPMD over 8 NeuronCores.
#
# Sharding: core i handles batch b=i//2 and query-token half t0=(i%2)*512.
# Each core computes its 512 output tokens end-to-end (the K/V convolution
# over the full clean sequence is duplicated between the two cores of a
# batch; no collectives).  Big matmuls run in bf16 with fp32 PSUM
# accumulation; layernorms / softmax / residuals stay fp32.
#
# Layout convention: "T" suffix = channels on partitions, tokens on the free
# dim (the natural matmul layout here); plain tiles = tokens on partitions.
import numpy as np
import ml_dtypes
from contextlib import ExitStack

import concourse.bass as bass
import concourse.mybir as mybir
import concourse.tile as tile
from concourse import bacc
from concourse.bass_utils import run_bass_kernel_spmd
from concourse.masks import make_identity

BF16 = mybir.dt.bfloat16
F32 = mybir.dt.float32
AF = mybir.ActivationFunctionType
ALU = mybir.AluOpType
AX = mybir.AxisListType

B, T, D, H, DK, KW, FF = 4, 1024, 1024, 16, 64, 13, 4096
TQ = 512          # query tokens per core
NHW = 768         # noisy halo window rows (zero-padded on host)
NT2W = 528        # nt2 width (valid cols 0..523)
CT2W = 1040       # ct2 width (valid cols 0..1035)
P = 128
EPS1, EPS2 = 1e-5, 1e-6
NEG = -1.0e30


def _ln_apply(nc, pool, x, out, eps_ap, affine=None):
    """out = (x - mean)/sqrt(var + eps) rowwise; x [p, D] f32 in SBUF.

    Heavy passes run on the scalar engine (accum_out reductions + fused
    scale/bias apply); DVE only does tiny [p,1] ops."""
    p = x.shape[0]
    s = pool.tile([P, 1], F32, tag="ln_s", name="ln_s")[:p]
    sq = pool.tile([P, 1], F32, tag="ln_sq", name="ln_sq")[:p]
    scr = pool.tile([P, D], BF16, tag="ln_scr", name="ln_scr", bufs=2)[:p]
    nc.vector.reduce_sum(s, x, axis=AX.X)
    nc.scalar.activation(scr, x, AF.Square, accum_out=sq)
    mu = pool.tile([P, 1], F32, tag="ln_mu", name="ln_mu")[:p]
    nc.vector.tensor_scalar_mul(mu, s, 1.0 / D)
    musq = pool.tile([P, 1], F32, tag="ln_musq", name="ln_musq")[:p]
    nc.vector.tensor_tensor(musq, mu, mu, ALU.mult)
    var = pool.tile([P, 1], F32, tag="ln_var", name="ln_var")[:p]
    nc.vector.tensor_scalar(var, sq, 1.0 / D, musq, ALU.mult, ALU.subtract)
    std = pool.tile([P, 1], F32, tag="ln_std", name="ln_std")[:p]
    nc.scalar.activation(std, var, AF.Sqrt, bias=eps_ap[:p])
    rstd = pool.tile([P, 1], F32, tag="ln_rstd", name="ln_rstd")[:p]
    nc.vector.reciprocal(rstd, std)
    beta = pool.tile([P, 1], F32, tag="ln_beta", name="ln_beta")[:p]
    nc.vector.tensor_tensor(beta, mu, rstd, ALU.mult)
    nc.vector.tensor_scalar_mul(beta, beta, -1.0)
    if affine is not None:
        g, b = affine
        negmu = pool.tile([P, 1], F32, tag="ln_negmu", name="ln_negmu")[:p]
        nc.vector.tensor_scalar_mul(negmu, mu, -1.0)
        nc.vector.scalar_tensor_tensor(out, x, negmu, g, ALU.add, ALU.mult)
        nc.vector.scalar_tensor_tensor(out, out, rstd, b, ALU.mult, ALU.add)
    else:
        nc.vector.tensor_scalar(out, x, rstd, beta, ALU.mult, ALU.add)


def _nt2_rng(r, shift, width):
    """Dest/src col ranges for copying transpose block r into a shifted row."""
    lo = r * P - shift
    hi = lo + P
    d0, d1 = max(lo, 0), min(hi, width)
    if d1 <= d0:
        return None
    return d0, d1, d0 - lo


def build_nc():
    nc = bacc.Bacc("TRN2", target_bir_lowering=False, debug=False,
                   num_devices=8)
    dt = {}

    def din(name, shape, dtype):
        dt[name] = nc.dram_tensor(name, list(shape), dtype,
                                  kind="ExternalInput").ap()

    din("noisyH", (NHW, D), F32)          # rows [t0-128, t0+640), zero padded
    din("clean", (T, D), F32)
    din("hm", (P, NHW), BF16)              # halo-token validity (rows equal)
    din("maskb", (T,), F32)               # 0 / -1e30 additive key mask
    din("mod", (6, D), F32)               # sh_msa,1+sc_msa,g_msa,sh_mlp,1+sc_mlp,g_mlp
    din("lng", (P, D), F32)               # ln_noisy_g broadcast to 128 rows
    din("lnb", (P, D), F32)
    din("clng", (D,), F32)
    din("clnb", (D,), F32)
    din("wql", (P, H, 7, DK), BF16)
    din("wkl", (P, H, 7, DK), BF16)
    din("wvl", (P, H, 7, DK), BF16)
    din("bq", (D,), F32)
    din("bk", (D,), F32)
    din("bv", (D,), F32)
    din("fcw", (8, P, 8, P), BF16)        # fc_w.T tiles [mc][kp][ko][mj]
    din("fcb", (D,), F32)
    din("w1t", (32, P, 8, P), BF16)       # ff_w1.T tiles [mc][kp][ko][mj]
    din("fb1", (FF,), F32)
    din("w2t", (8, 4, P, 8, P), BF16)     # ff_w2.T tiles [mc][kq][kp][k8][mj]
    din("fb2", (D,), F32)
    out_ap = nc.dram_tensor("out", [TQ, D], F32, kind="ExternalOutput").ap()

    with tile.TileContext(nc) as tc:
        _emit(tc, dt, out_ap)
    nc.compile()
    return nc


def _emit(tc, dt, out_ap):
    nc = tc.nc
    with ExitStack() as ctx:
        const = ctx.enter_context(tc.tile_pool(name="const", bufs=1))
        small = ctx.enter_context(tc.tile_pool(name="small", bufs=3))
        lnio = ctx.enter_context(tc.tile_pool(name="lnio", bufs=3))
        big = ctx.enter_context(tc.tile_pool(name="bigsb", bufs=1))
        trans = ctx.enter_context(tc.tile_pool(name="trans", bufs=3))
        wpool = ctx.enter_context(tc.tile_pool(name="wstream", bufs=6))
        psc = ctx.enter_context(tc.tile_pool(name="psc", bufs=2, space="PSUM"))
        ppv = ctx.enter_context(tc.tile_pool(name="ppv", bufs=1, space="PSUM"))
        ptp = ctx.enter_context(tc.tile_pool(name="ptp", bufs=3, space="PSUM"))
        psm = ctx.enter_context(tc.tile_pool(name="psm", bufs=2, space="PSUM"))

        ident = const.tile([P, P], BF16)
        make_identity(nc, ident)
        eps1_t = const.tile([P, 1], F32)
        nc.vector.memset(eps1_t, EPS1)
        eps2_t = const.tile([P, 1], F32)
        nc.vector.memset(eps2_t, EPS2)

        def chanvec(name, w=8):
            t = const.tile([P, w], F32, tag=f"cv_{name}")
            nc.sync.dma_start(t, dt[name].rearrange("(m p) -> p m", p=P))
            return t

        bq_s, bk_s, bv_s = chanvec("bq"), chanvec("bk"), chanvec("bv")
        fcb_s, fb2_s = chanvec("fcb"), chanvec("fb2")
        clng_s, clnb_s = chanvec("clng"), chanvec("clnb")
        maskb_s = chanvec("maskb")
        fb1_s = chanvec("fb1", 32)
        mod_s = const.tile([P, 6, 8], F32)
        for s in range(6):
            nc.sync.dma_start(mod_s[:, s, :],
                              dt["mod"][s].rearrange("(m p) -> p m", p=P))
        sh_msa, sc_msa, g_msa = mod_s[:, 0, :], mod_s[:, 1, :], mod_s[:, 2, :]
        sh_mlp, sc_mlp, g_mlp = mod_s[:, 3, :], mod_s[:, 4, :], mod_s[:, 5, :]
        hm_s = const.tile([P, NHW], BF16)
        nc.sync.dma_start(hm_s, dt["hm"])
        lng_s = const.tile([P, D], F32)
        nc.sync.dma_start(lng_s, dt["lng"])
        lnb_s = const.tile([P, D], F32)
        nc.sync.dma_start(lnb_s, dt["lnb"])

        xres = big.tile([P, 4, D], F32)        # LN1 rows [t0, t0+512); later x
        attnT = big.tile([P, 8, TQ], BF16)     # concat_h(out_h/l_h), chan-major

        with tc.tile_pool(name="bigc", bufs=1) as bigc:
            # ---- Phase A: noisy LNs -> nt2 builds -> all q convs ------------
            lnpN_cm = tc.tile_pool(name="lnpN", bufs=1)
            lnpN = lnpN_cm.__enter__()
            lnall = [lnpN.tile([P, D], BF16, name=f"lnall_{i}")
                     for i in range(6)]  # noisy ln2 tiles
            for r in range(6):
                xt = lnio.tile([P, D], F32, tag="ln_in", bufs=2)
                nc.sync.dma_start(xt, dt["noisyH"][r * P:(r + 1) * P, :])
                if 1 <= r <= 4:
                    ln1 = xres[:, r - 1, :]
                else:
                    ln1 = lnio.tile([P, D], F32, tag="ln1_tmp", bufs=1)
                _ln_apply(nc, small, xt, ln1, eps1_t,
                          affine=(lng_s, lnb_s))
                _ln_apply(nc, small, ln1, lnall[r], eps2_t)

            nt2s, ct2s = [], []
            cp_eng = [nc.vector, nc.gpsimd]
            for m in range(8):
                nt2m = bigc.tile([P, 2, NT2W], BF16, name=f"nt2_{m}")
                nt2s.append(nt2m)
                tmn = trans.tile([P, NHW], BF16, tag="tmn", bufs=2)
                for r in range(6):
                    pt = ptp.tile([P, P], BF16, tag="tpbf")
                    nc.tensor.transpose(pt, lnall[r][:, m * P:(m + 1) * P],
                                        ident)
                    dst = tmn[:, r * P:(r + 1) * P]
                    if r % 2 == 0:
                        nc.vector.tensor_scalar(dst, pt, sc_msa[:, m:m + 1],
                                                sh_msa[:, m:m + 1],
                                                ALU.mult, ALU.add)
                    else:
                        nc.scalar.activation(dst, pt, AF.Identity,
                                             bias=sh_msa[:, m:m + 1],
                                             scale=sc_msa[:, m:m + 1])
                for hh in range(2):
                    sl = slice(hh * DK, (hh + 1) * DK)
                    e0, e1 = cp_eng[hh], cp_eng[1 - hh]
                    e0.tensor_tensor(nt2m[0:DK, hh, 0:524], tmn[sl, 122:646],
                                     hm_s[sl, 122:646], ALU.mult)
                    e1.tensor_tensor(nt2m[DK:P, hh, 0:524], tmn[sl, 123:647],
                                     hm_s[sl, 123:647], ALU.mult)

            lnpN_cm.__exit__(None, None, None)
            hpool_cm = tc.tile_pool(name="hpool", bufs=2)
            hpool = hpool_cm.__enter__()

            def conv(h, wname, bias_s, x2, nchunk, name, bufs=2):
                hp, hc = h % 2, h // 2
                wsb = wpool.tile([P, 7, DK], BF16, tag="convw", bufs=4,
                                 name=f"w_{name}")
                nc.sync.dma_start(wsb, dt[wname][:, h])
                outT = hpool.tile([DK, nchunk * TQ], BF16, tag=f"cv_{name}",
                                  bufs=bufs, name=f"cv_{name}_{h}")
                for c in range(nchunk):
                    ps = psm.tile([DK, TQ], F32, tag="conv")
                    for j in range(7):
                        nc.tensor.matmul(
                            ps, wsb[:, j, :],
                            x2[:, hp, c * TQ + 2 * j:c * TQ + 2 * j + TQ],
                            start=(j == 0), stop=(j == 6))
                    if (h + c) % 2 == 0:
                        nc.vector.tensor_scalar_add(
                            outT[:, c * TQ:(c + 1) * TQ], ps,
                            bias_s[hp * DK:(hp + 1) * DK, hc:hc + 1])
                    else:
                        nc.scalar.activation(
                            outT[:, c * TQ:(c + 1) * TQ], ps, AF.Identity,
                            bias=bias_s[hp * DK:(hp + 1) * DK, hc:hc + 1])
                return outT

            qTs = [conv(h, "wql", bq_s, nt2s[h // 2], 1, f"q{h}", bufs=1)
                   for h in range(H)]

            # ---- Phase B: clean LNs -> ct2 builds ---------------------------
            lnpC_cm = tc.tile_pool(name="lnpC", bufs=1)
            lnpC = lnpC_cm.__enter__()
            clnall = [lnpC.tile([P, D], BF16, name=f"clnall_{i}")
                      for i in range(8)]
            for r in range(8):
                xt = lnio.tile([P, D], F32, tag="ln_in", bufs=2)
                nc.sync.dma_start(xt, dt["clean"][r * P:(r + 1) * P, :])
                _ln_apply(nc, small, xt, clnall[r], eps1_t)
            for m in range(8):
                ct2m = bigc.tile([P, 2, CT2W], BF16, name=f"ct2_{m}")
                ct2s.append(ct2m)
                for hh in range(2):
                    nc.gpsimd.memset(ct2m[0:DK, hh, 0:6], 0.0)
                    nc.gpsimd.memset(ct2m[0:DK, hh, 1030:CT2W], 0.0)
                    nc.gpsimd.memset(ct2m[DK:P, hh, 0:5], 0.0)
                    nc.gpsimd.memset(ct2m[DK:P, hh, 1029:CT2W], 0.0)
                tmc = trans.tile([P, T], BF16, tag="tmc", bufs=2)
                for r in range(8):
                    pt = ptp.tile([P, P], BF16, tag="tpbf")
                    nc.tensor.transpose(pt, clnall[r][:, m * P:(m + 1) * P],
                                        ident)
                    dst = tmc[:, r * P:(r + 1) * P]
                    if r % 2 == 0:
                        nc.vector.tensor_scalar(dst, pt, clng_s[:, m:m + 1],
                                                clnb_s[:, m:m + 1],
                                                ALU.mult, ALU.add)
                    else:
                        nc.scalar.activation(dst, pt, AF.Identity,
                                             bias=clnb_s[:, m:m + 1],
                                             scale=clng_s[:, m:m + 1])
                for hh in range(2):
                    sl = slice(hh * DK, (hh + 1) * DK)
                    e0, e1 = cp_eng[hh], cp_eng[1 - hh]
                    e0.tensor_copy(ct2m[0:DK, hh, 6:1030], tmc[sl, :])
                    e1.tensor_copy(ct2m[DK:P, hh, 5:1029], tmc[sl, :])
            lnpC_cm.__exit__(None, None, None)

            # ---- Phase C: per-head conv K/V + cross attention ---------------
            for h in range(H):
                hp = h % 2
                hc = h // 2
                ct2 = ct2s[hc]
                kT = conv(h, "wkl", bk_s, ct2, 2, "k")
                vT = conv(h, "wvl", bv_s, ct2, 2, "v")
                qT = qTs[h]

                # v65: v tokens-on-partitions plus ones column for row sums
                v65 = hpool.tile([P, 8, 66], BF16, tag="v65", bufs=1)
                nc.vector.memset(v65[:, :, 64:65], 1.0)
                for c in range(8):
                    pt = ptp.tile([P, P], BF16, tag="tpbf")
                    nc.tensor.transpose(pt[:, :DK], vT[:, c * P:(c + 1) * P],
                                        ident[:DK, :DK])
                    nc.vector.tensor_copy(v65[:, c, 0:DK], pt[:, :DK])

                # transposed scores; fused mask/scale/exp (T2 on partitions)
                pT = hpool.tile([P, 8, TQ], BF16, tag="pT", bufs=2)
                for c in range(8):
                    ps = psc.tile([P, TQ], F32, tag="sc")
                    nc.tensor.matmul(ps, kT[:, c * P:(c + 1) * P], qT,
                                     start=True, stop=True)
                    nc.scalar.activation(pT[:, c, :], ps, AF.Exp,
                                         bias=maskb_s[:, c:c + 1], scale=0.125)

                # PV: out[65, TQ] = [v|1]^T @ p (row 64 = softmax denominator)
                pv = ppv.tile([P, TQ], F32, tag="pv")
                for c in range(8):
                    nc.tensor.matmul(pv[:65, :], v65[:, c, 0:65], pT[:, c, :],
                                     start=(c == 0), stop=(c == 7))
                linv = trans.tile([1, TQ], F32, tag="linv")
                nc.vector.reciprocal(linv, pv[64:65, :])
                bc_sb = trans.tile([DK, TQ], F32, tag="bcsb", bufs=2)
                nc.gpsimd.partition_broadcast(bc_sb, linv)
                nc.vector.tensor_tensor(attnT[hp * DK:(hp + 1) * DK, hc, :],
                                        pv[0:DK, :], bc_sb, ALU.mult)
            hpool_cm.__exit__(None, None, None)

        # ---- Phase D: fc projection + gate + residual into xres -------------
        fcgs = []
        for m in range(8):
            wt = wpool.tile([P, 8, P], BF16, tag="wt")
            nc.sync.dma_start(wt, dt["fcw"][m])
            ps = psc.tile([P, TQ], F32, tag="sc")
            for k in range(8):
                nc.tensor.matmul(ps, wt[:, k, :], attnT[:, k, :],
                                 start=(k == 0), stop=(k == 7))
            fcg = trans.tile([P, TQ], BF16, tag="fcg", bufs=8,
                             name=f"fcg_{m}")
            fcbg = small.tile([P, 1], F32, tag="fcbg", name="fcbg")
            nc.vector.tensor_tensor(fcbg, fcb_s[:, m:m + 1],
                                    g_msa[:, m:m + 1], ALU.mult)
            nc.scalar.activation(fcg, ps, AF.Identity, bias=fcbg,
                                 scale=g_msa[:, m:m + 1])
            fcgs.append(fcg)
        for j in range(4):
            for m in range(8):
                pt = ptp.tile([P, P], BF16, tag="tpbf")
                nc.tensor.transpose(pt, fcgs[m][:, j * P:(j + 1) * P], ident)
                nc.vector.tensor_tensor(xres[:, j, m * P:(m + 1) * P], pt,
                                        xres[:, j, m * P:(m + 1) * P], ALU.add)

        # ---- Phase E: LN3 + mlp modulation -> n2T ---------------------------
        bigf_cm = tc.tile_pool(name="bigf", bufs=1)
        bigf = bigf_cm.__enter__()
        n2T = bigf.tile([P, 8, TQ], BF16)
        for s in range(4):
            l3 = lnio.tile([P, D], BF16, tag="ln2b")
            _ln_apply(nc, small, xres[:, s, :], l3, eps2_t)
            for m in range(8):
                pt = ptp.tile([P, P], BF16, tag="tpbf")
                nc.tensor.transpose(pt, l3[:, m * P:(m + 1) * P], ident)
                nc.vector.tensor_scalar(n2T[:, m, s * P:(s + 1) * P], pt,
                                        sc_mlp[:, m:m + 1], sh_mlp[:, m:m + 1],
                                        ALU.mult, ALU.add)

        # ---- Phase F: FFN (single pass; SBUF freed by bigc/hpool exit) ------
        if True:
            ffa = bigf.tile([P, 32, TQ], BF16)
            for m in range(32):
                wt = wpool.tile([P, 8, P], BF16, tag="wt")
                nc.sync.dma_start(wt, dt["w1t"][m])
                ps = psc.tile([P, TQ], F32, tag="sc")
                for k in range(8):
                    nc.tensor.matmul(ps, wt[:, k, :], n2T[:, k, :],
                                     start=(k == 0), stop=(k == 7))
                nc.scalar.activation(ffa[:, m, :], ps, AF.Gelu_apprx_tanh,
                                     bias=fb1_s[:, m:m + 1])
            for m in range(8):
                ps = psc.tile([P, TQ], F32, tag="sc")
                for kq in range(4):
                    wt = wpool.tile([P, 8, P], BF16, tag="wt")
                    nc.sync.dma_start(wt, dt["w2t"][m, kq])
                    for k8 in range(8):
                        k = kq * 8 + k8
                        nc.tensor.matmul(ps, wt[:, k8, :], ffa[:, k, :],
                                         start=(k == 0), stop=(k == 31))
                ffog = trans.tile([P, TQ], BF16, tag="ffog", bufs=2)
                fbg = small.tile([P, 1], F32, tag="fcbg", name="fbg")
                nc.vector.tensor_tensor(fbg, fb2_s[:, m:m + 1],
                                        g_mlp[:, m:m + 1], ALU.mult)
                nc.scalar.activation(ffog, ps, AF.Identity, bias=fbg,
                                     scale=g_mlp[:, m:m + 1])
                for j in range(4):
                    pt = ptp.tile([P, P], BF16, tag="tpbf")
                    nc.tensor.transpose(pt, ffog[:, j * P:(j + 1) * P], ident)
                    nc.vector.tensor_tensor(xres[:, j, m * P:(m + 1) * P], pt,
                                            xres[:, j, m * P:(m + 1) * P],
                                            ALU.add)
        bigf_cm.__exit__(None, None, None)

        for s in range(4):
            nc.sync.dma_start(out_ap[s * P:(s + 1) * P, :], xres[:, s, :])


# --------------------------- host side --------------------------------------
_NC_CACHE = None


def _prep_conv_w(w):
    # w: (D, DK, KW) grouped conv weights -> [128, H, 7, DK] bf16 tap-pair lhsT
    wr = w.reshape(H, DK, DK, KW)                      # [h, m, c, tap]
    arr = np.zeros((P, H, 7, DK), np.float32)
    arr[0:DK] = wr[:, :, :, 0::2].transpose(2, 0, 3, 1)      # taps 0,2,..,12
    arr[DK:P, :, 0:6] = wr[:, :, :, 1::2].transpose(2, 0, 3, 1)
    return arr.astype(ml_dtypes.bfloat16)


def kernel(**inputs):
    global _NC_CACHE
    if _NC_CACHE is None:
        _NC_CACHE = build_nc()
    nc = _NC_CACHE

    f32 = np.float32
    bf = ml_dtypes.bfloat16
    noisy = np.asarray(inputs["noisy_feats"], f32)
    clean = np.asarray(inputs["clean_feats"], f32)
    t = np.asarray(inputs["t"], f32)
    clean_len = np.asarray(inputs["clean_lengths"]).astype(np.int64)

    # AdaLayerNormZero on host (0.02% of FLOPs): emb = silu(t) @ ada_w.T + b
    st = t * (1.0 / (1.0 + np.exp(-t, dtype=f32)))
    emb = st @ np.asarray(inputs["ada_w"], f32).T + np.asarray(inputs["ada_b"], f32)
    sh_msa, sc_msa, g_msa, sh_mlp, sc_mlp, g_mlp = np.split(emb, 6, axis=1)

    wql = _prep_conv_w(np.asarray(inputs["wq"], f32))
    wkl = _prep_conv_w(np.asarray(inputs["wk"], f32))
    wvl = _prep_conv_w(np.asarray(inputs["wv"], f32))
    fcw = np.asarray(inputs["fc_w"], f32).T.reshape(8, P, 8, P) \
        .transpose(2, 1, 0, 3).astype(bf).copy()
    w1t = np.asarray(inputs["ff_w1"], f32).T.reshape(8, P, 32, P) \
        .transpose(2, 1, 0, 3).astype(bf).copy()
    w2t = np.asarray(inputs["ff_w2"], f32).T.reshape(32, P, 8, P) \
        .transpose(2, 0, 1, 3).reshape(8, 4, 8, P, P) \
        .transpose(0, 1, 3, 2, 4).astype(bf).copy()

    common = dict(
        lng=np.broadcast_to(np.asarray(inputs["ln_noisy_g"], f32), (P, D)).copy(),
        lnb=np.broadcast_to(np.asarray(inputs["ln_noisy_b"], f32), (P, D)).copy(),
        clng=np.asarray(inputs["ln_clean_g"], f32).copy(),
        clnb=np.asarray(inputs["ln_clean_b"], f32).copy(),
        wql=wql, wkl=wkl, wvl=wvl,
        bq=np.asarray(inputs["bq"], f32).copy(),
        bk=np.asarray(inputs["bk"], f32).copy(),
        bv=np.asarray(inputs["bv"], f32).copy(),
        fcw=fcw, fcb=np.asarray(inputs["fc_b"], f32).copy(),
        w1t=w1t, fb1=np.asarray(inputs["ff_b1"], f32).copy(),
        w2t=w2t, fb2=np.asarray(inputs["ff_b2"], f32).copy(),
    )

    in_maps = []
    for i in range(8):
        b, half = i // 2, i % 2
        t0 = half * TQ
        noisyH = np.zeros((NHW, D), f32)
        lo, hi = t0 - P, t0 + 640
        clo, chi = max(lo, 0), min(hi, T)
        noisyH[clo - lo:chi - lo] = noisy[b, clo:chi]
        hm = np.zeros((NHW,), f32)
        hm[clo - lo:chi - lo] = 1.0
        maskb = np.where(np.arange(T) >= clean_len[b], NEG, 0.0).astype(f32)
        mod = np.stack([sh_msa[b], 1.0 + sc_msa[b], g_msa[b],
                        sh_mlp[b], 1.0 + sc_mlp[b], g_mlp[b]]).astype(f32)
        m = dict(common)
        m.update(noisyH=noisyH, clean=clean[b].copy(),
                 hm=np.broadcast_to(hm, (P, NHW)).astype(bf).copy(),
                 maskb=maskb, mod=mod)
        in_maps.append(m)

    global _LAST_INMAPS
    _LAST_INMAPS = in_maps
    res = run_bass_kernel_spmd(nc, in_maps, core_ids=list(range(8)))
    out = np.empty((B, T, D), f32)
    for i in range(8):
        b, half = i // 2, i % 2
        out[b, half * TQ:(half + 1) * TQ] = res.results[i]["out"]
    return out


_LAST_INMAPS = None


def run_profiled(tmpdir=None):
    """Re-run the last kernel invocation with NTFF tracing; return exec ns."""
    if _NC_CACHE is None or _LAST_INMAPS is None:
        return None
    res = run_bass_kernel_spmd(_NC_CACHE, _LAST_INMAPS,
                               core_ids=list(range(8)), trace=True,
                               tmpdir=tmpdir)
    return res.exec_time_ns


if __name__ == "__main__":
    build_nc()
    print("build ok")



# revision 15
# speedup vs baseline: 1.0147x; 1.0147x over previous
# Trainium2 Bass kernel for nn_CrossAttention_noise (B=4, T1=T2=1024, D=1024,
# H=16, DK=64, K=13, FF=4096), SPMD over 8 NeuronCores.
#
# Sharding: core i handles batch b=i//2 and query-token half t0=(i%2)*512.
# Each core computes its 512 output tokens end-to-end (the K/V convolution
# over the full clean sequence is duplicated between the two cores of a
# batch; no collectives).  Big matmuls run in bf16 with fp32 PSUM
# accumulation; layernorms / softmax / residuals stay fp32.
#
# Layout convention: "T" suffix = channels on partitions, tokens on the free
# dim (the natural matmul layout here); plain tiles = tokens on partitions.
import numpy as np
import ml_dtypes
from contextlib import ExitStack

import concourse.bass as bass
import concourse.mybir as mybir
import concourse.tile as tile
from concourse import bacc
from concourse.bass_utils import run_bass_kernel_spmd
from concourse.masks import make_identity

BF16 = mybir.dt.bfloat16
F32 = mybir.dt.float32
FP8 = mybir.dt.float8e4
AF = mybir.ActivationFunctionType
ALU = mybir.AluOpType
AX = mybir.AxisListType
DR = mybir.MatmulPerfMode.DoubleRow
WSC = 32.0            # fp8 weight prescale (fc_w, ff_w1, ff_w2 stored *32)

B, T, D, H, DK, KW, FF = 4, 1024, 1024, 16, 64, 13, 4096
TQ = 512          # query tokens per core
NHW = 768         # noisy halo window rows (zero-padded on host)
NT2W = 528        # nt2 width (valid cols 0..523)
CT2W = 1040       # ct2 width (valid cols 0..1035)
P = 128
EPS1, EPS2 = 1e-5, 1e-6
NEG = -1.0e30


def _ln_apply(nc, pool, x, out, eps_ap, affine=None):
    """out = (x - mean)/sqrt(var + eps) rowwise; x [p, D] f32 in SBUF.

    Heavy passes run on the scalar engine (accum_out reductions + fused
    scale/bias apply); DVE only does tiny [p,1] ops."""
    p = x.shape[0]
    s = pool.tile([P, 1], F32, tag="ln_s", name="ln_s")[:p]
    sq = pool.tile([P, 1], F32, tag="ln_sq", name="ln_sq")[:p]
    scr = pool.tile([P, D], BF16, tag="ln_scr", name="ln_scr", bufs=2)[:p]
    nc.vector.reduce_sum(s, x, axis=AX.X)
    nc.scalar.activation(scr, x, AF.Square, accum_out=sq)
    mu = pool.tile([P, 1], F32, tag="ln_mu", name="ln_mu")[:p]
    nc.vector.tensor_scalar_mul(mu, s, 1.0 / D)
    musq = pool.tile([P, 1], F32, tag="ln_musq", name="ln_musq")[:p]
    nc.vector.tensor_tensor(musq, mu, mu, ALU.mult)
    var = pool.tile([P, 1], F32, tag="ln_var", name="ln_var")[:p]
    nc.vector.tensor_scalar(var, sq, 1.0 / D, musq, ALU.mult, ALU.subtract)
    std = pool.tile([P, 1], F32, tag="ln_std", name="ln_std")[:p]
    nc.scalar.activation(std, var, AF.Sqrt, bias=eps_ap[:p])
    rstd = pool.tile([P, 1], F32, tag="ln_rstd", name="ln_rstd")[:p]
    nc.vector.reciprocal(rstd, std)
    beta = pool.tile([P, 1], F32, tag="ln_beta", name="ln_beta")[:p]
    nc.vector.tensor_tensor(beta, mu, rstd, ALU.mult)
    nc.vector.tensor_scalar_mul(beta, beta, -1.0)
    if affine is not None:
        g, b = affine
        negmu = pool.tile([P, 1], F32, tag="ln_negmu", name="ln_negmu")[:p]
        nc.vector.tensor_scalar_mul(negmu, mu, -1.0)
        nc.vector.scalar_tensor_tensor(out, x, negmu, g, ALU.add, ALU.mult)
        nc.vector.scalar_tensor_tensor(out, out, rstd, b, ALU.mult, ALU.add)
    else:
        nc.vector.tensor_scalar(out, x, rstd, beta, ALU.mult, ALU.add)


def _nt2_rng(r, shift, width):
    """Dest/src col ranges for copying transpose block r into a shifted row."""
    lo = r * P - shift
    hi = lo + P
    d0, d1 = max(lo, 0), min(hi, width)
    if d1 <= d0:
        return None
    return d0, d1, d0 - lo


def build_nc():
    nc = bacc.Bacc("TRN2", target_bir_lowering=False, debug=False,
                   num_devices=8)
    dt = {}

    def din(name, shape, dtype):
        dt[name] = nc.dram_tensor(name, list(shape), dtype,
                                  kind="ExternalInput").ap()

    din("noisyH", (NHW, D), F32)          # rows [t0-128, t0+640), zero padded
    din("clean", (T, D), F32)
    din("hm", (P, NHW), BF16)              # halo-token validity (rows equal)
    din("maskb", (T,), F32)               # 0 / -1e30 additive key mask
    din("mod", (8, D), F32)               # sh_msa,1+sc_msa,g_msa,sh_mlp,1+sc_mlp,g_mlp,g_msa/WSC,g_mlp/WSC
    din("lng", (P, D), F32)               # ln_noisy_g broadcast to 128 rows
    din("lnb", (P, D), F32)
    din("clng", (D,), F32)
    din("clnb", (D,), F32)
    din("wql", (P, H, 7, DK), BF16)
    din("wkl", (P, H, 7, DK), BF16)
    din("wvl", (P, H, 7, DK), BF16)
    din("bq", (D,), F32)
    din("bk", (D,), F32)
    din("bv", (D,), F32)
    din("fcw", (8, P, 8, P), FP8)         # fc_w.T*WSC tiles [mc][kp][ko][mj]
    din("fcb", (D,), F32)
    din("w1t", (32, P, 8, P), BF16)       # ff_w1.T tiles [mc][kp][ko][mj]
    din("fb1", (FF,), F32)
    din("w2t", (8, 4, P, 8, P), BF16)     # ff_w2.T tiles [mc][kq][kp][k8][mj]
    din("fb2", (D,), F32)
    out_ap = nc.dram_tensor("out", [TQ, D], F32, kind="ExternalOutput").ap()

    with tile.TileContext(nc) as tc:
        _emit(tc, dt, out_ap)
    nc.compile()
    return nc


def _emit(tc, dt, out_ap):
    nc = tc.nc
    with ExitStack() as ctx:
        const = ctx.enter_context(tc.tile_pool(name="const", bufs=1))
        small = ctx.enter_context(tc.tile_pool(name="small", bufs=3))
        lnio = ctx.enter_context(tc.tile_pool(name="lnio", bufs=3))
        big = ctx.enter_context(tc.tile_pool(name="bigsb", bufs=1))
        trans = ctx.enter_context(tc.tile_pool(name="trans", bufs=3))
        wpool = ctx.enter_context(tc.tile_pool(name="wstream", bufs=6))
        psc = ctx.enter_context(tc.tile_pool(name="psc", bufs=2, space="PSUM"))
        ppv = ctx.enter_context(tc.tile_pool(name="ppv", bufs=1, space="PSUM"))
        ptp = ctx.enter_context(tc.tile_pool(name="ptp", bufs=3, space="PSUM"))
        psm = ctx.enter_context(tc.tile_pool(name="psm", bufs=2, space="PSUM"))

        ident = const.tile([P, P], BF16)
        make_identity(nc, ident)
        eps1_t = const.tile([P, 1], F32)
        nc.vector.memset(eps1_t, EPS1)
        eps2_t = const.tile([P, 1], F32)
        nc.vector.memset(eps2_t, EPS2)

        def chanvec(name, w=8):
            t = const.tile([P, w], F32, tag=f"cv_{name}")
            nc.sync.dma_start(t, dt[name].rearrange("(m p) -> p m", p=P))
            return t

        bq_s, bk_s, bv_s = chanvec("bq"), chanvec("bk"), chanvec("bv")
        fcb_s, fb2_s = chanvec("fcb"), chanvec("fb2")
        clng_s, clnb_s = chanvec("clng"), chanvec("clnb")
        maskb_s = chanvec("maskb")
        fb1_s = chanvec("fb1", 32)
        mod_s = const.tile([P, 8, 8], F32)
        for s in range(8):
            nc.sync.dma_start(mod_s[:, s, :],
                              dt["mod"][s].rearrange("(m p) -> p m", p=P))
        sh_msa, sc_msa, g_msa = mod_s[:, 0, :], mod_s[:, 1, :], mod_s[:, 2, :]
        sh_mlp, sc_mlp, g_mlp = mod_s[:, 3, :], mod_s[:, 4, :], mod_s[:, 5, :]
        g_msa_ds, g_mlp_ds = mod_s[:, 6, :], mod_s[:, 7, :]
        hm_s = const.tile([P, NHW], BF16)
        nc.sync.dma_start(hm_s, dt["hm"])
        lng_s = const.tile([P, D], F32)
        nc.sync.dma_start(lng_s, dt["lng"])
        lnb_s = const.tile([P, D], F32)
        nc.sync.dma_start(lnb_s, dt["lnb"])

        xres = big.tile([P, 4, D], F32)        # LN1 rows [t0, t0+512); later x
        attnT = big.tile([P, 8, TQ], FP8)      # concat_h(out_h/l_h), chan-major

        with tc.tile_pool(name="bigc", bufs=1) as bigc:
            # ---- Phase A: noisy LNs -> nt2 builds -> all q convs ------------
            lnpN_cm = tc.tile_pool(name="lnpN", bufs=1)
            lnpN = lnpN_cm.__enter__()
            lnall = [lnpN.tile([P, D], BF16, name=f"lnall_{i}")
                     for i in range(6)]  # noisy ln2 tiles
            for r in range(6):
                xt = lnio.tile([P, D], F32, tag="ln_in", bufs=2)
                nc.sync.dma_start(xt, dt["noisyH"][r * P:(r + 1) * P, :])
                if 1 <= r <= 4:
                    ln1 = xres[:, r - 1, :]
                else:
                    ln1 = lnio.tile([P, D], F32, tag="ln1_tmp", bufs=1)
                _ln_apply(nc, small, xt, ln1, eps1_t,
                          affine=(lng_s, lnb_s))
                _ln_apply(nc, small, ln1, lnall[r], eps2_t)

            nt2s, ct2s = [], []
            cp_eng = [nc.vector, nc.gpsimd]
            for m in range(8):
                nt2m = bigc.tile([P, 2, NT2W], BF16, name=f"nt2_{m}")
                nt2s.append(nt2m)
                tmn = trans.tile([P, NHW], BF16, tag="tmn", bufs=2)
                for r in range(6):
                    pt = ptp.tile([P, P], BF16, tag="tpbf")
                    nc.tensor.transpose(pt, lnall[r][:, m * P:(m + 1) * P],
                                        ident)
                    dst = tmn[:, r * P:(r + 1) * P]
                    if r % 2 == 0:
                        nc.vector.tensor_scalar(dst, pt, sc_msa[:, m:m + 1],
                                                sh_msa[:, m:m + 1],
                                                ALU.mult, ALU.add)
                    else:
                        nc.scalar.activation(dst, pt, AF.Identity,
                                             bias=sh_msa[:, m:m + 1],
                                             scale=sc_msa[:, m:m + 1])
                for hh in range(2):
                    sl = slice(hh * DK, (hh + 1) * DK)
                    e0, e1 = cp_eng[hh], cp_eng[1 - hh]
                    e0.tensor_tensor(nt2m[0:DK, hh, 0:524], tmn[sl, 122:646],
                                     hm_s[sl, 122:646], ALU.mult)
                    e1.tensor_tensor(nt2m[DK:P, hh, 0:524], tmn[sl, 123:647],
                                     hm_s[sl, 123:647], ALU.mult)

            lnpN_cm.__exit__(None, None, None)
            hpool_cm = tc.tile_pool(name="hpool", bufs=2)
            hpool = hpool_cm.__enter__()

            def conv(h, wname, bias_s, x2, nchunk, name, bufs=2):
                hp, hc = h % 2, h // 2
                wsb = wpool.tile([P, 7, DK], BF16, tag="convw", bufs=4,
                                 name=f"w_{name}")
                nc.sync.dma_start(wsb, dt[wname][:, h])
                outT = hpool.tile([DK, nchunk * TQ], BF16, tag=f"cv_{name}",
                                  bufs=bufs, name=f"cv_{name}_{h}")
                for c in range(nchunk):
                    ps = psm.tile([DK, TQ], F32, tag="conv")
                    for j in range(7):
                        nc.tensor.matmul(
                            ps, wsb[:, j, :],
                            x2[:, hp, c * TQ + 2 * j:c * TQ + 2 * j + TQ],
                            start=(j == 0), stop=(j == 6))
                    if (h + c) % 2 == 0:
                        nc.vector.tensor_scalar_add(
                            outT[:, c * TQ:(c + 1) * TQ], ps,
                            bias_s[hp * DK:(hp + 1) * DK, hc:hc + 1])
                    else:
                        nc.scalar.activation(
                            outT[:, c * TQ:(c + 1) * TQ], ps, AF.Identity,
                            bias=bias_s[hp * DK:(hp + 1) * DK, hc:hc + 1])
                return outT

            qTs = [conv(h, "wql", bq_s, nt2s[h // 2], 1, f"q{h}", bufs=1)
                   for h in range(H)]

            # ---- Phase B: clean LNs -> ct2 builds ---------------------------
            lnpC_cm = tc.tile_pool(name="lnpC", bufs=1)
            lnpC = lnpC_cm.__enter__()
            clnall = [lnpC.tile([P, D], BF16, name=f"clnall_{i}")
                      for i in range(8)]
            for r in range(8):
                xt = lnio.tile([P, D], F32, tag="ln_in", bufs=2)
                nc.sync.dma_start(xt, dt["clean"][r * P:(r + 1) * P, :])
                _ln_apply(nc, small, xt, clnall[r], eps1_t)
            for m in range(8):
                ct2m = bigc.tile([P, 2, CT2W], BF16, name=f"ct2_{m}")
                ct2s.append(ct2m)
                for hh in range(2):
                    nc.gpsimd.memset(ct2m[0:DK, hh, 0:6], 0.0)
                    nc.gpsimd.memset(ct2m[0:DK, hh, 1030:CT2W], 0.0)
                    nc.gpsimd.memset(ct2m[DK:P, hh, 0:5], 0.0)
                    nc.gpsimd.memset(ct2m[DK:P, hh, 1029:CT2W], 0.0)
                tmc = trans.tile([P, T], BF16, tag="tmc", bufs=2)
                for r in range(8):
                    pt = ptp.tile([P, P], BF16, tag="tpbf")
                    nc.tensor.transpose(pt, clnall[r][:, m * P:(m + 1) * P],
                                        ident)
                    dst = tmc[:, r * P:(r + 1) * P]
                    if r % 2 == 0:
                        nc.vector.tensor_scalar(dst, pt, clng_s[:, m:m + 1],
                                                clnb_s[:, m:m + 1],
                                                ALU.mult, ALU.add)
                    else:
                        nc.scalar.activation(dst, pt, AF.Identity,
                                             bias=clnb_s[:, m:m + 1],
                                             scale=clng_s[:, m:m + 1])
                for hh in range(2):
                    sl = slice(hh * DK, (hh + 1) * DK)
                    e0, e1 = cp_eng[hh], cp_eng[1 - hh]
                    e0.tensor_copy(ct2m[0:DK, hh, 6:1030], tmc[sl, :])
                    e1.tensor_copy(ct2m[DK:P, hh, 5:1029], tmc[sl, :])
            lnpC_cm.__exit__(None, None, None)

            # ---- Phase C: per-head conv K/V + cross attention ---------------
            for h in range(H):
                hp = h % 2
                hc = h // 2
                ct2 = ct2s[hc]
                kT = conv(h, "wkl", bk_s, ct2, 2, "k")
                vT = conv(h, "wvl", bv_s, ct2, 2, "v")
                qT = qTs[h]

                # v65: v tokens-on-partitions plus ones column for row sums
                v65 = hpool.tile([P, 8, 66], BF16, tag="v65", bufs=1)
                nc.vector.memset(v65[:, :, 64:65], 1.0)
                for c in range(8):
                    pt = ptp.tile([P, P], BF16, tag="tpbf")
                    nc.tensor.transpose(pt[:, :DK], vT[:, c * P:(c + 1) * P],
                                        ident[:DK, :DK])
                    nc.vector.tensor_copy(v65[:, c, 0:DK], pt[:, :DK])

                # transposed scores; fused mask/scale/exp (T2 on partitions)
                pT = hpool.tile([P, 8, TQ], BF16, tag="pT", bufs=2)
                for c in range(8):
                    ps = psc.tile([P, TQ], F32, tag="sc")
                    nc.tensor.matmul(ps, kT[:, c * P:(c + 1) * P], qT,
                                     start=True, stop=True)
                    nc.scalar.activation(pT[:, c, :], ps, AF.Exp,
                                         bias=maskb_s[:, c:c + 1], scale=0.125)

                # PV: out[65, TQ] = [v|1]^T @ p (row 64 = softmax denominator)
                pv = ppv.tile([P, TQ], F32, tag="pv")
                for c in range(8):
                    nc.tensor.matmul(pv[:65, :], v65[:, c, 0:65], pT[:, c, :],
                                     start=(c == 0), stop=(c == 7))
                linv = trans.tile([1, TQ], F32, tag="linv")
                nc.vector.reciprocal(linv, pv[64:65, :])
                bc_sb = trans.tile([DK, TQ], F32, tag="bcsb", bufs=2)
                nc.gpsimd.partition_broadcast(bc_sb, linv)
                nc.vector.tensor_tensor(attnT[hp * DK:(hp + 1) * DK, hc, :],
                                        pv[0:DK, :], bc_sb, ALU.mult)
            hpool_cm.__exit__(None, None, None)

        # ---- Phase D: fc projection + gate + residual into xres -------------
        fcgs = []
        for m in range(8):
            wt = wpool.tile([P, 8, P], FP8, tag="wt")
            nc.sync.dma_start(wt, dt["fcw"][m])
            ps = psc.tile([P, TQ], F32, tag="sc")
            for k in range(4):
                nc.tensor.matmul(ps, wt[:, 2 * k:2 * k + 2, :],
                                 attnT[:, 2 * k:2 * k + 2, :],
                                 start=(k == 0), stop=(k == 3), perf_mode=DR)
            fcg = trans.tile([P, TQ], BF16, tag="fcg", bufs=8,
                             name=f"fcg_{m}")
            fcbg = small.tile([P, 1], F32, tag="fcbg", name="fcbg")
            nc.vector.tensor_tensor(fcbg, fcb_s[:, m:m + 1],
                                    g_msa[:, m:m + 1], ALU.mult)
            nc.scalar.activation(fcg, ps, AF.Identity, bias=fcbg,
                                 scale=g_msa_ds[:, m:m + 1])
            fcgs.append(fcg)
        for j in range(4):
            for m in range(8):
                pt = ptp.tile([P, P], BF16, tag="tpbf")
                nc.tensor.transpose(pt, fcgs[m][:, j * P:(j + 1) * P], ident)
                nc.vector.tensor_tensor(xres[:, j, m * P:(m + 1) * P], pt,
                                        xres[:, j, m * P:(m + 1) * P], ALU.add)

        # ---- Phase E: LN3 + mlp modulation -> n2T ---------------------------
        bigf_cm = tc.tile_pool(name="bigf", bufs=1)
        bigf = bigf_cm.__enter__()
        n2T = bigf.tile([P, 8, TQ], BF16)
        for s in range(4):
            l3 = lnio.tile([P, D], BF16, tag="ln2b")
            _ln_apply(nc, small, xres[:, s, :], l3, eps2_t)
            for m in range(8):
                pt = ptp.tile([P, P], BF16, tag="tpbf")
                nc.tensor.transpose(pt, l3[:, m * P:(m + 1) * P], ident)
                nc.vector.tensor_scalar(n2T[:, m, s * P:(s + 1) * P], pt,
                                        sc_mlp[:, m:m + 1], sh_mlp[:, m:m + 1],
                                        ALU.mult, ALU.add)

        # ---- Phase F: FFN (single pass; SBUF freed by bigc/hpool exit) ------
        if True:
            ffa = bigf.tile([P, 32, TQ], BF16)
            for m in range(32):
                wt = wpool.tile([P, 8, P], BF16, tag="wt")
                nc.sync.dma_start(wt, dt["w1t"][m])
                ps = psc.tile([P, TQ], F32, tag="sc")
                for k in range(8):
                    nc.tensor.matmul(ps, wt[:, k, :], n2T[:, k, :],
                                     start=(k == 0), stop=(k == 7))
                nc.scalar.activation(ffa[:, m, :], ps, AF.Gelu_apprx_tanh,
                                     bias=fb1_s[:, m:m + 1])
            for m in range(8):
                ps = psc.tile([P, TQ], F32, tag="sc")
                for kq in range(4):
                    wt = wpool.tile([P, 8, P], BF16, tag="wt")
                    nc.sync.dma_start(wt, dt["w2t"][m, kq])
                    for k8 in range(8):
                        k = kq * 8 + k8
                        nc.tensor.matmul(ps, wt[:, k8, :], ffa[:, k, :],
                                         start=(k == 0), stop=(k == 31))
                ffog = trans.tile([P, TQ], BF16, tag="ffog", bufs=2)
                fbg = small.tile([P, 1], F32, tag="fcbg", name="fbg")
                nc.vector.tensor_tensor(fbg, fb2_s[:, m:m + 1],
                                        g_mlp[:, m:m + 1], ALU.mult)
                nc.scalar.activation(ffog, ps, AF.Identity, bias=fbg,
                                     scale=g_mlp[:, m:m + 1])
                for j in range(4):
                    pt = ptp.tile([P, P], BF16, tag="tpbf")
                    nc.tensor.transpose(pt, ffog[:, j * P:(j + 1) * P], ident)
                    nc.vector.tensor_tensor(xres[:, j, m * P:(m + 1) * P], pt,
                                            xres[:, j, m * P:(m + 1) * P],
                                            ALU.add)
        bigf_cm.__exit__(None, None, None)

        for s in range(4):
            nc.sync.dma_start(out_ap[s * P:(s + 1) * P, :], xres[:, s, :])


# --------------------------- host side --------------------------------------
_NC_CACHE = None


def _prep_conv_w(w):
    # w: (D, DK, KW) grouped conv weights -> [128, H, 7, DK] bf16 tap-pair lhsT
    wr = w.reshape(H, DK, DK, KW)                      # [h, m, c, tap]
    arr = np.zeros((P, H, 7, DK), np.float32)
    arr[0:DK] = wr[:, :, :, 0::2].transpose(2, 0, 3, 1)      # taps 0,2,..,12
    arr[DK:P, :, 0:6] = wr[:, :, :, 1::2].transpose(2, 0, 3, 1)
    return arr.astype(ml_dtypes.bfloat16)


def kernel(**inputs):
    global _NC_CACHE
    if _NC_CACHE is None:
        _NC_CACHE = build_nc()
    nc = _NC_CACHE

    f32 = np.float32
    bf = ml_dtypes.bfloat16
    noisy = np.asarray(inputs["noisy_feats"], f32)
    clean = np.asarray(inputs["clean_feats"], f32)
    t = np.asarray(inputs["t"], f32)
    clean_len = np.asarray(inputs["clean_lengths"]).astype(np.int64)

    # AdaLayerNormZero on host (0.02% of FLOPs): emb = silu(t) @ ada_w.T + b
    st = t * (1.0 / (1.0 + np.exp(-t, dtype=f32)))
    emb = st @ np.asarray(inputs["ada_w"], f32).T + np.asarray(inputs["ada_b"], f32)
    sh_msa, sc_msa, g_msa, sh_mlp, sc_mlp, g_mlp = np.split(emb, 6, axis=1)

    fp8 = ml_dtypes.float8_e4m3fn
    wql = _prep_conv_w(np.asarray(inputs["wq"], f32))
    wkl = _prep_conv_w(np.asarray(inputs["wk"], f32))
    wvl = _prep_conv_w(np.asarray(inputs["wv"], f32))
    fcw = (np.asarray(inputs["fc_w"], f32).T * WSC).reshape(8, P, 8, P) \
        .transpose(2, 1, 0, 3).astype(fp8).copy()
    w1t = np.asarray(inputs["ff_w1"], f32).T.reshape(8, P, 32, P) \
        .transpose(2, 1, 0, 3).astype(bf).copy()
    w2t = np.asarray(inputs["ff_w2"], f32).T.reshape(32, P, 8, P) \
        .transpose(2, 0, 1, 3).reshape(8, 4, 8, P, P) \
        .transpose(0, 1, 3, 2, 4).astype(bf).copy()

    common = dict(
        lng=np.broadcast_to(np.asarray(inputs["ln_noisy_g"], f32), (P, D)).copy(),
        lnb=np.broadcast_to(np.asarray(inputs["ln_noisy_b"], f32), (P, D)).copy(),
        clng=np.asarray(inputs["ln_clean_g"], f32).copy(),
        clnb=np.asarray(inputs["ln_clean_b"], f32).copy(),
        wql=wql, wkl=wkl, wvl=wvl,
        bq=np.asarray(inputs["bq"], f32).copy(),
        bk=np.asarray(inputs["bk"], f32).copy(),
        bv=np.asarray(inputs["bv"], f32).copy(),
        fcw=fcw, fcb=np.asarray(inputs["fc_b"], f32).copy(),
        w1t=w1t, fb1=np.asarray(inputs["ff_b1"], f32).copy(),
        w2t=w2t, fb2=np.asarray(inputs["ff_b2"], f32).copy(),
    )

    in_maps = []
    for i in range(8):
        b, half = i // 2, i % 2
        t0 = half * TQ
        noisyH = np.zeros((NHW, D), f32)
        lo, hi = t0 - P, t0 + 640
        clo, chi = max(lo, 0), min(hi, T)
        noisyH[clo - lo:chi - lo] = noisy[b, clo:chi]
        hm = np.zeros((NHW,), f32)
        hm[clo - lo:chi - lo] = 1.0
        maskb = np.where(np.arange(T) >= clean_len[b], NEG, 0.0).astype(f32)
        mod = np.stack([sh_msa[b], 1.0 + sc_msa[b], g_msa[b],
                        sh_mlp[b], 1.0 + sc_mlp[b], g_mlp[b],
                        g_msa[b] / WSC, g_mlp[b] / WSC]).astype(f32)
        m = dict(common)
        m.update(noisyH=noisyH, clean=clean[b].copy(),
                 hm=np.broadcast_to(hm, (P, NHW)).astype(bf).copy(),
                 maskb=maskb, mod=mod)
        in_maps.append(m)

    global _LAST_INMAPS
    _LAST_INMAPS = in_maps
    res = run_bass_kernel_spmd(nc, in_maps, core_ids=list(range(8)))
    out = np.empty((B, T, D), f32)
    for i in range(8):
        b, half = i // 2, i % 2
        out[b, half * TQ:(half + 1) * TQ] = res.results[i]["out"]
    return out


_LAST_INMAPS = None


def run_profiled(tmpdir=None):
    """Re-run the last kernel invocation with NTFF tracing; return exec ns."""
    if _NC_CACHE is None or _LAST_INMAPS is None:
        return None
    res = run_bass_kernel_spmd(_NC_CACHE, _LAST_INMAPS,
                               core_ids=list(range(8)), trace=True,
                               tmpdir=tmpdir)
    return res.exec_time_ns


if __name__ == "__main__":
    build_nc()
    print("build ok")



# revision 34
# speedup vs baseline: 1.0412x; 1.0262x over previous
# Trainium2 Bass kernel for nn_CrossAttention_noise (B=4, T1=T2=1024, D=1024,
# H=16, DK=64, K=13, FF=4096), SPMD over 8 NeuronCores.
#
# Sharding: core i handles batch b=i//2 and query-token half t0=(i%2)*512.
# Each core computes its 512 output tokens end-to-end (the K/V convolution
# over the full clean sequence is duplicated between the two cores of a
# batch; no collectives).
#
# Key structure:
#  - QKV grouped convs run "polyphase": even/odd output tokens are separate
#    PE-array columns, so each matmul streams N=256 with a full 128x128 array
#    (2x fewer PE cycles than the shifted-window form).  Conv weights+inputs
#    are fp8e4 (weights prescaled x32); K/V sbuf stores keep the x32 scale
#    and fold it into the softmax exp scale / the v65 ones-column.
#  - K/V token order is "pair-major" (evens then odds per 512-block), which
#    is softmax-invariant; the key-padding mask rides as row 64 of kT and a
#    ones row 64 of qT, so exp needs no per-chunk bias and batches 2 chunks.
#  - PV runs fp8 DoubleRow (contract 256/matmul): pT (exp output) and v65
#    are fp8e4.  fc runs fp8 DoubleRow too (attnT fp8, fcw fp8 x32).
#  - k bias is dropped (softmax-shift invariant), v bias is folded into
#    fc_b on the host (fc_b += fc_w @ bv), q bias stays on-device.
#  - FFN stays bf16 (fp8 there costs ~1.7e-2 rel err; over budget).
#  - Clean-attention path is emitted before the noisy path so PE work
#    (ct2 transposes, k/v convs) starts as soon as the 8 clean LNs finish.
import numpy as np
import ml_dtypes
from contextlib import ExitStack

import concourse.bass as bass
import concourse.mybir as mybir
import concourse.tile as tile
from concourse import bacc
from concourse.bass_utils import run_bass_kernel_spmd
from concourse.masks import make_identity

BF16 = mybir.dt.bfloat16
F32 = mybir.dt.float32
FP8 = mybir.dt.float8e4
AF = mybir.ActivationFunctionType
ALU = mybir.AluOpType
AX = mybir.AxisListType
DR = mybir.MatmulPerfMode.DoubleRow
WSC = 32.0            # fp8 weight prescale (conv weights, fc_w stored *32)

B, T, D, H, DK, KW, FF = 4, 1024, 1024, 16, 64, 13, 4096
TQ = 512          # query tokens per core
NHW = 768         # noisy halo window rows (zero-padded on host)
NT2W = 264        # nt2 pair-major width (valid cols 0..261)
CT2W = 520        # ct2 pair-major width (valid cols 3..514)
P = 128
EPS1, EPS2 = 1e-5, 1e-6
NEG = -1.0e30


def _ln_apply(nc, pool, x, out, eps_ap, affine=None, apply_eng="dve"):
    """out = (x - mean)/sqrt(var + eps) rowwise; x [p, D] f32 in SBUF."""
    p = x.shape[0]
    s = pool.tile([P, 1], F32, tag="ln_s", name="ln_s")
    sq = pool.tile([P, 1], F32, tag="ln_sq", name="ln_sq")
    scr = pool.tile([P, D], BF16, tag="ln_scr", name="ln_scr", bufs=2)[:p]
    nc.vector.reduce_sum(s[:p], x, axis=AX.X)
    nc.scalar.activation(scr, x, AF.Square, accum_out=sq[:p])
    mu = pool.tile([P, 1], F32, tag="ln_mu", name="ln_mu")[:p]
    nc.vector.tensor_scalar_mul(mu, s[:p], 1.0 / D)
    musq = pool.tile([P, 1], F32, tag="ln_musq", name="ln_musq")[:p]
    nc.vector.tensor_tensor(musq, mu, mu, ALU.mult)
    var = pool.tile([P, 1], F32, tag="ln_var", name="ln_var")[:p]
    nc.vector.tensor_scalar(var, sq[:p], 1.0 / D, musq, ALU.mult, ALU.subtract)
    std = pool.tile([P, 1], F32, tag="ln_std", name="ln_std")[:p]
    nc.scalar.activation(std, var, AF.Sqrt, bias=eps_ap[:p])
    rstd = pool.tile([P, 1], F32, tag="ln_rstd", name="ln_rstd")[:p]
    nc.vector.reciprocal(rstd, std)
    beta = pool.tile([P, 1], F32, tag="ln_beta", name="ln_beta")[:p]
    nc.vector.tensor_tensor(beta, mu, rstd, ALU.mult)
    nc.vector.tensor_scalar_mul(beta, beta, -1.0)
    if affine is not None:
        g, b = affine
        negmu = pool.tile([P, 1], F32, tag="ln_negmu", name="ln_negmu")[:p]
        nc.vector.tensor_scalar_mul(negmu, mu, -1.0)
        nc.vector.scalar_tensor_tensor(out, x, negmu, g, ALU.add, ALU.mult)
        nc.vector.scalar_tensor_tensor(out, out, rstd, b, ALU.mult, ALU.add)
    elif apply_eng == "act":
        nc.scalar.activation(out, x, AF.Identity, bias=beta, scale=rstd)
    else:
        nc.vector.tensor_scalar(out, x, rstd, beta, ALU.mult, ALU.add)


def build_nc():
    nc = bacc.Bacc("TRN2", target_bir_lowering=False, debug=False,
                   num_devices=8)
    dt = {}

    def din(name, shape, dtype):
        dt[name] = nc.dram_tensor(name, list(shape), dtype,
                                  kind="ExternalInput").ap()

    din("noisyH", (NHW, D), F32)          # rows [t0-128, t0+640), zero padded
    din("clean", (T, D), F32)
    din("hm", (P, NHW), BF16)              # halo-token validity (rows equal)
    din("maskb", (1, T), BF16)            # 0 / -1e30 key mask, PAIR-MAJOR
    din("mod", (8, D), F32)               # sh/sc/g msa+mlp rows + g_msa/WSC
    din("lng", (P, D), F32)               # ln_noisy_g broadcast to 128 rows
    din("lnb", (P, D), F32)
    din("clng", (D,), F32)
    din("clnb", (D,), F32)
    din("wql", (P, H, 7, P), FP8)         # polyphase conv lhsT, *WSC
    din("wkl", (P, H, 7, P), FP8)
    din("wvl", (P, H, 7, P), FP8)
    din("bq", (D,), F32)
    din("fcw", (8, P, 8, P), FP8)         # fc_w.T*WSC tiles [mc][kp][ko][mj]
    din("fcb", (D,), F32)                 # fc_b + fc_w @ bv (host-folded)
    din("w1t", (32, P, 8, P), BF16)       # ff_w1.T tiles [mc][kp][ko][mj]
    din("fb1", (FF,), F32)
    din("w2t", (8, 4, P, 8, P), BF16)     # ff_w2.T tiles [mc][kq][kp][k8][mj]
    din("fb2", (D,), F32)
    out_ap = nc.dram_tensor("out", [TQ, D], F32, kind="ExternalOutput").ap()

    with tile.TileContext(nc) as tc:
        _emit(tc, dt, out_ap)
    nc.compile()
    return nc


def _emit(tc, dt, out_ap):
    nc = tc.nc
    with ExitStack() as ctx:
        const = ctx.enter_context(tc.tile_pool(name="const", bufs=1))
        small = ctx.enter_context(tc.tile_pool(name="small", bufs=3))
        lnio = ctx.enter_context(tc.tile_pool(name="lnio", bufs=3))
        big = ctx.enter_context(tc.tile_pool(name="bigsb", bufs=1))
        trans = ctx.enter_context(tc.tile_pool(name="trans", bufs=3))
        wpool = ctx.enter_context(tc.tile_pool(name="wstream", bufs=6))
        psc = ctx.enter_context(tc.tile_pool(name="psc", bufs=2, space="PSUM"))
        ptp = ctx.enter_context(tc.tile_pool(name="ptp", bufs=2, space="PSUM"))

        ident = const.tile([P, P], BF16)
        make_identity(nc, ident)
        eps1_t = const.tile([P, 1], F32)
        nc.vector.memset(eps1_t, EPS1)
        eps2_t = const.tile([P, 1], F32)
        nc.vector.memset(eps2_t, EPS2)

        def chanvec(name, w=8):
            t = const.tile([P, w], F32, tag=f"cv_{name}")
            nc.sync.dma_start(t, dt[name].rearrange("(m p) -> p m", p=P))
            return t

        bq_s = chanvec("bq")
        fcb_s, fb2_s = chanvec("fcb"), chanvec("fb2")
        clng_s, clnb_s = chanvec("clng"), chanvec("clnb")
        fb1_s = chanvec("fb1", 32)
        mod_s = const.tile([P, 8, 8], F32)
        for s in range(8):
            nc.sync.dma_start(mod_s[:, s, :],
                              dt["mod"][s].rearrange("(m p) -> p m", p=P))
        sh_msa, sc_msa, g_msa = mod_s[:, 0, :], mod_s[:, 1, :], mod_s[:, 2, :]
        sh_mlp, sc_mlp, g_mlp = mod_s[:, 3, :], mod_s[:, 4, :], mod_s[:, 5, :]
        g_msa_ds = mod_s[:, 6, :]
        hm_s = const.tile([P, NHW], BF16)
        nc.sync.dma_start(hm_s, dt["hm"])
        lng_s = const.tile([P, D], F32)
        nc.sync.dma_start(lng_s, dt["lng"])
        lnb_s = const.tile([P, D], F32)
        nc.sync.dma_start(lnb_s, dt["lnb"])

        xres = big.tile([P, 4, D], F32)        # LN1 rows [t0, t0+512); later x
        attnT = big.tile([P, 8, TQ], FP8)      # concat_h(attn_h), chan-major

        with tc.tile_pool(name="bigc", bufs=1) as bigc, \
             tc.tile_pool(name="kvpool", bufs=1) as kvpool:
            psm_cm = tc.tile_pool(name="psm", bufs=2, space="PSUM")
            psm = psm_cm.__enter__()
            # ---- Phase A: clean LNs -> ct2P builds (pair-major) -------------
            lnpC_cm = tc.tile_pool(name="lnpC", bufs=1)
            lnpC = lnpC_cm.__enter__()
            clnall = [lnpC.tile([P, D], BF16, name=f"clnall_{i}")
                      for i in range(8)]
            for r in range(8):
                xt = lnio.tile([P, D], F32, tag="ln_in", bufs=2)
                nc.sync.dma_start(xt, dt["clean"][r * P:(r + 1) * P, :])
                _ln_apply(nc, small, xt, clnall[r], eps1_t,
                          apply_eng=("act" if r % 2 else "dve"))
            ct2s = []
            cp_eng = [nc.vector, nc.gpsimd]
            for m in range(8):
                ct2m = bigc.tile([P, 2, CT2W], FP8, name=f"ct2_{m}")
                ct2s.append(ct2m)
                for hh in range(2):
                    nc.gpsimd.memset(ct2m[:, hh, 0:3], 0.0)
                    nc.gpsimd.memset(ct2m[:, hh, 515:CT2W], 0.0)
                tmc = trans.tile([P, T], BF16, tag="tmc", bufs=2)
                for r in range(8):
                    pt = ptp.tile([P, P], BF16, tag="tpbf")
                    nc.tensor.transpose(pt, clnall[r][:, m * P:(m + 1) * P],
                                        ident)
                    dst = tmc[:, r * P:(r + 1) * P]
                    if r % 2 == 0:
                        nc.vector.tensor_scalar(dst, pt, clng_s[:, m:m + 1],
                                                clnb_s[:, m:m + 1],
                                                ALU.mult, ALU.add)
                    else:
                        nc.scalar.activation(dst, pt, AF.Identity,
                                             bias=clnb_s[:, m:m + 1],
                                             scale=clng_s[:, m:m + 1])
                for hh in range(2):
                    sl = slice(hh * DK, (hh + 1) * DK)
                    e0, e1 = cp_eng[hh], cp_eng[1 - hh]
                    e0.tensor_copy(ct2m[0:DK, hh, 3:515], tmc[sl, 0::2])
                    e1.tensor_copy(ct2m[DK:P, hh, 3:515], tmc[sl, 1::2])
            lnpC_cm.__exit__(None, None, None)

            def conv(h, wname, x2, nchunk, outT, bias=None, descale=None):
                """Polyphase grouped conv for head h into outT[0:64, :]."""
                hp, hc = h % 2, h // 2
                wsb = wpool.tile([P, 7, P], FP8, tag="convw", bufs=4,
                                 name=f"w_{wname}_{h}")
                nc.sync.dma_start(wsb, dt[wname][:, h])
                for c in range(nchunk):
                    ps = psm.tile([P, 256], F32, tag="conv")
                    for j in range(7):
                        nc.tensor.matmul(
                            ps, wsb[:, j, :],
                            x2[:, hp, c * 256 + j:c * 256 + j + 256],
                            start=(j == 0), stop=(j == 6))
                    if bias is not None:   # q: token-major interleave + bias
                        dst = outT[:DK, c * TQ:(c + 1) * TQ].rearrange(
                            "p (n two) -> p two n", two=2)
                        b = bias[hp * DK:(hp + 1) * DK, hc:hc + 1]
                        nc.scalar.activation(dst[:, 0, :], ps[0:DK, :],
                                             AF.Identity, bias=b,
                                             scale=descale)
                        nc.scalar.activation(dst[:, 1, :], ps[DK:P, :],
                                             AF.Identity, bias=b,
                                             scale=descale)
                    else:                  # k/v: pair-major contiguous
                        d0 = outT[:DK, c * TQ:c * TQ + 256]
                        d1 = outT[:DK, c * TQ + 256:(c + 1) * TQ]
                        if (h + c) % 2 == 0:
                            nc.vector.tensor_copy(d0, ps[0:DK, :])
                            nc.scalar.activation(d1, ps[DK:P, :], AF.Identity)
                        else:
                            nc.scalar.activation(d0, ps[0:DK, :], AF.Identity)
                            nc.vector.tensor_copy(d1, ps[DK:P, :])

            # ---- Phase B: all k/v convs (x32 scale riding on k and v);
            # v65 built immediately per head so vT can rotate ----------------
            kTs, v65s = [], []
            vt_cm = tc.tile_pool(name="vtmp", bufs=2)
            vtp = vt_cm.__enter__()
            for h in range(H):
                kT = kvpool.tile([65, T], BF16, name=f"kT_{h}")
                nc.sync.dma_start(kT[64:65, :], dt["maskb"])
                conv(h, "wkl", ct2s[h // 2], 2, kT)
                kTs.append(kT)
                vT = vtp.tile([DK, T], BF16, tag="vT")
                conv(h, "wvl", ct2s[h // 2], 2, vT)
                # v65: v tokens-on-partitions + 32.0 col (cancels x32 scale)
                v65 = kvpool.tile([P, 8, 80], FP8, name=f"v65_{h}")
                nc.vector.memset(v65[:, :, 64:65], WSC)
                nc.vector.memset(v65[:, :, 65:80], 0.0)
                for c2 in range(4):
                    pt = ptp.tile([P, P], BF16, tag="tpbf")
                    nc.tensor.transpose(pt[:, 0:DK],
                                        vT[:, 2 * c2 * P:(2 * c2 + 1) * P],
                                        ident[:DK, :DK])
                    nc.tensor.transpose(pt[:, DK:P],
                                        vT[:, (2 * c2 + 1) * P:(2 * c2 + 2) * P],
                                        ident[:DK, :DK])
                    nc.vector.tensor_copy(
                        v65[:, 2 * c2:2 * c2 + 2, 0:DK],
                        pt.rearrange("p (two n) -> p two n", two=2))
                v65s.append(v65)
            vt_cm.__exit__(None, None, None)

            # ---- Phase C1: noisy LNs -> nt2P builds -> q convs --------------
            lnpN_cm = tc.tile_pool(name="lnpN", bufs=1)
            lnpN = lnpN_cm.__enter__()
            lnall = [lnpN.tile([P, D], BF16, name=f"lnall_{i}")
                     for i in range(6)]  # noisy ln2 tiles
            for r in range(6):
                xt = lnio.tile([P, D], F32, tag="ln_in", bufs=2)
                nc.sync.dma_start(xt, dt["noisyH"][r * P:(r + 1) * P, :])
                if 1 <= r <= 4:
                    ln1 = xres[:, r - 1, :]
                else:
                    ln1 = lnio.tile([P, D], F32, tag="ln1_tmp", bufs=1)
                _ln_apply(nc, small, xt, ln1, eps1_t,
                          affine=(lng_s, lnb_s))
                _ln_apply(nc, small, ln1, lnall[r], eps2_t,
                          apply_eng=("act" if r % 2 else "dve"))

            nt2s = []
            for m in range(8):
                nt2m = bigc.tile([P, 2, NT2W], FP8, name=f"nt2_{m}")
                nt2s.append(nt2m)
                tmn = trans.tile([P, NHW], BF16, tag="tmn", bufs=2)
                for r in range(6):
                    pt = ptp.tile([P, P], BF16, tag="tpbf")
                    nc.tensor.transpose(pt, lnall[r][:, m * P:(m + 1) * P],
                                        ident)
                    dst = tmn[:, r * P:(r + 1) * P]
                    if r % 2 == 0:
                        nc.vector.tensor_scalar(dst, pt, sc_msa[:, m:m + 1],
                                                sh_msa[:, m:m + 1],
                                                ALU.mult, ALU.add)
                    else:
                        nc.scalar.activation(dst, pt, AF.Identity,
                                             bias=sh_msa[:, m:m + 1],
                                             scale=sc_msa[:, m:m + 1])
                for hh in range(2):
                    sl = slice(hh * DK, (hh + 1) * DK)
                    e0, e1 = cp_eng[hh], cp_eng[1 - hh]
                    e0.tensor_tensor(nt2m[0:DK, hh, 0:262],
                                     tmn[sl, 122:646:2],
                                     hm_s[sl, 122:646:2], ALU.mult)
                    e1.tensor_tensor(nt2m[DK:P, hh, 0:262],
                                     tmn[sl, 123:647:2],
                                     hm_s[sl, 123:647:2], ALU.mult)
            lnpN_cm.__exit__(None, None, None)

            qTs = []
            for h in range(H):
                qT = kvpool.tile([65, TQ], BF16, name=f"qT_{h}")
                nc.vector.memset(qT[64:65, :], 1.0)
                conv(h, "wql", nt2s[h // 2], 1, qT, bias=bq_s,
                     descale=1.0 / WSC)
                qTs.append(qT)
            psm_cm.__exit__(None, None, None)

            # ---- Phase C2: per-head cross attention -------------------------
            hpool_cm = tc.tile_pool(name="hpool", bufs=2)
            hpool = hpool_cm.__enter__()
            ppv_cm = tc.tile_pool(name="ppv", bufs=1, space="PSUM")
            ppv = ppv_cm.__enter__()
            for h in range(H):
                hp, hc = h % 2, h // 2
                kT, v65, qT = kTs[h], v65s[h], qTs[h]

                # scores (x32): mask rides on kT row 64 * qT ones row;
                # fused scale/exp over 2 chunks at a time -> fp8 pT
                pT = hpool.tile([P, 8, TQ], FP8, tag="pT", bufs=2)
                for g in range(4):
                    ps2 = psc.tile([P, 2, TQ], F32, tag="sc")
                    for i in range(2):
                        cc = 2 * g + i
                        nc.tensor.matmul(ps2[:, i, :],
                                         kT[:, cc * P:(cc + 1) * P], qT,
                                         start=True, stop=True)
                    nc.scalar.activation(pT[:, 2 * g:2 * g + 2, :], ps2,
                                         AF.Exp, scale=0.125 / WSC)

                # PV fp8 DoubleRow: out[66, TQ] (row 64 = denom * 32)
                pv = ppv.tile([P, TQ], F32, tag="pv")
                for c2 in range(4):
                    nc.tensor.matmul(pv[:80, :],
                                     v65[:, 2 * c2:2 * c2 + 2, 0:80],
                                     pT[:, 2 * c2:2 * c2 + 2, :],
                                     start=(c2 == 0), stop=(c2 == 3),
                                     perf_mode=DR)
                linv = trans.tile([1, TQ], F32, tag="linv")
                nc.vector.reciprocal(linv, pv[64:65, :])
                bc_sb = trans.tile([DK, TQ], F32, tag="bcsb", bufs=2)
                nc.gpsimd.partition_broadcast(bc_sb, linv)
                nc.vector.tensor_tensor(attnT[hp * DK:(hp + 1) * DK, hc, :],
                                        pv[0:DK, :], bc_sb, ALU.mult)
            ppv_cm.__exit__(None, None, None)
            hpool_cm.__exit__(None, None, None)

        # ---- Phase D: fc projection (fp8 DR) + gate + residual --------------
        fcgs = []
        for m in range(8):
            wt = wpool.tile([P, 8, P], FP8, tag="wt")
            nc.sync.dma_start(wt, dt["fcw"][m])
            ps = psc.tile([P, 2, TQ], F32, tag="sc")
            for k in range(4):
                nc.tensor.matmul(ps[:, 0, :], wt[:, 2 * k:2 * k + 2, :],
                                 attnT[:, 2 * k:2 * k + 2, :],
                                 start=(k == 0), stop=(k == 3), perf_mode=DR)
            fcg = trans.tile([P, TQ], BF16, tag="fcg", bufs=8,
                             name=f"fcg_{m}")
            fcbg = small.tile([P, 1], F32, tag="fcbg", name="fcbg")
            nc.vector.tensor_tensor(fcbg, fcb_s[:, m:m + 1],
                                    g_msa[:, m:m + 1], ALU.mult)
            nc.scalar.activation(fcg, ps[:, 0, :], AF.Identity, bias=fcbg,
                                 scale=g_msa_ds[:, m:m + 1])
            fcgs.append(fcg)
        for j in range(4):
            for m in range(8):
                pt = ptp.tile([P, P], BF16, tag="tpbf")
                nc.tensor.transpose(pt, fcgs[m][:, j * P:(j + 1) * P], ident)
                nc.vector.tensor_tensor(xres[:, j, m * P:(m + 1) * P], pt,
                                        xres[:, j, m * P:(m + 1) * P], ALU.add)

        # ---- Phase E: LN3 + mlp modulation -> n2T ---------------------------
        bigf_cm = tc.tile_pool(name="bigf", bufs=1)
        bigf = bigf_cm.__enter__()
        n2T = bigf.tile([P, 8, TQ], BF16)
        for s in range(4):
            l3 = lnio.tile([P, D], BF16, tag="ln2b")
            _ln_apply(nc, small, xres[:, s, :], l3, eps2_t)
            for m in range(8):
                pt = ptp.tile([P, P], BF16, tag="tpbf")
                nc.tensor.transpose(pt, l3[:, m * P:(m + 1) * P], ident)
                nc.vector.tensor_scalar(n2T[:, m, s * P:(s + 1) * P], pt,
                                        sc_mlp[:, m:m + 1], sh_mlp[:, m:m + 1],
                                        ALU.mult, ALU.add)

        # ---- Phase F: FFN (bf16) --------------------------------------------
        if True:
            ffa = bigf.tile([P, 32, TQ], BF16)
            for m in range(32):
                wt = wpool.tile([P, 8, P], BF16, tag="wtf")
                nc.sync.dma_start(wt, dt["w1t"][m])
                ps = psc.tile([P, 2, TQ], F32, tag="sc")
                for k in range(8):
                    nc.tensor.matmul(ps[:, 0, :], wt[:, k, :], n2T[:, k, :],
                                     start=(k == 0), stop=(k == 7))
                nc.scalar.activation(ffa[:, m, :], ps[:, 0, :],
                                     AF.Gelu_apprx_tanh,
                                     bias=fb1_s[:, m:m + 1])
            for m in range(8):
                ps = psc.tile([P, 2, TQ], F32, tag="sc")
                for kq in range(4):
                    wt = wpool.tile([P, 8, P], BF16, tag="wtf")
                    nc.sync.dma_start(wt, dt["w2t"][m, kq])
                    for k8 in range(8):
                        k = kq * 8 + k8
                        nc.tensor.matmul(ps[:, 0, :], wt[:, k8, :],
                                         ffa[:, k, :],
                                         start=(k == 0), stop=(k == 31))
                ffog = trans.tile([P, TQ], BF16, tag="ffog", bufs=2)
                fbg = small.tile([P, 1], F32, tag="fcbg", name="fbg")
                nc.vector.tensor_tensor(fbg, fb2_s[:, m:m + 1],
                                        g_mlp[:, m:m + 1], ALU.mult)
                nc.scalar.activation(ffog, ps[:, 0, :], AF.Identity, bias=fbg,
                                     scale=g_mlp[:, m:m + 1])
                for j in range(4):
                    pt = ptp.tile([P, P], BF16, tag="tpbf")
                    nc.tensor.transpose(pt, ffog[:, j * P:(j + 1) * P], ident)
                    nc.vector.tensor_tensor(xres[:, j, m * P:(m + 1) * P], pt,
                                            xres[:, j, m * P:(m + 1) * P],
                                            ALU.add)
        bigf_cm.__exit__(None, None, None)

        for s in range(4):
            nc.sync.dma_start(out_ap[s * P:(s + 1) * P, :], xres[:, s, :])


# --------------------------- host side --------------------------------------
_NC_CACHE = None


def _prep_conv_w_poly(w):
    # w: (D, DK, KW) grouped conv -> [128, H, 7, 128] fp8 polyphase lhsT *WSC
    wr = (np.asarray(w, np.float32) * WSC).reshape(H, DK, DK, KW)  # h,o,c,tap
    arr = np.zeros((P, H, 7, P), np.float32)
    for j in range(7):
        t = lambda k: wr[:, :, :, k].transpose(2, 0, 1)    # -> [c, h, o]
        arr[0:DK, :, j, 0:DK] = t(2 * j)                   # even out, tap 2j
        if 2 * j + 1 <= 12:
            arr[DK:P, :, j, 0:DK] = t(2 * j + 1)           # even out, 2j+1
        if j >= 1:
            arr[0:DK, :, j, DK:P] = t(2 * j - 1)           # odd out, 2j-1
        arr[DK:P, :, j, DK:P] = t(2 * j)                   # odd out, 2j
    return arr.astype(ml_dtypes.float8_e4m3fn)


def kernel(**inputs):
    global _NC_CACHE
    if _NC_CACHE is None:
        _NC_CACHE = build_nc()
    nc = _NC_CACHE

    f32 = np.float32
    bf = ml_dtypes.bfloat16
    fp8 = ml_dtypes.float8_e4m3fn
    noisy = np.asarray(inputs["noisy_feats"], f32)
    clean = np.asarray(inputs["clean_feats"], f32)
    t = np.asarray(inputs["t"], f32)
    clean_len = np.asarray(inputs["clean_lengths"]).astype(np.int64)

    # AdaLayerNormZero on host (0.02% of FLOPs): emb = silu(t) @ ada_w.T + b
    st = t * (1.0 / (1.0 + np.exp(-t, dtype=f32)))
    emb = st @ np.asarray(inputs["ada_w"], f32).T + np.asarray(inputs["ada_b"], f32)
    sh_msa, sc_msa, g_msa, sh_mlp, sc_mlp, g_mlp = np.split(emb, 6, axis=1)

    wql = _prep_conv_w_poly(inputs["wq"])
    wkl = _prep_conv_w_poly(inputs["wk"])
    wvl = _prep_conv_w_poly(inputs["wv"])
    fcw = (np.asarray(inputs["fc_w"], f32).T * WSC).reshape(8, P, 8, P) \
        .transpose(2, 1, 0, 3).astype(fp8).copy()
    w1t = np.asarray(inputs["ff_w1"], f32).T.reshape(8, P, 32, P) \
        .transpose(2, 1, 0, 3).astype(bf).copy()
    w2t = np.asarray(inputs["ff_w2"], f32).T.reshape(32, P, 8, P) \
        .transpose(2, 0, 1, 3).reshape(8, 4, 8, P, P) \
        .transpose(0, 1, 3, 2, 4).astype(bf).copy()
    # fold v bias into fc bias: fc(attn + bv) = fc(attn) + fc_w @ bv
    fcb_eff = (np.asarray(inputs["fc_b"], f32)
               + np.asarray(inputs["fc_w"], f32) @ np.asarray(inputs["bv"], f32))

    # pair-major t2 permutation: col c*512+par*256+n <-> token c*512+2n+par
    perm = np.empty(T, np.int64)
    for c in range(2):
        for par in range(2):
            base = c * 512 + par * 256
            perm[base:base + 256] = c * 512 + 2 * np.arange(256) + par

    common = dict(
        lng=np.broadcast_to(np.asarray(inputs["ln_noisy_g"], f32), (P, D)).copy(),
        lnb=np.broadcast_to(np.asarray(inputs["ln_noisy_b"], f32), (P, D)).copy(),
        clng=np.asarray(inputs["ln_clean_g"], f32).copy(),
        clnb=np.asarray(inputs["ln_clean_b"], f32).copy(),
        wql=wql, wkl=wkl, wvl=wvl,
        bq=np.asarray(inputs["bq"], f32).copy(),
        fcw=fcw, fcb=fcb_eff.copy(),
        w1t=w1t, fb1=np.asarray(inputs["ff_b1"], f32).copy(),
        w2t=w2t, fb2=np.asarray(inputs["ff_b2"], f32).copy(),
    )

    in_maps = []
    for i in range(8):
        b, half = i // 2, i % 2
        t0 = half * TQ
        noisyH = np.zeros((NHW, D), f32)
        lo, hi = t0 - P, t0 + 640
        clo, chi = max(lo, 0), min(hi, T)
        noisyH[clo - lo:chi - lo] = noisy[b, clo:chi]
        hm = np.zeros((NHW,), f32)
        hm[clo - lo:chi - lo] = 1.0
        maskb = np.where(np.arange(T) >= clean_len[b], NEG, 0.0).astype(f32)
        maskb_pm = maskb[perm][None, :].astype(bf).copy()
        mod = np.stack([sh_msa[b], 1.0 + sc_msa[b], g_msa[b],
                        sh_mlp[b], 1.0 + sc_mlp[b], g_mlp[b],
                        g_msa[b] / WSC, g_mlp[b] / WSC]).astype(f32)
        m = dict(common)
        m.update(noisyH=noisyH, clean=clean[b].copy(),
                 hm=np.broadcast_to(hm, (P, NHW)).astype(bf).copy(),
                 maskb=maskb_pm, mod=mod)
        in_maps.append(m)

    global _LAST_INMAPS
    _LAST_INMAPS = in_maps
    res = run_bass_kernel_spmd(nc, in_maps, core_ids=list(range(8)))
    out = np.empty((B, T, D), f32)
    for i in range(8):
        b, half = i // 2, i % 2
        out[b, half * TQ:(half + 1) * TQ] = res.results[i]["out"]
    return out


_LAST_INMAPS = None


def run_profiled(tmpdir=None):
    """Re-run the last kernel invocation with NTFF tracing; return exec ns."""
    if _NC_CACHE is None or _LAST_INMAPS is None:
        return None
    res = run_bass_kernel_spmd(_NC_CACHE, _LAST_INMAPS,
                               core_ids=list(range(8)), trace=True,
                               tmpdir=tmpdir)
    return res.exec_time_ns


if __name__ == "__main__":
    build_nc()
    print("build ok")


# revision 38
# speedup vs baseline: 1.0836x; 1.0407x over previous
# Trainium2 Bass kernel for nn_CrossAttention_noise (B=4, T1=T2=1024, D=1024,
# H=16, DK=64, K=13, FF=4096), SPMD over 8 NeuronCores.
#
# Sharding: core i handles batch b=i//2 and query-token half t0=(i%2)*512.
# Each core computes its 512 output tokens end-to-end (the K/V convolution
# over the full clean sequence is duplicated between the two cores of a
# batch; no collectives).
#
# Key structure:
#  - QKV grouped convs run "polyphase": even/odd output tokens are separate
#    PE-array columns, so each matmul streams N=256 with a full 128x128 array
#    (2x fewer PE cycles than the shifted-window form).  Conv weights+inputs
#    are fp8e4 (weights prescaled x32); K/V sbuf stores keep the x32 scale
#    and fold it into the softmax exp scale / the v65 ones-column.
#  - K/V token order is "pair-major" (evens then odds per 512-block), which
#    is softmax-invariant; the key-padding mask rides as row 64 of kT and a
#    ones row 64 of qT, so exp needs no per-chunk bias and batches 2 chunks.
#  - PV runs fp8 DoubleRow (contract 256/matmul): pT (exp output) and v65
#    are fp8e4.  fc runs fp8 DoubleRow too (attnT fp8, fcw fp8 x32).
#  - k bias is dropped (softmax-shift invariant), v bias is folded into
#    fc_b on the host (fc_b += fc_w @ bv), q bias stays on-device.
#  - FFN stays bf16 (fp8 there costs ~1.7e-2 rel err; over budget).
#  - Clean-attention path is emitted before the noisy path so PE work
#    (ct2 transposes, k/v convs) starts as soon as the 8 clean LNs finish.
import numpy as np
import ml_dtypes
from contextlib import ExitStack

import concourse.bass as bass
import concourse.mybir as mybir
import concourse.tile as tile
from concourse import bacc
from concourse.bass_utils import run_bass_kernel_spmd
from concourse.masks import make_identity

BF16 = mybir.dt.bfloat16
F32 = mybir.dt.float32
FP8 = mybir.dt.float8e4
AF = mybir.ActivationFunctionType
ALU = mybir.AluOpType
AX = mybir.AxisListType
DR = mybir.MatmulPerfMode.DoubleRow
WSC = 32.0            # fp8 weight prescale (conv weights, fc_w stored *32)

B, T, D, H, DK, KW, FF = 4, 1024, 1024, 16, 64, 13, 4096
TQ = 512          # query tokens per core
NHW = 768         # noisy halo window rows (zero-padded on host)
NT2W = 264        # nt2 pair-major width (valid cols 0..261)
CT2W = 520        # ct2 pair-major width (valid cols 3..514)
P = 128
EPS1, EPS2 = 1e-5, 1e-6
NEG = -1.0e30


def _ln_apply(nc, pool, x, out, eps_ap, affine=None, apply_eng="dve"):
    """out = (x - mean)/sqrt(var + eps) rowwise; x [p, D] f32 in SBUF."""
    p = x.shape[0]
    s = pool.tile([P, 1], F32, tag="ln_s", name="ln_s")
    sq = pool.tile([P, 1], F32, tag="ln_sq", name="ln_sq")
    scr = pool.tile([P, D], BF16, tag="ln_scr", name="ln_scr", bufs=2)[:p]
    nc.vector.reduce_sum(s[:p], x, axis=AX.X)
    nc.scalar.activation(scr, x, AF.Square, accum_out=sq[:p])
    mu = pool.tile([P, 1], F32, tag="ln_mu", name="ln_mu")[:p]
    nc.vector.tensor_scalar_mul(mu, s[:p], 1.0 / D)
    musq = pool.tile([P, 1], F32, tag="ln_musq", name="ln_musq")[:p]
    nc.vector.tensor_tensor(musq, mu, mu, ALU.mult)
    var = pool.tile([P, 1], F32, tag="ln_var", name="ln_var")[:p]
    nc.vector.tensor_scalar(var, sq[:p], 1.0 / D, musq, ALU.mult, ALU.subtract)
    std = pool.tile([P, 1], F32, tag="ln_std", name="ln_std")[:p]
    nc.scalar.activation(std, var, AF.Sqrt, bias=eps_ap[:p])
    rstd = pool.tile([P, 1], F32, tag="ln_rstd", name="ln_rstd")[:p]
    nc.vector.reciprocal(rstd, std)
    beta = pool.tile([P, 1], F32, tag="ln_beta", name="ln_beta")[:p]
    nc.vector.tensor_tensor(beta, mu, rstd, ALU.mult)
    nc.vector.tensor_scalar_mul(beta, beta, -1.0)
    if affine is not None:
        g, b = affine
        negmu = pool.tile([P, 1], F32, tag="ln_negmu", name="ln_negmu")[:p]
        nc.vector.tensor_scalar_mul(negmu, mu, -1.0)
        nc.vector.scalar_tensor_tensor(out, x, negmu, g, ALU.add, ALU.mult)
        nc.vector.scalar_tensor_tensor(out, out, rstd, b, ALU.mult, ALU.add)
    elif apply_eng == "act":
        nc.scalar.activation(out, x, AF.Identity, bias=beta, scale=rstd)
    else:
        nc.vector.tensor_scalar(out, x, rstd, beta, ALU.mult, ALU.add)


def build_nc():
    nc = bacc.Bacc("TRN2", target_bir_lowering=False, debug=False,
                   num_devices=8)
    dt = {}

    def din(name, shape, dtype):
        dt[name] = nc.dram_tensor(name, list(shape), dtype,
                                  kind="ExternalInput").ap()

    din("noisyH", (NHW, D), F32)          # rows [t0-128, t0+640), zero padded
    din("clean", (T, D), F32)
    din("hm", (P, NHW), BF16)              # halo-token validity (rows equal)
    din("maskb", (1, T), BF16)            # 0 / -1e30 key mask, PAIR-MAJOR
    din("mod", (8, D), F32)               # sh/sc/g msa+mlp rows + g_msa/WSC
    din("lng", (P, D), F32)               # ln_noisy_g broadcast to 128 rows
    din("lnb", (P, D), F32)
    din("clng", (D,), F32)
    din("clnb", (D,), F32)
    din("wql", (P, H, 7, P), FP8)         # polyphase conv lhsT, *WSC
    din("wkl", (P, H, 7, P), FP8)
    din("wvl", (P, H, 7, P), FP8)
    din("bq", (D,), F32)
    din("fcw", (8, P, 8, P), FP8)         # fc_w.T*WSC tiles [mc][kp][ko][mj]
    din("fcb", (D,), F32)                 # fc_b + fc_w @ bv (host-folded)
    din("w1t", (32, P, 8, P), BF16)       # ff_w1.T tiles [mc][kp][ko][mj]
    din("fb1", (FF,), F32)
    din("w2t", (8, 4, P, 8, P), BF16)     # ff_w2.T tiles [mc][kq][kp][k8][mj]
    din("fb2", (D,), F32)
    out_ap = nc.dram_tensor("out", [TQ, D], F32, kind="ExternalOutput").ap()

    with tile.TileContext(nc) as tc:
        _emit(tc, dt, out_ap)
    nc.compile()
    return nc


def _emit(tc, dt, out_ap):
    nc = tc.nc
    with ExitStack() as ctx:
        const = ctx.enter_context(tc.tile_pool(name="const", bufs=1))
        small = ctx.enter_context(tc.tile_pool(name="small", bufs=3))
        lnio = ctx.enter_context(tc.tile_pool(name="lnio", bufs=3))
        big = ctx.enter_context(tc.tile_pool(name="bigsb", bufs=1))
        trans = ctx.enter_context(tc.tile_pool(name="trans", bufs=3))
        wpool = ctx.enter_context(tc.tile_pool(name="wstream", bufs=6))
        psc = ctx.enter_context(tc.tile_pool(name="psc", bufs=2, space="PSUM"))
        ptp = ctx.enter_context(tc.tile_pool(name="ptp", bufs=2, space="PSUM"))

        ident = const.tile([P, P], BF16)
        make_identity(nc, ident)
        eps1_t = const.tile([P, 1], F32)
        nc.vector.memset(eps1_t, EPS1)
        eps2_t = const.tile([P, 1], F32)
        nc.vector.memset(eps2_t, EPS2)

        def chanvec(name, w=8):
            t = const.tile([P, w], F32, tag=f"cv_{name}")
            nc.sync.dma_start(t, dt[name].rearrange("(m p) -> p m", p=P))
            return t

        bq_s = chanvec("bq")
        fcb_s, fb2_s = chanvec("fcb"), chanvec("fb2")
        clng_s, clnb_s = chanvec("clng"), chanvec("clnb")
        fb1_s = chanvec("fb1", 32)
        mod_s = const.tile([P, 8, 8], F32)
        for s in range(8):
            nc.sync.dma_start(mod_s[:, s, :],
                              dt["mod"][s].rearrange("(m p) -> p m", p=P))
        sh_msa, sc_msa, g_msa = mod_s[:, 0, :], mod_s[:, 1, :], mod_s[:, 2, :]
        sh_mlp, sc_mlp, g_mlp = mod_s[:, 3, :], mod_s[:, 4, :], mod_s[:, 5, :]
        g_msa_ds = mod_s[:, 6, :]
        hm_s = const.tile([P, NHW], BF16)
        nc.sync.dma_start(hm_s, dt["hm"])
        lng_s = const.tile([P, D], F32)
        nc.sync.dma_start(lng_s, dt["lng"])
        lnb_s = const.tile([P, D], F32)
        nc.sync.dma_start(lnb_s, dt["lnb"])

        xres = big.tile([P, 4, D], F32)        # LN1 rows [t0, t0+512); later x
        attnT = big.tile([P, 8, TQ], FP8)      # concat_h(attn_h), chan-major

        with tc.tile_pool(name="bigc", bufs=1) as bigc:
            psm_cm = tc.tile_pool(name="psm", bufs=2, space="PSUM")
            psm = psm_cm.__enter__()

            def conv(h, wname, x2, nchunk, outT, bias=None, descale=None):
                """Polyphase grouped conv for head h into outT[0:64, :]."""
                hp, hc = h % 2, h // 2
                wsb = wpool.tile([P, 7, P], FP8, tag="convw", bufs=4,
                                 name=f"w_{wname}_{h}")
                nc.sync.dma_start(wsb, dt[wname][:, h])
                for c in range(nchunk):
                    ps = psm.tile([P, 256], F32, tag="conv")
                    for j in range(7):
                        nc.tensor.matmul(
                            ps, wsb[:, j, :],
                            x2[:, hp, c * 256 + j:c * 256 + j + 256],
                            start=(j == 0), stop=(j == 6))
                    if bias is not None:   # q: token-major interleave + bias
                        dst = outT[:DK, c * TQ:(c + 1) * TQ].rearrange(
                            "p (n two) -> p two n", two=2)
                        b = bias[hp * DK:(hp + 1) * DK, hc:hc + 1]
                        if h % 2 == 0:
                            nc.vector.tensor_scalar(dst[:, 0, :], ps[0:DK, :],
                                                    descale, b,
                                                    ALU.mult, ALU.add)
                            nc.scalar.activation(dst[:, 1, :], ps[DK:P, :],
                                                 AF.Identity, bias=b,
                                                 scale=descale)
                        else:
                            nc.scalar.activation(dst[:, 0, :], ps[0:DK, :],
                                                 AF.Identity, bias=b,
                                                 scale=descale)
                            nc.vector.tensor_scalar(dst[:, 1, :], ps[DK:P, :],
                                                    descale, b,
                                                    ALU.mult, ALU.add)
                    else:                  # k/v: pair-major contiguous
                        d0 = outT[:DK, c * TQ:c * TQ + 256]
                        d1 = outT[:DK, c * TQ + 256:(c + 1) * TQ]
                        if (h + c) % 2 == 0:
                            nc.vector.tensor_copy(d0, ps[0:DK, :])
                            nc.scalar.activation(d1, ps[DK:P, :], AF.Identity)
                        else:
                            nc.scalar.activation(d0, ps[0:DK, :], AF.Identity)
                            nc.vector.tensor_copy(d1, ps[DK:P, :])

            # ---- Phase N: noisy LNs -> nt2P builds -> q convs ---------------
            lnpN_cm = tc.tile_pool(name="lnpN", bufs=1)
            lnpN = lnpN_cm.__enter__()
            lnall = [lnpN.tile([P, D], BF16, name=f"lnall_{i}")
                     for i in range(6)]  # noisy ln2 tiles
            for r in range(6):
                xt = lnio.tile([P, D], F32, tag="ln_in", bufs=2)
                nc.sync.dma_start(xt, dt["noisyH"][r * P:(r + 1) * P, :])
                if 1 <= r <= 4:
                    ln1 = xres[:, r - 1, :]
                else:
                    ln1 = lnio.tile([P, D], F32, tag="ln1_tmp", bufs=1)
                _ln_apply(nc, small, xt, ln1, eps1_t,
                          affine=(lng_s, lnb_s))
                _ln_apply(nc, small, ln1, lnall[r], eps2_t,
                          apply_eng=("act" if r % 2 else "dve"))

            nt2s = []
            for m in range(8):
                nt2m = bigc.tile([P, 2, NT2W], FP8, name=f"nt2_{m}")
                nt2s.append(nt2m)
                tmn = trans.tile([P, NHW], BF16, tag="tmn", bufs=2)
                for r in range(6):
                    pt = ptp.tile([P, P], BF16, tag="tpbf")
                    nc.tensor.transpose(pt, lnall[r][:, m * P:(m + 1) * P],
                                        ident)
                    dst = tmn[:, r * P:(r + 1) * P]
                    if r % 2 == 0:
                        nc.vector.tensor_scalar(dst, pt, sc_msa[:, m:m + 1],
                                                sh_msa[:, m:m + 1],
                                                ALU.mult, ALU.add)
                    else:
                        nc.scalar.activation(dst, pt, AF.Identity,
                                             bias=sh_msa[:, m:m + 1],
                                             scale=sc_msa[:, m:m + 1])
                for hh in range(2):
                    sl = slice(hh * DK, (hh + 1) * DK)
                    nc.gpsimd.tensor_tensor(nt2m[0:DK, hh, 0:262],
                                            tmn[sl, 122:646:2],
                                            hm_s[sl, 122:646:2], ALU.mult)
                    nc.gpsimd.tensor_tensor(nt2m[DK:P, hh, 0:262],
                                            tmn[sl, 123:647:2],
                                            hm_s[sl, 123:647:2], ALU.mult)
            lnpN_cm.__exit__(None, None, None)

            qTs = []
            for h in range(H):
                qT = bigc.tile([65, TQ], BF16, name=f"qT_{h}")
                nc.vector.memset(qT[64:65, :], 1.0)
                conv(h, "wql", nt2s[h // 2], 1, qT, bias=bq_s,
                     descale=1.0 / WSC)
                qTs.append(qT)

            # ---- Phase C: clean LNs -> ct2P builds (pair-major) -------------
            lnpC_cm = tc.tile_pool(name="lnpC", bufs=1)
            lnpC = lnpC_cm.__enter__()
            clnall = [lnpC.tile([P, D], BF16, name=f"clnall_{i}")
                      for i in range(8)]
            for r in range(8):
                xt = lnio.tile([P, D], F32, tag="ln_in", bufs=2)
                nc.sync.dma_start(xt, dt["clean"][r * P:(r + 1) * P, :])
                _ln_apply(nc, small, xt, clnall[r], eps1_t,
                          apply_eng=("act" if r % 2 else "dve"))
            ct2s = []
            for m in range(8):
                ct2m = bigc.tile([P, 2, CT2W], FP8, name=f"ct2_{m}")
                ct2s.append(ct2m)
                for hh in range(2):
                    nc.gpsimd.memset(ct2m[:, hh, 0:3], 0.0)
                    nc.gpsimd.memset(ct2m[:, hh, 515:CT2W], 0.0)
                tmc = trans.tile([P, T], BF16, tag="tmc", bufs=2)
                for r in range(8):
                    pt = ptp.tile([P, P], BF16, tag="tpbf")
                    nc.tensor.transpose(pt, clnall[r][:, m * P:(m + 1) * P],
                                        ident)
                    dst = tmc[:, r * P:(r + 1) * P]
                    if r % 2 == 0:
                        nc.vector.tensor_scalar(dst, pt, clng_s[:, m:m + 1],
                                                clnb_s[:, m:m + 1],
                                                ALU.mult, ALU.add)
                    else:
                        nc.scalar.activation(dst, pt, AF.Identity,
                                             bias=clnb_s[:, m:m + 1],
                                             scale=clng_s[:, m:m + 1])
                for hh in range(2):
                    sl = slice(hh * DK, (hh + 1) * DK)
                    nc.gpsimd.tensor_copy(ct2m[0:DK, hh, 3:515], tmc[sl, 0::2])
                    nc.gpsimd.tensor_copy(ct2m[DK:P, hh, 3:515], tmc[sl, 1::2])
            lnpC_cm.__exit__(None, None, None)

            qTs = []
            for h in range(H):
                qT = bigc.tile([65, TQ], BF16, name=f"qT_{h}")
                nc.vector.memset(qT[64:65, :], 1.0)
                conv(h, "wql", nt2s[h // 2], 1, qT, bias=bq_s,
                     descale=1.0 / WSC)
                qTs.append(qT)

            # ---- Phase K: per-head k/v conv + cross attention ---------------
            hpool_cm = tc.tile_pool(name="hpool", bufs=2)
            hpool = hpool_cm.__enter__()
            for h in range(H):
                hp, hc = h % 2, h // 2
                qT = qTs[h]
                kT = hpool.tile([65, T], BF16, tag="kT", bufs=2)
                nc.sync.dma_start(kT[64:65, :], dt["maskb"])
                conv(h, "wkl", ct2s[h // 2], 2, kT)
                vT = hpool.tile([DK, T], BF16, tag="vT", bufs=2)
                conv(h, "wvl", ct2s[h // 2], 2, vT)
                # v65: v tokens-on-partitions + 32.0 col (cancels x32 scale)
                v65 = hpool.tile([P, 8, 80], FP8, tag="v65", bufs=2)
                nc.vector.memset(v65[:, :, 64:65], WSC)
                nc.vector.memset(v65[:, :, 65:80], 0.0)
                for c2 in range(4):
                    pt = ptp.tile([P, P], BF16, tag="tpbf")
                    nc.tensor.transpose(pt[:, 0:DK],
                                        vT[:, 2 * c2 * P:(2 * c2 + 1) * P],
                                        ident[:DK, :DK])
                    nc.tensor.transpose(pt[:, DK:P],
                                        vT[:, (2 * c2 + 1) * P:(2 * c2 + 2) * P],
                                        ident[:DK, :DK])
                    nc.vector.tensor_copy(
                        v65[:, 2 * c2:2 * c2 + 2, 0:DK],
                        pt.rearrange("p (two n) -> p two n", two=2))

                # scores (x32): mask rides on kT row 64 * qT ones row;
                # fused scale/exp over 2 chunks at a time -> fp8 pT
                pT = hpool.tile([P, 8, TQ], FP8, tag="pT", bufs=2)
                for g in range(4):
                    ps2 = psc.tile([P, 2, TQ], F32, tag="sc")
                    for i in range(2):
                        cc = 2 * g + i
                        nc.tensor.matmul(ps2[:, i, :],
                                         kT[:, cc * P:(cc + 1) * P], qT,
                                         start=True, stop=True)
                    nc.scalar.activation(pT[:, 2 * g:2 * g + 2, :], ps2,
                                         AF.Exp, scale=0.125 / WSC)

                # PV fp8 DoubleRow: out rows 0:64 = 32*attn, row 64 = 32*denom
                pvt = psc.tile([P, 2, TQ], F32, tag="sc")
                pv = pvt[:, 0, :]
                for c2 in range(4):
                    nc.tensor.matmul(pv[:80, :],
                                     v65[:, 2 * c2:2 * c2 + 2, 0:80],
                                     pT[:, 2 * c2:2 * c2 + 2, :],
                                     start=(c2 == 0), stop=(c2 == 3),
                                     perf_mode=DR)
                linv = trans.tile([1, TQ], F32, tag="linv")
                nc.vector.reciprocal(linv, pv[64:65, :])
                bc_sb = trans.tile([DK, TQ], F32, tag="bcsb", bufs=2)
                nc.gpsimd.partition_broadcast(bc_sb, linv)
                nc.vector.tensor_tensor(attnT[hp * DK:(hp + 1) * DK, hc, :],
                                        pv[0:DK, :], bc_sb, ALU.mult)
            hpool_cm.__exit__(None, None, None)
            psm_cm.__exit__(None, None, None)

        # ---- Phase D: fc projection (fp8 DR) + gate + residual --------------
        fcgs = []
        for m in range(8):
            wt = wpool.tile([P, 8, P], FP8, tag="wt")
            nc.sync.dma_start(wt, dt["fcw"][m])
            ps = psc.tile([P, 2, TQ], F32, tag="sc")
            for k in range(4):
                nc.tensor.matmul(ps[:, 0, :], wt[:, 2 * k:2 * k + 2, :],
                                 attnT[:, 2 * k:2 * k + 2, :],
                                 start=(k == 0), stop=(k == 3), perf_mode=DR)
            fcg = trans.tile([P, TQ], BF16, tag="fcg", bufs=8,
                             name=f"fcg_{m}")
            fcbg = small.tile([P, 1], F32, tag="fcbg", name="fcbg")
            nc.vector.tensor_tensor(fcbg, fcb_s[:, m:m + 1],
                                    g_msa[:, m:m + 1], ALU.mult)
            nc.scalar.activation(fcg, ps[:, 0, :], AF.Identity, bias=fcbg,
                                 scale=g_msa_ds[:, m:m + 1])
            fcgs.append(fcg)
        for j in range(4):
            for m in range(8):
                pt = ptp.tile([P, P], BF16, tag="tpbf")
                nc.tensor.transpose(pt, fcgs[m][:, j * P:(j + 1) * P], ident)
                nc.vector.tensor_tensor(xres[:, j, m * P:(m + 1) * P], pt,
                                        xres[:, j, m * P:(m + 1) * P], ALU.add)

        # ---- Phase E: LN3 + mlp modulation -> n2T ---------------------------
        bigf_cm = tc.tile_pool(name="bigf", bufs=1)
        bigf = bigf_cm.__enter__()
        n2T = bigf.tile([P, 8, TQ], BF16)
        for s in range(4):
            l3 = lnio.tile([P, D], BF16, tag="ln2b")
            _ln_apply(nc, small, xres[:, s, :], l3, eps2_t)
            for m in range(8):
                pt = ptp.tile([P, P], BF16, tag="tpbf")
                nc.tensor.transpose(pt, l3[:, m * P:(m + 1) * P], ident)
                nc.vector.tensor_scalar(n2T[:, m, s * P:(s + 1) * P], pt,
                                        sc_mlp[:, m:m + 1], sh_mlp[:, m:m + 1],
                                        ALU.mult, ALU.add)

        # ---- Phase F: FFN (bf16) --------------------------------------------
        if True:
            ffa = bigf.tile([P, 32, TQ], BF16)
            for m in range(32):
                wt = wpool.tile([P, 8, P], BF16, tag="wtf")
                nc.sync.dma_start(wt, dt["w1t"][m])
                ps = psc.tile([P, 2, TQ], F32, tag="sc")
                for k in range(8):
                    nc.tensor.matmul(ps[:, 0, :], wt[:, k, :], n2T[:, k, :],
                                     start=(k == 0), stop=(k == 7))
                nc.scalar.activation(ffa[:, m, :], ps[:, 0, :],
                                     AF.Gelu_apprx_tanh,
                                     bias=fb1_s[:, m:m + 1])
            ffogs = []
            for m in range(8):
                ps = psc.tile([P, 2, TQ], F32, tag="sc")
                for kq in range(4):
                    wt = wpool.tile([P, 8, P], BF16, tag="wtf")
                    nc.sync.dma_start(wt, dt["w2t"][m, kq])
                    for k8 in range(8):
                        k = kq * 8 + k8
                        nc.tensor.matmul(ps[:, 0, :], wt[:, k8, :],
                                         ffa[:, k, :],
                                         start=(k == 0), stop=(k == 31))
                ffog = trans.tile([P, TQ], BF16, tag="ffog", bufs=8,
                                  name=f"ffog_{m}")
                fbg = small.tile([P, 1], F32, tag="fcbg", name="fbg")
                nc.vector.tensor_tensor(fbg, fb2_s[:, m:m + 1],
                                        g_mlp[:, m:m + 1], ALU.mult)
                nc.scalar.activation(ffog, ps[:, 0, :], AF.Identity, bias=fbg,
                                     scale=g_mlp[:, m:m + 1])
                ffogs.append(ffog)
            for j in range(4):
                for m in range(8):
                    pt = ptp.tile([P, P], BF16, tag="tpbf")
                    nc.tensor.transpose(pt, ffogs[m][:, j * P:(j + 1) * P],
                                        ident)
                    nc.vector.tensor_tensor(xres[:, j, m * P:(m + 1) * P], pt,
                                            xres[:, j, m * P:(m + 1) * P],
                                            ALU.add)
                nc.sync.dma_start(out_ap[j * P:(j + 1) * P, :], xres[:, j, :])
        bigf_cm.__exit__(None, None, None)


# --------------------------- host side --------------------------------------
_NC_CACHE = None


def _prep_conv_w_poly(w):
    # w: (D, DK, KW) grouped conv -> [128, H, 7, 128] fp8 polyphase lhsT *WSC
    wr = (np.asarray(w, np.float32) * WSC).reshape(H, DK, DK, KW)  # h,o,c,tap
    arr = np.zeros((P, H, 7, P), np.float32)
    for j in range(7):
        t = lambda k: wr[:, :, :, k].transpose(2, 0, 1)    # -> [c, h, o]
        arr[0:DK, :, j, 0:DK] = t(2 * j)                   # even out, tap 2j
        if 2 * j + 1 <= 12:
            arr[DK:P, :, j, 0:DK] = t(2 * j + 1)           # even out, 2j+1
        if j >= 1:
            arr[0:DK, :, j, DK:P] = t(2 * j - 1)           # odd out, 2j-1
        arr[DK:P, :, j, DK:P] = t(2 * j)                   # odd out, 2j
    return arr.astype(ml_dtypes.float8_e4m3fn)


def kernel(**inputs):
    global _NC_CACHE
    if _NC_CACHE is None:
        _NC_CACHE = build_nc()
    nc = _NC_CACHE

    f32 = np.float32
    bf = ml_dtypes.bfloat16
    fp8 = ml_dtypes.float8_e4m3fn
    noisy = np.asarray(inputs["noisy_feats"], f32)
    clean = np.asarray(inputs["clean_feats"], f32)
    t = np.asarray(inputs["t"], f32)
    clean_len = np.asarray(inputs["clean_lengths"]).astype(np.int64)

    # AdaLayerNormZero on host (0.02% of FLOPs): emb = silu(t) @ ada_w.T + b
    st = t * (1.0 / (1.0 + np.exp(-t, dtype=f32)))
    emb = st @ np.asarray(inputs["ada_w"], f32).T + np.asarray(inputs["ada_b"], f32)
    sh_msa, sc_msa, g_msa, sh_mlp, sc_mlp, g_mlp = np.split(emb, 6, axis=1)

    wql = _prep_conv_w_poly(inputs["wq"])
    wkl = _prep_conv_w_poly(inputs["wk"])
    wvl = _prep_conv_w_poly(inputs["wv"])
    fcw = (np.asarray(inputs["fc_w"], f32).T * WSC).reshape(8, P, 8, P) \
        .transpose(2, 1, 0, 3).astype(fp8).copy()
    w1t = np.asarray(inputs["ff_w1"], f32).T.reshape(8, P, 32, P) \
        .transpose(2, 1, 0, 3).astype(bf).copy()
    w2t = np.asarray(inputs["ff_w2"], f32).T.reshape(32, P, 8, P) \
        .transpose(2, 0, 1, 3).reshape(8, 4, 8, P, P) \
        .transpose(0, 1, 3, 2, 4).astype(bf).copy()
    # fold v bias into fc bias: fc(attn + bv) = fc(attn) + fc_w @ bv
    fcb_eff = (np.asarray(inputs["fc_b"], f32)
               + np.asarray(inputs["fc_w"], f32) @ np.asarray(inputs["bv"], f32))

    # pair-major t2 permutation: col c*512+par*256+n <-> token c*512+2n+par
    perm = np.empty(T, np.int64)
    for c in range(2):
        for par in range(2):
            base = c * 512 + par * 256
            perm[base:base + 256] = c * 512 + 2 * np.arange(256) + par

    common = dict(
        lng=np.broadcast_to(np.asarray(inputs["ln_noisy_g"], f32), (P, D)).copy(),
        lnb=np.broadcast_to(np.asarray(inputs["ln_noisy_b"], f32), (P, D)).copy(),
        clng=np.asarray(inputs["ln_clean_g"], f32).copy(),
        clnb=np.asarray(inputs["ln_clean_b"], f32).copy(),
        wql=wql, wkl=wkl, wvl=wvl,
        bq=np.asarray(inputs["bq"], f32).copy(),
        fcw=fcw, fcb=fcb_eff.copy(),
        w1t=w1t, fb1=np.asarray(inputs["ff_b1"], f32).copy(),
        w2t=w2t, fb2=np.asarray(inputs["ff_b2"], f32).copy(),
    )

    in_maps = []
    for i in range(8):
        b, half = i // 2, i % 2
        t0 = half * TQ
        noisyH = np.zeros((NHW, D), f32)
        lo, hi = t0 - P, t0 + 640
        clo, chi = max(lo, 0), min(hi, T)
        noisyH[clo - lo:chi - lo] = noisy[b, clo:chi]
        hm = np.zeros((NHW,), f32)
        hm[clo - lo:chi - lo] = 1.0
        maskb = np.where(np.arange(T) >= clean_len[b], NEG, 0.0).astype(f32)
        maskb_pm = maskb[perm][None, :].astype(bf).copy()
        mod = np.stack([sh_msa[b], 1.0 + sc_msa[b], g_msa[b],
                        sh_mlp[b], 1.0 + sc_mlp[b], g_mlp[b],
                        g_msa[b] / WSC, g_mlp[b] / WSC]).astype(f32)
        m = dict(common)
        m.update(noisyH=noisyH, clean=clean[b].copy(),
                 hm=np.broadcast_to(hm, (P, NHW)).astype(bf).copy(),
                 maskb=maskb_pm, mod=mod)
        in_maps.append(m)

    global _LAST_INMAPS
    _LAST_INMAPS = in_maps
    res = run_bass_kernel_spmd(nc, in_maps, core_ids=list(range(8)))
    out = np.empty((B, T, D), f32)
    for i in range(8):
        b, half = i // 2, i % 2
        out[b, half * TQ:(half + 1) * TQ] = res.results[i]["out"]
    return out


_LAST_INMAPS = None


def run_profiled(tmpdir=None):
    """Re-run the last kernel invocation with NTFF tracing; return exec ns."""
    if _NC_CACHE is None or _LAST_INMAPS is None:
        return None
    res = run_bass_kernel_spmd(_NC_CACHE, _LAST_INMAPS,
                               core_ids=list(range(8)), trace=True,
                               tmpdir=tmpdir)
    return res.exec_time_ns


if __name__ == "__main__":
    build_nc()
    print("build ok")


# revision 45
# speedup vs baseline: 1.1097x; 1.0241x over previous
# Trainium2 Bass kernel for nn_CrossAttention_noise (B=4, T1=T2=1024, D=1024,
# H=16, DK=64, K=13, FF=4096), SPMD over 8 NeuronCores.
#
# Sharding: core i handles batch b=i//2 and query-token half t0=(i%2)*512.
# Each core computes its 512 output tokens end-to-end (the K/V convolution
# over the full clean sequence is duplicated between the two cores of a
# batch; no collectives).
#
# Key structure:
#  - QKV grouped convs run "polyphase": even/odd output tokens are separate
#    PE-array columns, so each matmul streams N=256 with a full 128x128 array
#    (2x fewer PE cycles than the shifted-window form).  Conv weights+inputs
#    are fp8e4 (weights prescaled x32); K/V sbuf stores keep the x32 scale
#    and fold it into the softmax exp scale / the v65 ones-column.
#  - K/V token order is "pair-major" (evens then odds per 512-block), which
#    is softmax-invariant; the key-padding mask rides as row 64 of kT and a
#    ones row 64 of qT, so exp needs no per-chunk bias and batches 2 chunks.
#  - PV runs fp8 DoubleRow (contract 256/matmul): pT (exp output) and v65
#    are fp8e4.  fc runs fp8 DoubleRow too (attnT fp8, fcw fp8 x32).
#  - k bias is dropped (softmax-shift invariant), v bias is folded into
#    fc_b on the host (fc_b += fc_w @ bv), q bias stays on-device.
#  - FFN stays bf16 (fp8 there costs ~1.7e-2 rel err; over budget).
#  - Clean-attention path is emitted before the noisy path so PE work
#    (ct2 transposes, k/v convs) starts as soon as the 8 clean LNs finish.
import numpy as np
import ml_dtypes
from contextlib import ExitStack

import concourse.bass as bass
import concourse.mybir as mybir
import concourse.tile as tile
from concourse import bacc
from concourse.bass_utils import run_bass_kernel_spmd
from concourse.masks import make_identity

BF16 = mybir.dt.bfloat16
F32 = mybir.dt.float32
FP8 = mybir.dt.float8e4
AF = mybir.ActivationFunctionType
ALU = mybir.AluOpType
AX = mybir.AxisListType
DR = mybir.MatmulPerfMode.DoubleRow
WSC = 32.0            # fp8 weight prescale (conv weights, fc_w stored *32)

B, T, D, H, DK, KW, FF = 4, 1024, 1024, 16, 64, 13, 4096
TQ = 512          # query tokens per core
NHW = 768         # noisy halo window rows (zero-padded on host)
NT2W = 264        # nt2 pair-major width (valid cols 0..261)
CT2W = 520        # ct2 pair-major width (valid cols 3..514)
P = 128
EPS1, EPS2 = 1e-5, 1e-6
NEG = -1.0e30


def _ln_apply(nc, pool, x, out, eps_ap, affine=None, apply_eng="dve",
              reduce_eng="dve"):
    """out = (x - mean)/sqrt(var + eps) rowwise; x [p, D] in SBUF."""
    p = x.shape[0]
    s = pool.tile([P, 1], F32, tag="ln_s", name="ln_s")
    sq = pool.tile([P, 1], F32, tag="ln_sq", name="ln_sq")
    scr = pool.tile([P, D], BF16, tag="ln_scr", name="ln_scr", bufs=3)[:p]
    if reduce_eng == "dve":
        nc.vector.reduce_sum(s[:p], x, axis=AX.X)
    else:
        scr2 = pool.tile([P, D], BF16, tag="ln_scr", name="ln_scr2",
                         bufs=3)[:p]
        nc.scalar.activation(scr2, x, AF.Identity, accum_out=s[:p])
    nc.scalar.activation(scr, x, AF.Square, accum_out=sq[:p])
    mu = pool.tile([P, 1], F32, tag="ln_mu", name="ln_mu")[:p]
    nc.vector.tensor_scalar_mul(mu, s[:p], 1.0 / D)
    musq = pool.tile([P, 1], F32, tag="ln_musq", name="ln_musq")[:p]
    nc.vector.tensor_tensor(musq, mu, mu, ALU.mult)
    var = pool.tile([P, 1], F32, tag="ln_var", name="ln_var")[:p]
    nc.vector.tensor_scalar(var, sq[:p], 1.0 / D, musq, ALU.mult, ALU.subtract)
    std = pool.tile([P, 1], F32, tag="ln_std", name="ln_std")[:p]
    nc.scalar.activation(std, var, AF.Sqrt, bias=eps_ap[:p])
    rstd = pool.tile([P, 1], F32, tag="ln_rstd", name="ln_rstd")[:p]
    nc.vector.reciprocal(rstd, std)
    beta = pool.tile([P, 1], F32, tag="ln_beta", name="ln_beta")[:p]
    nc.vector.tensor_tensor(beta, mu, rstd, ALU.mult)
    nc.vector.tensor_scalar_mul(beta, beta, -1.0)
    if affine is not None:
        g, b = affine
        negmu = pool.tile([P, 1], F32, tag="ln_negmu", name="ln_negmu")[:p]
        nc.vector.tensor_scalar_mul(negmu, mu, -1.0)
        nc.vector.scalar_tensor_tensor(out, x, negmu, g, ALU.add, ALU.mult)
        nc.vector.scalar_tensor_tensor(out, out, rstd, b, ALU.mult, ALU.add)
    elif apply_eng == "act":
        nc.scalar.activation(out, x, AF.Identity, bias=beta, scale=rstd)
    else:
        nc.vector.tensor_scalar(out, x, rstd, beta, ALU.mult, ALU.add)


def build_nc():
    nc = bacc.Bacc("TRN2", target_bir_lowering=False, debug=False,
                   num_devices=8)
    dt = {}

    def din(name, shape, dtype):
        dt[name] = nc.dram_tensor(name, list(shape), dtype,
                                  kind="ExternalInput").ap()

    din("noisyH", (NHW, D), BF16)          # rows [t0-128, t0+640), zero padded
    din("clean", (T, D), BF16)
    din("hm", (P, NHW), BF16)              # halo-token validity (rows equal)
    din("maskb", (1, T), BF16)            # 0 / -1e30 key mask, PAIR-MAJOR
    din("mod", (8, D), F32)               # sh/sc/g msa+mlp rows + g_msa/WSC
    din("lng", (P, D), F32)               # ln_noisy_g broadcast to 128 rows
    din("lnb", (P, D), F32)
    din("clng", (D,), F32)
    din("clnb", (D,), F32)
    din("wql", (P, H, 7, P), FP8)         # polyphase conv lhsT, *WSC
    din("wkl", (P, H, 7, P), FP8)
    din("wvl", (P, H, 7, P), FP8)
    din("bq", (D,), F32)
    din("fcw", (8, P, 8, P), FP8)         # fc_w.T*WSC tiles [mc][kp][ko][mj]
    din("fcb", (D,), F32)                 # fc_b + fc_w @ bv (host-folded)
    din("w1t", (32, P, 8, P), BF16)       # ff_w1.T tiles [mc][kp][ko][mj]
    din("fb1", (FF,), F32)
    din("w2t", (8, 4, P, 8, P), BF16)     # ff_w2.T tiles [mc][kq][kp][k8][mj]
    din("fb2", (D,), F32)
    out_ap = nc.dram_tensor("out", [TQ, D], F32, kind="ExternalOutput").ap()

    with tile.TileContext(nc) as tc:
        _emit(tc, dt, out_ap)
    nc.compile()
    return nc


def _emit(tc, dt, out_ap):
    nc = tc.nc
    with ExitStack() as ctx:
        const = ctx.enter_context(tc.tile_pool(name="const", bufs=1))
        small = ctx.enter_context(tc.tile_pool(name="small", bufs=3))
        lnio = ctx.enter_context(tc.tile_pool(name="lnio", bufs=3))
        big = ctx.enter_context(tc.tile_pool(name="bigsb", bufs=1))
        trans = ctx.enter_context(tc.tile_pool(name="trans", bufs=3))
        wpool = ctx.enter_context(tc.tile_pool(name="wstream", bufs=6))
        psc = ctx.enter_context(tc.tile_pool(name="psc", bufs=2, space="PSUM"))
        ptp = ctx.enter_context(tc.tile_pool(name="ptp", bufs=2, space="PSUM"))

        ident = const.tile([P, P], BF16)
        make_identity(nc, ident)
        eps1_t = const.tile([P, 1], F32)
        nc.vector.memset(eps1_t, EPS1)
        eps2_t = const.tile([P, 1], F32)
        nc.vector.memset(eps2_t, EPS2)

        def chanvec(name, w=8):
            t = const.tile([P, w], F32, tag=f"cv_{name}")
            nc.sync.dma_start(t, dt[name].rearrange("(m p) -> p m", p=P))
            return t

        bq_s = chanvec("bq")
        fcb_s, fb2_s = chanvec("fcb"), chanvec("fb2")
        clng_s, clnb_s = chanvec("clng"), chanvec("clnb")
        fb1_s = chanvec("fb1", 32)
        mod_s = const.tile([P, 8, 8], F32)
        for s in range(8):
            nc.sync.dma_start(mod_s[:, s, :],
                              dt["mod"][s].rearrange("(m p) -> p m", p=P))
        sh_msa, sc_msa, g_msa = mod_s[:, 0, :], mod_s[:, 1, :], mod_s[:, 2, :]
        sh_mlp, sc_mlp, g_mlp = mod_s[:, 3, :], mod_s[:, 4, :], mod_s[:, 5, :]
        g_msa_ds = mod_s[:, 6, :]
        hm_s = const.tile([P, NHW], BF16)
        nc.sync.dma_start(hm_s, dt["hm"])
        lng_s = const.tile([P, D], F32)
        nc.sync.dma_start(lng_s, dt["lng"])
        lnb_s = const.tile([P, D], F32)
        nc.sync.dma_start(lnb_s, dt["lnb"])

        xres = big.tile([P, 4, D], F32)        # LN1 rows [t0, t0+512); later x
        attnT = big.tile([P, 8, TQ], FP8)      # concat_h(attn_h), chan-major

        with tc.tile_pool(name="bigc", bufs=1) as bigc:
            psm_cm = tc.tile_pool(name="psm", bufs=2, space="PSUM")
            psm = psm_cm.__enter__()

            def conv(h, wname, x2, nchunk, outT, bias=None, descale=None):
                """Polyphase grouped conv for head h into outT[0:64, :]."""
                hp, hc = h % 2, h // 2
                wsb = wpool.tile([P, 7, P], FP8, tag="convw", bufs=4,
                                 name=f"w_{wname}_{h}")
                nc.sync.dma_start(wsb, dt[wname][:, h])
                for c in range(nchunk):
                    ps = psm.tile([P, 256], F32, tag="conv")
                    for j in range(7):
                        nc.tensor.matmul(
                            ps, wsb[:, j, :],
                            x2[:, hp, c * 256 + j:c * 256 + j + 256],
                            start=(j == 0), stop=(j == 6))
                    if bias is not None:   # q: token-major interleave + bias
                        dst = outT[:DK, c * TQ:(c + 1) * TQ].rearrange(
                            "p (n two) -> p two n", two=2)
                        b = bias[hp * DK:(hp + 1) * DK, hc:hc + 1]
                        if h % 2 == 0:
                            nc.vector.tensor_scalar(dst[:, 0, :], ps[0:DK, :],
                                                    descale, b,
                                                    ALU.mult, ALU.add)
                            nc.scalar.activation(dst[:, 1, :], ps[DK:P, :],
                                                 AF.Identity, bias=b,
                                                 scale=descale)
                        else:
                            nc.scalar.activation(dst[:, 0, :], ps[0:DK, :],
                                                 AF.Identity, bias=b,
                                                 scale=descale)
                            nc.vector.tensor_scalar(dst[:, 1, :], ps[DK:P, :],
                                                    descale, b,
                                                    ALU.mult, ALU.add)
                    else:                  # k/v: pair-major contiguous
                        d0 = outT[:DK, c * TQ:c * TQ + 256]
                        d1 = outT[:DK, c * TQ + 256:(c + 1) * TQ]
                        if (h + c) % 2 == 0:
                            nc.vector.tensor_copy(d0, ps[0:DK, :])
                            nc.scalar.activation(d1, ps[DK:P, :], AF.Identity)
                        else:
                            nc.scalar.activation(d0, ps[0:DK, :], AF.Identity)
                            nc.vector.tensor_copy(d1, ps[DK:P, :])

            # ---- Phase N: noisy LNs -> nt2P builds -> q convs ---------------
            lnpN_cm = tc.tile_pool(name="lnpN", bufs=1)
            lnpN = lnpN_cm.__enter__()
            lnall = [lnpN.tile([P, D], BF16, name=f"lnall_{i}")
                     for i in range(6)]  # noisy ln2 tiles
            for r in range(6):
                xt = lnio.tile([P, D], BF16, tag="ln_in", bufs=2)
                nc.sync.dma_start(xt, dt["noisyH"][r * P:(r + 1) * P, :])
                if 1 <= r <= 4:
                    ln1 = xres[:, r - 1, :]
                else:
                    ln1 = lnio.tile([P, D], F32, tag="ln1_tmp", bufs=1)
                _ln_apply(nc, small, xt, ln1, eps1_t,
                          affine=(lng_s, lnb_s))
                _ln_apply(nc, small, ln1, lnall[r], eps2_t,
                          apply_eng=("act" if r % 2 else "dve"),
                          reduce_eng="act")

            nt2s = []
            for m in range(8):
                nt2m = bigc.tile([P, 2, NT2W], FP8, name=f"nt2_{m}")
                nt2s.append(nt2m)
                tmn = trans.tile([P, NHW], BF16, tag="tmn", bufs=2)
                for r in range(6):
                    pt = ptp.tile([P, P], BF16, tag="tpbf")
                    nc.tensor.transpose(pt, lnall[r][:, m * P:(m + 1) * P],
                                        ident)
                    dst = tmn[:, r * P:(r + 1) * P]
                    if r % 2 == 0:
                        nc.vector.tensor_scalar(dst, pt, sc_msa[:, m:m + 1],
                                                sh_msa[:, m:m + 1],
                                                ALU.mult, ALU.add)
                    else:
                        nc.scalar.activation(dst, pt, AF.Identity,
                                             bias=sh_msa[:, m:m + 1],
                                             scale=sc_msa[:, m:m + 1])
                for hh in range(2):
                    sl = slice(hh * DK, (hh + 1) * DK)
                    nc.gpsimd.tensor_tensor(nt2m[0:DK, hh, 0:262],
                                            tmn[sl, 122:646:2],
                                            hm_s[sl, 122:646:2], ALU.mult)
                    nc.gpsimd.tensor_tensor(nt2m[DK:P, hh, 0:262],
                                            tmn[sl, 123:647:2],
                                            hm_s[sl, 123:647:2], ALU.mult)
            lnpN_cm.__exit__(None, None, None)

            qTs = []
            for h in range(H):
                qT = bigc.tile([65, TQ], BF16, name=f"qT_{h}")
                nc.vector.memset(qT[64:65, :], 1.0)
                conv(h, "wql", nt2s[h // 2], 1, qT, bias=bq_s,
                     descale=1.0 / WSC)
                qTs.append(qT)

            # ---- Phase C: clean LNs -> ct2P builds (pair-major) -------------
            lnpC_cm = tc.tile_pool(name="lnpC", bufs=1)
            lnpC = lnpC_cm.__enter__()
            clnall = [lnpC.tile([P, D], BF16, name=f"clnall_{i}")
                      for i in range(8)]
            for r in range(8):
                xt = lnio.tile([P, D], BF16, tag="ln_in", bufs=2)
                nc.sync.dma_start(xt, dt["clean"][r * P:(r + 1) * P, :])
                _ln_apply(nc, small, xt, clnall[r], eps1_t,
                          apply_eng=("act" if r % 2 else "dve"),
                          reduce_eng=("act" if r % 2 else "dve"))
            ct2s = []
            for m in range(8):
                ct2m = bigc.tile([P, 2, CT2W], FP8, name=f"ct2_{m}")
                ct2s.append(ct2m)
                for hh in range(2):
                    nc.gpsimd.memset(ct2m[:, hh, 0:3], 0.0)
                    nc.gpsimd.memset(ct2m[:, hh, 515:CT2W], 0.0)
                tmc = trans.tile([P, T], BF16, tag="tmc", bufs=2)
                for r in range(8):
                    pt = ptp.tile([P, P], BF16, tag="tpbf")
                    nc.tensor.transpose(pt, clnall[r][:, m * P:(m + 1) * P],
                                        ident)
                    dst = tmc[:, r * P:(r + 1) * P]
                    if r % 2 == 0:
                        nc.vector.tensor_scalar(dst, pt, clng_s[:, m:m + 1],
                                                clnb_s[:, m:m + 1],
                                                ALU.mult, ALU.add)
                    else:
                        nc.scalar.activation(dst, pt, AF.Identity,
                                             bias=clnb_s[:, m:m + 1],
                                             scale=clng_s[:, m:m + 1])
                for hh in range(2):
                    sl = slice(hh * DK, (hh + 1) * DK)
                    i = (m * 2 + hh) % 4
                    if i == 0:
                        nc.gpsimd.tensor_copy(ct2m[0:DK, hh, 3:515],
                                              tmc[sl, 0::2])
                        nc.vector.tensor_copy(ct2m[DK:P, hh, 3:515],
                                              tmc[sl, 1::2])
                    elif i == 1:
                        nc.scalar.activation(ct2m[0:DK, hh, 3:515],
                                             tmc[sl, 0::2], AF.Identity)
                        nc.gpsimd.tensor_copy(ct2m[DK:P, hh, 3:515],
                                              tmc[sl, 1::2])
                    elif i == 2:
                        nc.vector.tensor_copy(ct2m[0:DK, hh, 3:515],
                                              tmc[sl, 0::2])
                        nc.gpsimd.tensor_copy(ct2m[DK:P, hh, 3:515],
                                              tmc[sl, 1::2])
                    else:
                        nc.gpsimd.tensor_copy(ct2m[0:DK, hh, 3:515],
                                              tmc[sl, 0::2])
                        nc.scalar.activation(ct2m[DK:P, hh, 3:515],
                                             tmc[sl, 1::2], AF.Identity)
            lnpC_cm.__exit__(None, None, None)

            qTs = []
            for h in range(H):
                qT = bigc.tile([65, TQ], BF16, name=f"qT_{h}")
                nc.vector.memset(qT[64:65, :], 1.0)
                conv(h, "wql", nt2s[h // 2], 1, qT, bias=bq_s,
                     descale=1.0 / WSC)
                qTs.append(qT)

            # ---- Phase K: per-head k/v conv + cross attention ---------------
            hpool_cm = tc.tile_pool(name="hpool", bufs=2)
            hpool = hpool_cm.__enter__()
            for h in range(H):
                hp, hc = h % 2, h // 2
                qT = qTs[h]
                kT = hpool.tile([65, T], BF16, tag="kT", bufs=2)
                nc.sync.dma_start(kT[64:65, :], dt["maskb"])
                conv(h, "wkl", ct2s[h // 2], 2, kT)
                vT = hpool.tile([DK, T], BF16, tag="vT", bufs=2)
                conv(h, "wvl", ct2s[h // 2], 2, vT)
                # v65: v tokens-on-partitions + 32.0 col (cancels x32 scale)
                v65 = hpool.tile([P, 8, 80], FP8, tag="v65", bufs=2)
                nc.vector.memset(v65[:, :, 64:65], WSC)
                nc.vector.memset(v65[:, :, 65:80], 0.0)
                for c2 in range(4):
                    pt = ptp.tile([P, P], BF16, tag="tpbf")
                    nc.tensor.transpose(pt[:, 0:DK],
                                        vT[:, 2 * c2 * P:(2 * c2 + 1) * P],
                                        ident[:DK, :DK])
                    nc.tensor.transpose(pt[:, DK:P],
                                        vT[:, (2 * c2 + 1) * P:(2 * c2 + 2) * P],
                                        ident[:DK, :DK])
                    nc.vector.tensor_copy(
                        v65[:, 2 * c2:2 * c2 + 2, 0:DK],
                        pt.rearrange("p (two n) -> p two n", two=2))

                # scores (x32): mask rides on kT row 64 * qT ones row;
                # fused scale/exp over 2 chunks at a time -> fp8 pT
                pT = hpool.tile([P, 8, TQ], FP8, tag="pT", bufs=2)
                for g in range(4):
                    ps2 = psc.tile([P, 2, TQ], F32, tag="sc")
                    for i in range(2):
                        cc = 2 * g + i
                        nc.tensor.matmul(ps2[:, i, :],
                                         kT[:, cc * P:(cc + 1) * P], qT,
                                         start=True, stop=True)
                    nc.scalar.activation(pT[:, 2 * g:2 * g + 2, :], ps2,
                                         AF.Exp, scale=0.125 / WSC)

                # PV fp8 DoubleRow: out rows 0:64 = 32*attn, row 64 = 32*denom
                pvt = psc.tile([P, 2, TQ], F32, tag="sc")
                pv = pvt[:, 0, :]
                for c2 in range(4):
                    nc.tensor.matmul(pv[:80, :],
                                     v65[:, 2 * c2:2 * c2 + 2, 0:80],
                                     pT[:, 2 * c2:2 * c2 + 2, :],
                                     start=(c2 == 0), stop=(c2 == 3),
                                     perf_mode=DR)
                linv = trans.tile([1, TQ], F32, tag="linv")
                nc.vector.reciprocal(linv, pv[64:65, :])
                bc_sb = trans.tile([DK, TQ], F32, tag="bcsb", bufs=2)
                nc.gpsimd.partition_broadcast(bc_sb, linv)
                nc.vector.tensor_tensor(attnT[hp * DK:(hp + 1) * DK, hc, :],
                                        pv[0:DK, :], bc_sb, ALU.mult)
            hpool_cm.__exit__(None, None, None)
            psm_cm.__exit__(None, None, None)

        # ---- Phase D: fc projection (fp8 DR) + gate + residual --------------
        fcgs = []
        for m in range(8):
            wt = wpool.tile([P, 8, P], FP8, tag="wt")
            nc.sync.dma_start(wt, dt["fcw"][m])
            ps = psc.tile([P, 2, TQ], F32, tag="sc")
            for k in range(4):
                nc.tensor.matmul(ps[:, 0, :], wt[:, 2 * k:2 * k + 2, :],
                                 attnT[:, 2 * k:2 * k + 2, :],
                                 start=(k == 0), stop=(k == 3), perf_mode=DR)
            fcg = trans.tile([P, TQ], BF16, tag="fcg", bufs=8,
                             name=f"fcg_{m}")
            fcbg = small.tile([P, 1], F32, tag="fcbg", name="fcbg")
            nc.vector.tensor_tensor(fcbg, fcb_s[:, m:m + 1],
                                    g_msa[:, m:m + 1], ALU.mult)
            nc.scalar.activation(fcg, ps[:, 0, :], AF.Identity, bias=fcbg,
                                 scale=g_msa_ds[:, m:m + 1])
            fcgs.append(fcg)
        for j in range(4):
            for m in range(8):
                pt = ptp.tile([P, P], BF16, tag="tpbf")
                nc.tensor.transpose(pt, fcgs[m][:, j * P:(j + 1) * P], ident)
                nc.vector.tensor_tensor(xres[:, j, m * P:(m + 1) * P], pt,
                                        xres[:, j, m * P:(m + 1) * P],
                                        ALU.add)

        # ---- Phase E: LN3 + mlp modulation -> n2T ---------------------------
        bigf_cm = tc.tile_pool(name="bigf", bufs=1)
        bigf = bigf_cm.__enter__()
        n2T = bigf.tile([P, 8, TQ], BF16)
        for s in range(4):
            l3 = lnio.tile([P, D], BF16, tag="ln2b")
            _ln_apply(nc, small, xres[:, s, :], l3, eps2_t,
                      apply_eng=("act" if s % 2 else "dve"),
                      reduce_eng="act")
            for m in range(8):
                pt = ptp.tile([P, P], BF16, tag="tpbf")
                nc.tensor.transpose(pt, l3[:, m * P:(m + 1) * P], ident)
                nc.vector.tensor_scalar(n2T[:, m, s * P:(s + 1) * P], pt,
                                        sc_mlp[:, m:m + 1], sh_mlp[:, m:m + 1],
                                        ALU.mult, ALU.add)

        # ---- Phase F: FFN (bf16) --------------------------------------------
        if True:
            ffa = bigf.tile([P, 32, TQ], BF16)
            for m in range(32):
                wt = wpool.tile([P, 8, P], BF16, tag="wtf")
                nc.sync.dma_start(wt, dt["w1t"][m])
                ps = psc.tile([P, 2, TQ], F32, tag="sc")
                for k in range(8):
                    nc.tensor.matmul(ps[:, 0, :], wt[:, k, :], n2T[:, k, :],
                                     start=(k == 0), stop=(k == 7))
                nc.scalar.activation(ffa[:, m, :], ps[:, 0, :],
                                     AF.Gelu_apprx_tanh,
                                     bias=fb1_s[:, m:m + 1])
            for m in range(8):
                ps = psc.tile([P, 2, TQ], F32, tag="sc")
                for kq in range(4):
                    wt = wpool.tile([P, 8, P], BF16, tag="wtf")
                    nc.sync.dma_start(wt, dt["w2t"][m, kq])
                    for k8 in range(8):
                        k = kq * 8 + k8
                        nc.tensor.matmul(ps[:, 0, :], wt[:, k8, :],
                                         ffa[:, k, :],
                                         start=(k == 0), stop=(k == 31))
                ffog = trans.tile([P, TQ], BF16, tag="ffog", bufs=2)
                fbg = small.tile([P, 1], F32, tag="fcbg", name="fbg")
                nc.vector.tensor_tensor(fbg, fb2_s[:, m:m + 1],
                                        g_mlp[:, m:m + 1], ALU.mult)
                nc.scalar.activation(ffog, ps[:, 0, :], AF.Identity, bias=fbg,
                                     scale=g_mlp[:, m:m + 1])
                for j in range(4):
                    pt = ptp.tile([P, P], BF16, tag="tpbf")
                    nc.tensor.transpose(pt, ffog[:, j * P:(j + 1) * P], ident)
                    nc.vector.tensor_tensor(xres[:, j, m * P:(m + 1) * P], pt,
                                            xres[:, j, m * P:(m + 1) * P],
                                            ALU.add)
        bigf_cm.__exit__(None, None, None)

        for s in range(4):
            nc.sync.dma_start(out_ap[s * P:(s + 1) * P, :], xres[:, s, :])


# --------------------------- host side --------------------------------------
_NC_CACHE = None


def _prep_conv_w_poly(w):
    # w: (D, DK, KW) grouped conv -> [128, H, 7, 128] fp8 polyphase lhsT *WSC
    wr = (np.asarray(w, np.float32) * WSC).reshape(H, DK, DK, KW)  # h,o,c,tap
    arr = np.zeros((P, H, 7, P), np.float32)
    for j in range(7):
        t = lambda k: wr[:, :, :, k].transpose(2, 0, 1)    # -> [c, h, o]
        arr[0:DK, :, j, 0:DK] = t(2 * j)                   # even out, tap 2j
        if 2 * j + 1 <= 12:
            arr[DK:P, :, j, 0:DK] = t(2 * j + 1)           # even out, 2j+1
        if j >= 1:
            arr[0:DK, :, j, DK:P] = t(2 * j - 1)           # odd out, 2j-1
        arr[DK:P, :, j, DK:P] = t(2 * j)                   # odd out, 2j
    return arr.astype(ml_dtypes.float8_e4m3fn)


def kernel(**inputs):
    global _NC_CACHE
    if _NC_CACHE is None:
        _NC_CACHE = build_nc()
    nc = _NC_CACHE

    f32 = np.float32
    bf = ml_dtypes.bfloat16
    fp8 = ml_dtypes.float8_e4m3fn
    noisy = np.asarray(inputs["noisy_feats"], f32)
    clean = np.asarray(inputs["clean_feats"], f32)
    t = np.asarray(inputs["t"], f32)
    clean_len = np.asarray(inputs["clean_lengths"]).astype(np.int64)

    # AdaLayerNormZero on host (0.02% of FLOPs): emb = silu(t) @ ada_w.T + b
    st = t * (1.0 / (1.0 + np.exp(-t, dtype=f32)))
    emb = st @ np.asarray(inputs["ada_w"], f32).T + np.asarray(inputs["ada_b"], f32)
    sh_msa, sc_msa, g_msa, sh_mlp, sc_mlp, g_mlp = np.split(emb, 6, axis=1)

    wql = _prep_conv_w_poly(inputs["wq"])
    wkl = _prep_conv_w_poly(inputs["wk"])
    wvl = _prep_conv_w_poly(inputs["wv"])
    fcw = (np.asarray(inputs["fc_w"], f32).T * WSC).reshape(8, P, 8, P) \
        .transpose(2, 1, 0, 3).astype(fp8).copy()
    w1t = np.asarray(inputs["ff_w1"], f32).T.reshape(8, P, 32, P) \
        .transpose(2, 1, 0, 3).astype(bf).copy()
    w2t = np.asarray(inputs["ff_w2"], f32).T.reshape(32, P, 8, P) \
        .transpose(2, 0, 1, 3).reshape(8, 4, 8, P, P) \
        .transpose(0, 1, 3, 2, 4).astype(bf).copy()
    # fold v bias into fc bias: fc(attn + bv) = fc(attn) + fc_w @ bv
    fcb_eff = (np.asarray(inputs["fc_b"], f32)
               + np.asarray(inputs["fc_w"], f32) @ np.asarray(inputs["bv"], f32))

    # pair-major t2 permutation: col c*512+par*256+n <-> token c*512+2n+par
    perm = np.empty(T, np.int64)
    for c in range(2):
        for par in range(2):
            base = c * 512 + par * 256
            perm[base:base + 256] = c * 512 + 2 * np.arange(256) + par

    common = dict(
        lng=np.broadcast_to(np.asarray(inputs["ln_noisy_g"], f32), (P, D)).copy(),
        lnb=np.broadcast_to(np.asarray(inputs["ln_noisy_b"], f32), (P, D)).copy(),
        clng=np.asarray(inputs["ln_clean_g"], f32).copy(),
        clnb=np.asarray(inputs["ln_clean_b"], f32).copy(),
        wql=wql, wkl=wkl, wvl=wvl,
        bq=np.asarray(inputs["bq"], f32).copy(),
        fcw=fcw, fcb=fcb_eff.copy(),
        w1t=w1t, fb1=np.asarray(inputs["ff_b1"], f32).copy(),
        w2t=w2t, fb2=np.asarray(inputs["ff_b2"], f32).copy(),
    )

    in_maps = []
    for i in range(8):
        b, half = i // 2, i % 2
        t0 = half * TQ
        noisyH = np.zeros((NHW, D), bf)
        lo, hi = t0 - P, t0 + 640
        clo, chi = max(lo, 0), min(hi, T)
        noisyH[clo - lo:chi - lo] = noisy[b, clo:chi].astype(bf)
        hm = np.zeros((NHW,), f32)
        hm[clo - lo:chi - lo] = 1.0
        maskb = np.where(np.arange(T) >= clean_len[b], NEG, 0.0).astype(f32)
        maskb_pm = maskb[perm][None, :].astype(bf).copy()
        mod = np.stack([sh_msa[b], 1.0 + sc_msa[b], g_msa[b],
                        sh_mlp[b], 1.0 + sc_mlp[b], g_mlp[b],
                        g_msa[b] / WSC, g_mlp[b] / WSC]).astype(f32)
        m = dict(common)
        m.update(noisyH=noisyH, clean=clean[b].astype(bf).copy(),
                 hm=np.broadcast_to(hm, (P, NHW)).astype(bf).copy(),
                 maskb=maskb_pm, mod=mod)
        in_maps.append(m)

    global _LAST_INMAPS
    _LAST_INMAPS = in_maps
    res = run_bass_kernel_spmd(nc, in_maps, core_ids=list(range(8)))
    out = np.empty((B, T, D), f32)
    for i in range(8):
        b, half = i // 2, i % 2
        out[b, half * TQ:(half + 1) * TQ] = res.results[i]["out"]
    return out


_LAST_INMAPS = None


def run_profiled(tmpdir=None):
    """Re-run the last kernel invocation with NTFF tracing; return exec ns."""
    if _NC_CACHE is None or _LAST_INMAPS is None:
        return None
    res = run_bass_kernel_spmd(_NC_CACHE, _LAST_INMAPS,
                               core_ids=list(range(8)), trace=True,
                               tmpdir=tmpdir)
    return res.exec_time_ns


if __name__ == "__main__":
    build_nc()
    print("build ok")


# revision 47
# speedup vs baseline: 1.1485x; 1.0350x over previous
# Trainium2 Bass kernel for nn_CrossAttention_noise (B=4, T1=T2=1024, D=1024,
# H=16, DK=64, K=13, FF=4096), SPMD over 8 NeuronCores.
#
# Sharding: core i handles batch b=i//2 and query-token half t0=(i%2)*512.
# Each core computes its 512 output tokens end-to-end (the K/V convolution
# over the full clean sequence is duplicated between the two cores of a
# batch; no collectives).
#
# Key structure:
#  - QKV grouped convs run "polyphase": even/odd output tokens are separate
#    PE-array columns, so each matmul streams N=256 with a full 128x128 array
#    (2x fewer PE cycles than the shifted-window form).  Conv weights+inputs
#    are fp8e4 (weights prescaled x32); K/V sbuf stores keep the x32 scale
#    and fold it into the softmax exp scale / the v65 ones-column.
#  - K/V token order is "pair-major" (evens then odds per 512-block), which
#    is softmax-invariant; the key-padding mask rides as row 64 of kT and a
#    ones row 64 of qT, so exp needs no per-chunk bias and batches 2 chunks.
#  - PV runs fp8 DoubleRow (contract 256/matmul): pT (exp output) and v65
#    are fp8e4.  fc runs fp8 DoubleRow too (attnT fp8, fcw fp8 x32).
#  - k bias is dropped (softmax-shift invariant), v bias is folded into
#    fc_b on the host (fc_b += fc_w @ bv), q bias stays on-device.
#  - FFN stays bf16 (fp8 there costs ~1.7e-2 rel err; over budget).
#  - Clean-attention path is emitted before the noisy path so PE work
#    (ct2 transposes, k/v convs) starts as soon as the 8 clean LNs finish.
import numpy as np
import ml_dtypes
from contextlib import ExitStack

import concourse.bass as bass
import concourse.mybir as mybir
import concourse.tile as tile
from concourse import bacc
from concourse.bass_utils import run_bass_kernel_spmd
from concourse.masks import make_identity

BF16 = mybir.dt.bfloat16
F32 = mybir.dt.float32
FP8 = mybir.dt.float8e4
AF = mybir.ActivationFunctionType
ALU = mybir.AluOpType
AX = mybir.AxisListType
DR = mybir.MatmulPerfMode.DoubleRow
WSC = 32.0            # fp8 weight prescale (conv weights, fc_w stored *32)

B, T, D, H, DK, KW, FF = 4, 1024, 1024, 16, 64, 13, 4096
TQ = 512          # query tokens per core
NHW = 768         # noisy halo window rows (zero-padded on host)
NT2W = 264        # nt2 pair-major width (valid cols 0..261)
CT2W = 520        # ct2 pair-major width (valid cols 3..514)
P = 128
EPS1, EPS2 = 1e-5, 1e-6
NEG = -1.0e30


def _ln_apply(nc, pool, x, out, eps_ap, affine=None, apply_eng="dve",
              reduce_eng="dve"):
    """out = (x - mean)/sqrt(var + eps) rowwise; x [p, D] in SBUF."""
    p = x.shape[0]
    s = pool.tile([P, 1], F32, tag="ln_s", name="ln_s")
    sq = pool.tile([P, 1], F32, tag="ln_sq", name="ln_sq")
    scr = pool.tile([P, D], BF16, tag="ln_scr", name="ln_scr", bufs=3)[:p]
    if reduce_eng == "dve":
        nc.vector.reduce_sum(s[:p], x, axis=AX.X)
    else:
        scr2 = pool.tile([P, D], BF16, tag="ln_scr", name="ln_scr2",
                         bufs=3)[:p]
        nc.scalar.activation(scr2, x, AF.Identity, accum_out=s[:p])
    nc.scalar.activation(scr, x, AF.Square, accum_out=sq[:p])
    mu = pool.tile([P, 1], F32, tag="ln_mu", name="ln_mu")[:p]
    nc.vector.tensor_scalar_mul(mu, s[:p], 1.0 / D)
    musq = pool.tile([P, 1], F32, tag="ln_musq", name="ln_musq")[:p]
    nc.vector.tensor_tensor(musq, mu, mu, ALU.mult)
    var = pool.tile([P, 1], F32, tag="ln_var", name="ln_var")[:p]
    nc.vector.tensor_scalar(var, sq[:p], 1.0 / D, musq, ALU.mult, ALU.subtract)
    std = pool.tile([P, 1], F32, tag="ln_std", name="ln_std")[:p]
    nc.scalar.activation(std, var, AF.Sqrt, bias=eps_ap[:p])
    rstd = pool.tile([P, 1], F32, tag="ln_rstd", name="ln_rstd")[:p]
    nc.vector.reciprocal(rstd, std)
    beta = pool.tile([P, 1], F32, tag="ln_beta", name="ln_beta")[:p]
    nc.vector.tensor_tensor(beta, mu, rstd, ALU.mult)
    nc.vector.tensor_scalar_mul(beta, beta, -1.0)
    if affine is not None:
        g, b = affine
        negmu = pool.tile([P, 1], F32, tag="ln_negmu", name="ln_negmu")[:p]
        nc.vector.tensor_scalar_mul(negmu, mu, -1.0)
        nc.vector.scalar_tensor_tensor(out, x, negmu, g, ALU.add, ALU.mult)
        nc.vector.scalar_tensor_tensor(out, out, rstd, b, ALU.mult, ALU.add)
    elif apply_eng == "act":
        nc.scalar.activation(out, x, AF.Identity, bias=beta, scale=rstd)
    else:
        nc.vector.tensor_scalar(out, x, rstd, beta, ALU.mult, ALU.add)


def build_nc():
    nc = bacc.Bacc("TRN2", target_bir_lowering=False, debug=False,
                   num_devices=8)
    dt = {}

    def din(name, shape, dtype):
        dt[name] = nc.dram_tensor(name, list(shape), dtype,
                                  kind="ExternalInput").ap()

    din("noisyH", (NHW, D), BF16)          # rows [t0-128, t0+640), zero padded
    din("clean", (T, D), BF16)
    din("hm", (P, NHW), BF16)              # halo-token validity (rows equal)
    din("maskb", (1, T), BF16)            # 0 / -1e30 key mask, PAIR-MAJOR
    din("mod", (8, D), F32)               # sh/sc/g msa+mlp rows + g_msa/WSC
    din("lng", (P, D), F32)               # ln_noisy_g broadcast to 128 rows
    din("lnb", (P, D), F32)
    din("clng", (D,), F32)
    din("clnb", (D,), F32)
    din("wql", (P, H, 7, P), FP8)         # polyphase conv lhsT, *WSC
    din("wkl", (P, H, 7, P), FP8)
    din("wvl", (P, H, 7, P), FP8)
    din("bq", (D,), F32)
    din("fcw", (8, P, 8, P), FP8)         # fc_w.T*WSC tiles [mc][kp][ko][mj]
    din("fcb", (D,), F32)                 # fc_b + fc_w @ bv (host-folded)
    din("w1x", (32, P, 2, 8, P), FP8)     # ff_w1.T*WSC hi/lo [mc][kp][hl][ko][mj]
    din("fb1", (FF,), F32)
    din("w2x", (8, 4, P, 2, 8, P), FP8)   # ff_w2.T*WSC hi/lo tiles
    din("fb2", (D,), F32)
    out_ap = nc.dram_tensor("out", [TQ, D], F32, kind="ExternalOutput").ap()

    with tile.TileContext(nc) as tc:
        _emit(tc, dt, out_ap)
    nc.compile()
    return nc


def _emit(tc, dt, out_ap):
    nc = tc.nc
    with ExitStack() as ctx:
        const = ctx.enter_context(tc.tile_pool(name="const", bufs=1))
        small = ctx.enter_context(tc.tile_pool(name="small", bufs=3))
        lnio = ctx.enter_context(tc.tile_pool(name="lnio", bufs=3))
        big = ctx.enter_context(tc.tile_pool(name="bigsb", bufs=1))
        trans = ctx.enter_context(tc.tile_pool(name="trans", bufs=3))
        wpool = ctx.enter_context(tc.tile_pool(name="wstream", bufs=6))
        psc = ctx.enter_context(tc.tile_pool(name="psc", bufs=2, space="PSUM"))
        ptp = ctx.enter_context(tc.tile_pool(name="ptp", bufs=2, space="PSUM"))

        ident = const.tile([P, P], BF16)
        make_identity(nc, ident)
        eps1_t = const.tile([P, 1], F32)
        nc.vector.memset(eps1_t, EPS1)
        eps2_t = const.tile([P, 1], F32)
        nc.vector.memset(eps2_t, EPS2)

        def chanvec(name, w=8):
            t = const.tile([P, w], F32, tag=f"cv_{name}")
            nc.sync.dma_start(t, dt[name].rearrange("(m p) -> p m", p=P))
            return t

        bq_s = chanvec("bq")
        fcb_s, fb2_s = chanvec("fcb"), chanvec("fb2")
        clng_s, clnb_s = chanvec("clng"), chanvec("clnb")
        fb1_s = chanvec("fb1", 32)
        mod_s = const.tile([P, 8, 8], F32)
        for s in range(8):
            nc.sync.dma_start(mod_s[:, s, :],
                              dt["mod"][s].rearrange("(m p) -> p m", p=P))
        sh_msa, sc_msa, g_msa = mod_s[:, 0, :], mod_s[:, 1, :], mod_s[:, 2, :]
        sh_mlp, sc_mlp, g_mlp = mod_s[:, 3, :], mod_s[:, 4, :], mod_s[:, 5, :]
        g_msa_ds, g_mlp_ds = mod_s[:, 6, :], mod_s[:, 7, :]
        hm_s = const.tile([P, NHW], BF16)
        nc.sync.dma_start(hm_s, dt["hm"])
        lng_s = const.tile([P, D], F32)
        nc.sync.dma_start(lng_s, dt["lng"])
        lnb_s = const.tile([P, D], F32)
        nc.sync.dma_start(lnb_s, dt["lnb"])

        xres = big.tile([P, 4, D], F32)        # LN1 rows [t0, t0+512); later x
        attnT = big.tile([P, 8, TQ], FP8)      # concat_h(attn_h), chan-major

        with tc.tile_pool(name="bigc", bufs=1) as bigc:
            psm_cm = tc.tile_pool(name="psm", bufs=2, space="PSUM")
            psm = psm_cm.__enter__()

            def conv(h, wname, x2, nchunk, outT, bias=None, descale=None):
                """Polyphase grouped conv for head h into outT[0:64, :]."""
                hp, hc = h % 2, h // 2
                wsb = wpool.tile([P, 7, P], FP8, tag="convw", bufs=4,
                                 name=f"w_{wname}_{h}")
                nc.sync.dma_start(wsb, dt[wname][:, h])
                for c in range(nchunk):
                    ps = psm.tile([P, 256], F32, tag="conv")
                    for j in range(7):
                        nc.tensor.matmul(
                            ps, wsb[:, j, :],
                            x2[:, hp, c * 256 + j:c * 256 + j + 256],
                            start=(j == 0), stop=(j == 6))
                    if bias is not None:   # q: token-major interleave + bias
                        dst = outT[:DK, c * TQ:(c + 1) * TQ].rearrange(
                            "p (n two) -> p two n", two=2)
                        b = bias[hp * DK:(hp + 1) * DK, hc:hc + 1]
                        if h % 2 == 0:
                            nc.vector.tensor_scalar(dst[:, 0, :], ps[0:DK, :],
                                                    descale, b,
                                                    ALU.mult, ALU.add)
                            nc.scalar.activation(dst[:, 1, :], ps[DK:P, :],
                                                 AF.Identity, bias=b,
                                                 scale=descale)
                        else:
                            nc.scalar.activation(dst[:, 0, :], ps[0:DK, :],
                                                 AF.Identity, bias=b,
                                                 scale=descale)
                            nc.vector.tensor_scalar(dst[:, 1, :], ps[DK:P, :],
                                                    descale, b,
                                                    ALU.mult, ALU.add)
                    else:                  # k/v: pair-major contiguous
                        d0 = outT[:DK, c * TQ:c * TQ + 256]
                        d1 = outT[:DK, c * TQ + 256:(c + 1) * TQ]
                        nc.vector.tensor_copy(d0, ps[0:DK, :])
                        nc.vector.tensor_copy(d1, ps[DK:P, :])

            # ---- Phase N: noisy LNs -> nt2P builds -> q convs ---------------
            lnpN_cm = tc.tile_pool(name="lnpN", bufs=1)
            lnpN = lnpN_cm.__enter__()
            lnall = [lnpN.tile([P, D], BF16, name=f"lnall_{i}")
                     for i in range(6)]  # noisy ln2 tiles
            for r in range(6):
                xt = lnio.tile([P, D], BF16, tag="ln_in", bufs=2)
                nc.sync.dma_start(xt, dt["noisyH"][r * P:(r + 1) * P, :])
                if 1 <= r <= 4:
                    ln1 = xres[:, r - 1, :]
                else:
                    ln1 = lnio.tile([P, D], F32, tag="ln1_tmp", bufs=1)
                _ln_apply(nc, small, xt, ln1, eps1_t,
                          affine=(lng_s, lnb_s))
                _ln_apply(nc, small, ln1, lnall[r], eps2_t,
                          apply_eng=("act" if r % 2 else "dve"),
                          reduce_eng="act")

            nt2s = []
            for m in range(8):
                nt2m = bigc.tile([P, 2, NT2W], FP8, name=f"nt2_{m}")
                nt2s.append(nt2m)
                tmn = trans.tile([P, NHW], BF16, tag="tmn", bufs=2)
                for r in range(6):
                    pt = ptp.tile([P, P], BF16, tag="tpbf")
                    nc.tensor.transpose(pt, lnall[r][:, m * P:(m + 1) * P],
                                        ident)
                    dst = tmn[:, r * P:(r + 1) * P]
                    if r % 2 == 0:
                        nc.vector.tensor_scalar(dst, pt, sc_msa[:, m:m + 1],
                                                sh_msa[:, m:m + 1],
                                                ALU.mult, ALU.add)
                    else:
                        nc.scalar.activation(dst, pt, AF.Identity,
                                             bias=sh_msa[:, m:m + 1],
                                             scale=sc_msa[:, m:m + 1])
                for hh in range(2):
                    sl = slice(hh * DK, (hh + 1) * DK)
                    nc.gpsimd.tensor_tensor(nt2m[0:DK, hh, 0:262],
                                            tmn[sl, 122:646:2],
                                            hm_s[sl, 122:646:2], ALU.mult)
                    nc.gpsimd.tensor_tensor(nt2m[DK:P, hh, 0:262],
                                            tmn[sl, 123:647:2],
                                            hm_s[sl, 123:647:2], ALU.mult)
            lnpN_cm.__exit__(None, None, None)

            qTs = []
            for h in range(H):
                qT = bigc.tile([65, TQ], BF16, name=f"qT_{h}")
                nc.vector.memset(qT[64:65, :], 1.0)
                conv(h, "wql", nt2s[h // 2], 1, qT, bias=bq_s,
                     descale=1.0 / WSC)
                qTs.append(qT)

            # ---- Phase C: clean LNs -> ct2P builds (pair-major) -------------
            lnpC_cm = tc.tile_pool(name="lnpC", bufs=1)
            lnpC = lnpC_cm.__enter__()
            clnall = [lnpC.tile([P, D], BF16, name=f"clnall_{i}")
                      for i in range(8)]
            for r in range(8):
                xt = lnio.tile([P, D], BF16, tag="ln_in", bufs=2)
                nc.sync.dma_start(xt, dt["clean"][r * P:(r + 1) * P, :])
                _ln_apply(nc, small, xt, clnall[r], eps1_t,
                          apply_eng=("act" if r % 2 else "dve"),
                          reduce_eng=("act" if r % 2 else "dve"))
            ct2s = []
            for m in range(8):
                ct2m = bigc.tile([P, 2, CT2W], FP8, name=f"ct2_{m}")
                ct2s.append(ct2m)
                for hh in range(2):
                    nc.gpsimd.memset(ct2m[:, hh, 0:3], 0.0)
                    nc.gpsimd.memset(ct2m[:, hh, 515:CT2W], 0.0)
                tmc = trans.tile([P, T], BF16, tag="tmc", bufs=2)
                for r in range(8):
                    pt = ptp.tile([P, P], BF16, tag="tpbf")
                    nc.tensor.transpose(pt, clnall[r][:, m * P:(m + 1) * P],
                                        ident)
                    dst = tmc[:, r * P:(r + 1) * P]
                    if r % 2 == 0:
                        nc.vector.tensor_scalar(dst, pt, clng_s[:, m:m + 1],
                                                clnb_s[:, m:m + 1],
                                                ALU.mult, ALU.add)
                    else:
                        nc.scalar.activation(dst, pt, AF.Identity,
                                             bias=clnb_s[:, m:m + 1],
                                             scale=clng_s[:, m:m + 1])
                for hh in range(2):
                    sl = slice(hh * DK, (hh + 1) * DK)
                    i = (m * 2 + hh) % 4
                    if i == 0:
                        nc.gpsimd.tensor_copy(ct2m[0:DK, hh, 3:515],
                                              tmc[sl, 0::2])
                        nc.vector.tensor_copy(ct2m[DK:P, hh, 3:515],
                                              tmc[sl, 1::2])
                    elif i == 1:
                        nc.scalar.activation(ct2m[0:DK, hh, 3:515],
                                             tmc[sl, 0::2], AF.Identity)
                        nc.gpsimd.tensor_copy(ct2m[DK:P, hh, 3:515],
                                              tmc[sl, 1::2])
                    elif i == 2:
                        nc.vector.tensor_copy(ct2m[0:DK, hh, 3:515],
                                              tmc[sl, 0::2])
                        nc.gpsimd.tensor_copy(ct2m[DK:P, hh, 3:515],
                                              tmc[sl, 1::2])
                    else:
                        nc.gpsimd.tensor_copy(ct2m[0:DK, hh, 3:515],
                                              tmc[sl, 0::2])
                        nc.scalar.activation(ct2m[DK:P, hh, 3:515],
                                             tmc[sl, 1::2], AF.Identity)
            lnpC_cm.__exit__(None, None, None)

            qTs = []
            for h in range(H):
                qT = bigc.tile([65, TQ], BF16, name=f"qT_{h}")
                nc.vector.memset(qT[64:65, :], 1.0)
                conv(h, "wql", nt2s[h // 2], 1, qT, bias=bq_s,
                     descale=1.0 / WSC)
                qTs.append(qT)

            # ---- Phase K: per-head k/v conv + cross attention ---------------
            hpool_cm = tc.tile_pool(name="hpool", bufs=2)
            hpool = hpool_cm.__enter__()
            for h in range(H):
                hp, hc = h % 2, h // 2
                qT = qTs[h]
                kT = hpool.tile([65, T], BF16, tag="kT", bufs=2)
                nc.sync.dma_start(kT[64:65, :], dt["maskb"])
                conv(h, "wkl", ct2s[h // 2], 2, kT)
                vT = hpool.tile([DK, T], BF16, tag="vT", bufs=2)
                conv(h, "wvl", ct2s[h // 2], 2, vT)
                # v65: v tokens-on-partitions + 32.0 col (cancels x32 scale)
                v65 = hpool.tile([P, 8, 80], FP8, tag="v65", bufs=2)
                nc.vector.memset(v65[:, :, 64:65], WSC)
                nc.vector.memset(v65[:, :, 65:80], 0.0)
                for c2 in range(4):
                    pt = ptp.tile([P, P], BF16, tag="tpbf")
                    nc.tensor.transpose(pt[:, 0:DK],
                                        vT[:, 2 * c2 * P:(2 * c2 + 1) * P],
                                        ident[:DK, :DK])
                    nc.tensor.transpose(pt[:, DK:P],
                                        vT[:, (2 * c2 + 1) * P:(2 * c2 + 2) * P],
                                        ident[:DK, :DK])
                    nc.vector.tensor_copy(
                        v65[:, 2 * c2:2 * c2 + 2, 0:DK],
                        pt.rearrange("p (two n) -> p two n", two=2))

                # scores (x32): mask rides on kT row 64 * qT ones row;
                # fused scale/exp over 2 chunks at a time -> fp8 pT
                pT = hpool.tile([P, 8, TQ], FP8, tag="pT", bufs=2)
                for g in range(4):
                    ps2 = psc.tile([P, 2, TQ], F32, tag="sc")
                    for i in range(2):
                        cc = 2 * g + i
                        nc.tensor.matmul(ps2[:, i, :],
                                         kT[:, cc * P:(cc + 1) * P], qT,
                                         start=True, stop=True)
                    nc.scalar.activation(pT[:, 2 * g:2 * g + 2, :], ps2,
                                         AF.Exp, scale=0.125 / WSC)

                # PV fp8 DoubleRow: out rows 0:64 = 32*attn, row 64 = 32*denom
                pvt = psc.tile([P, 2, TQ], F32, tag="sc")
                pv = pvt[:, 0, :]
                for c2 in range(4):
                    nc.tensor.matmul(pv[:80, :],
                                     v65[:, 2 * c2:2 * c2 + 2, 0:80],
                                     pT[:, 2 * c2:2 * c2 + 2, :],
                                     start=(c2 == 0), stop=(c2 == 3),
                                     perf_mode=DR)
                linv = trans.tile([1, TQ], F32, tag="linv")
                nc.vector.reciprocal(linv, pv[64:65, :])
                bc_sb = trans.tile([DK, TQ], F32, tag="bcsb", bufs=2)
                nc.gpsimd.partition_broadcast(bc_sb, linv)
                nc.vector.tensor_tensor(attnT[hp * DK:(hp + 1) * DK, hc, :],
                                        pv[0:DK, :], bc_sb, ALU.mult)
            hpool_cm.__exit__(None, None, None)
            psm_cm.__exit__(None, None, None)

        # ---- Phase D: fc projection (fp8 DR) + gate + residual --------------
        fcgs = []
        for m in range(8):
            wt = wpool.tile([P, 8, P], FP8, tag="wt")
            nc.sync.dma_start(wt, dt["fcw"][m])
            ps = psc.tile([P, 2, TQ], F32, tag="sc")
            for k in range(4):
                nc.tensor.matmul(ps[:, 0, :], wt[:, 2 * k:2 * k + 2, :],
                                 attnT[:, 2 * k:2 * k + 2, :],
                                 start=(k == 0), stop=(k == 3), perf_mode=DR)
            fcg = trans.tile([P, TQ], BF16, tag="fcg", bufs=8,
                             name=f"fcg_{m}")
            fcbg = small.tile([P, 1], F32, tag="fcbg", name="fcbg")
            nc.vector.tensor_tensor(fcbg, fcb_s[:, m:m + 1],
                                    g_msa[:, m:m + 1], ALU.mult)
            nc.scalar.activation(fcg, ps[:, 0, :], AF.Identity, bias=fcbg,
                                 scale=g_msa_ds[:, m:m + 1])
            fcgs.append(fcg)
        for j in range(4):
            for m in range(8):
                pt = ptp.tile([P, P], BF16, tag="tpbf")
                nc.tensor.transpose(pt, fcgs[m][:, j * P:(j + 1) * P], ident)
                nc.vector.tensor_tensor(xres[:, j, m * P:(m + 1) * P], pt,
                                        xres[:, j, m * P:(m + 1) * P],
                                        ALU.add)

        # ---- Phase E: LN3 + mlp modulation -> n2T ---------------------------
        bigf_cm = tc.tile_pool(name="bigf", bufs=1)
        bigf = bigf_cm.__enter__()
        n2T = bigf.tile([P, 8, TQ], BF16)
        for s in range(4):
            l3 = lnio.tile([P, D], BF16, tag="ln2b")
            _ln_apply(nc, small, xres[:, s, :], l3, eps2_t,
                      apply_eng=("act" if s % 2 else "dve"),
                      reduce_eng="act")
            for m in range(8):
                pt = ptp.tile([P, P], BF16, tag="tpbf")
                nc.tensor.transpose(pt, l3[:, m * P:(m + 1) * P], ident)
                nc.vector.tensor_scalar(n2T[:, m, s * P:(s + 1) * P], pt,
                                        sc_mlp[:, m:m + 1], sh_mlp[:, m:m + 1],
                                        ALU.mult, ALU.add)

        # ---- Phase F: FFN (fp8 hi-lo DoubleRow, wl*al term dropped) ---------
        if True:
            n2h = bigf.tile([P, 8, TQ], FP8)
            n2l = bigf.tile([P, 8, TQ], FP8)
            for k2 in range(4):
                sl = slice(2 * k2, 2 * k2 + 2)
                nc.vector.tensor_copy(n2h[:, sl, :], n2T[:, sl, :])
                nc.vector.tensor_tensor(n2l[:, sl, :], n2T[:, sl, :],
                                        n2h[:, sl, :], ALU.subtract)
            ffah = bigf.tile([P, 32, TQ], FP8)
            ffal = bigf.tile([P, 32, TQ], FP8)
            for m in range(32):
                wt = wpool.tile([P, 2, 8, P], FP8, tag="wtf")
                nc.sync.dma_start(wt, dt["w1x"][m])
                ps = psc.tile([P, 2, TQ], F32, tag="sc")
                for k in range(4):
                    ksl = slice(2 * k, 2 * k + 2)
                    nc.tensor.matmul(ps[:, 0, :], wt[:, 0, ksl, :],
                                     n2h[:, ksl, :], start=(k == 0),
                                     stop=False, perf_mode=DR)
                for k in range(4):
                    ksl = slice(2 * k, 2 * k + 2)
                    nc.tensor.matmul(ps[:, 0, :], wt[:, 0, ksl, :],
                                     n2l[:, ksl, :], start=False,
                                     stop=False, perf_mode=DR)
                for k in range(4):
                    ksl = slice(2 * k, 2 * k + 2)
                    nc.tensor.matmul(ps[:, 0, :], wt[:, 1, ksl, :],
                                     n2h[:, ksl, :], start=False,
                                     stop=(k == 3), perf_mode=DR)
                nc.scalar.activation(ffah[:, m, :], ps[:, 0, :],
                                     AF.Gelu_apprx_tanh,
                                     bias=fb1_s[:, m:m + 1], scale=1.0 / WSC)
                ffg = trans.tile([P, TQ], BF16, tag="ffg", bufs=2)
                nc.scalar.activation(ffg, ps[:, 0, :], AF.Gelu_apprx_tanh,
                                     bias=fb1_s[:, m:m + 1], scale=1.0 / WSC)
                nc.vector.tensor_tensor(ffal[:, m, :], ffg, ffah[:, m, :],
                                        ALU.subtract)
            for m in range(8):
                ps = psc.tile([P, 2, TQ], F32, tag="sc")
                for kq in range(4):
                    wt = wpool.tile([P, 2, 8, P], FP8, tag="wtf")
                    nc.sync.dma_start(wt, dt["w2x"][m, kq])
                    for i in range(4):
                        isl = slice(2 * i, 2 * i + 2)
                        kk = slice(kq * 8 + 2 * i, kq * 8 + 2 * i + 2)
                        nc.tensor.matmul(ps[:, 0, :], wt[:, 0, isl, :],
                                         ffah[:, kk, :],
                                         start=(kq == 0 and i == 0),
                                         stop=False, perf_mode=DR)
                        nc.tensor.matmul(ps[:, 0, :], wt[:, 0, isl, :],
                                         ffal[:, kk, :], start=False,
                                         stop=False, perf_mode=DR)
                        nc.tensor.matmul(ps[:, 0, :], wt[:, 1, isl, :],
                                         ffah[:, kk, :], start=False,
                                         stop=(kq == 3 and i == 3),
                                         perf_mode=DR)
                ffog = trans.tile([P, TQ], BF16, tag="ffog", bufs=2)
                fbg = small.tile([P, 1], F32, tag="fcbg", name="fbg")
                nc.vector.tensor_tensor(fbg, fb2_s[:, m:m + 1],
                                        g_mlp[:, m:m + 1], ALU.mult)
                nc.scalar.activation(ffog, ps[:, 0, :], AF.Identity, bias=fbg,
                                     scale=g_mlp_ds[:, m:m + 1])
                for j in range(4):
                    pt = ptp.tile([P, P], BF16, tag="tpbf")
                    nc.tensor.transpose(pt, ffog[:, j * P:(j + 1) * P], ident)
                    nc.vector.tensor_tensor(xres[:, j, m * P:(m + 1) * P], pt,
                                            xres[:, j, m * P:(m + 1) * P],
                                            ALU.add)
        bigf_cm.__exit__(None, None, None)

        for s in range(4):
            nc.sync.dma_start(out_ap[s * P:(s + 1) * P, :], xres[:, s, :])


# --------------------------- host side --------------------------------------
_NC_CACHE = None


def _prep_conv_w_poly(w):
    # w: (D, DK, KW) grouped conv -> [128, H, 7, 128] fp8 polyphase lhsT *WSC
    wr = (np.asarray(w, np.float32) * WSC).reshape(H, DK, DK, KW)  # h,o,c,tap
    arr = np.zeros((P, H, 7, P), np.float32)
    for j in range(7):
        t = lambda k: wr[:, :, :, k].transpose(2, 0, 1)    # -> [c, h, o]
        arr[0:DK, :, j, 0:DK] = t(2 * j)                   # even out, tap 2j
        if 2 * j + 1 <= 12:
            arr[DK:P, :, j, 0:DK] = t(2 * j + 1)           # even out, 2j+1
        if j >= 1:
            arr[0:DK, :, j, DK:P] = t(2 * j - 1)           # odd out, 2j-1
        arr[DK:P, :, j, DK:P] = t(2 * j)                   # odd out, 2j
    return arr.astype(ml_dtypes.float8_e4m3fn)


def kernel(**inputs):
    global _NC_CACHE
    if _NC_CACHE is None:
        _NC_CACHE = build_nc()
    nc = _NC_CACHE

    f32 = np.float32
    bf = ml_dtypes.bfloat16
    fp8 = ml_dtypes.float8_e4m3fn
    noisy = np.asarray(inputs["noisy_feats"], f32)
    clean = np.asarray(inputs["clean_feats"], f32)
    t = np.asarray(inputs["t"], f32)
    clean_len = np.asarray(inputs["clean_lengths"]).astype(np.int64)

    # AdaLayerNormZero on host (0.02% of FLOPs): emb = silu(t) @ ada_w.T + b
    st = t * (1.0 / (1.0 + np.exp(-t, dtype=f32)))
    emb = st @ np.asarray(inputs["ada_w"], f32).T + np.asarray(inputs["ada_b"], f32)
    sh_msa, sc_msa, g_msa, sh_mlp, sc_mlp, g_mlp = np.split(emb, 6, axis=1)

    wql = _prep_conv_w_poly(inputs["wq"])
    wkl = _prep_conv_w_poly(inputs["wk"])
    wvl = _prep_conv_w_poly(inputs["wv"])
    fcw = (np.asarray(inputs["fc_w"], f32).T * WSC).reshape(8, P, 8, P) \
        .transpose(2, 1, 0, 3).astype(fp8).copy()
    def _hilo(w, axis):
        hi = w.astype(fp8)
        lo = (w - hi.astype(f32)).astype(fp8)
        return np.stack([hi, lo], axis=axis)
    w1f = (np.asarray(inputs["ff_w1"], f32).T * WSC).reshape(8, P, 32, P) \
        .transpose(2, 1, 0, 3)
    w1x = _hilo(w1f, 2).copy()
    w2f = (np.asarray(inputs["ff_w2"], f32).T * WSC).reshape(32, P, 8, P) \
        .transpose(2, 0, 1, 3).reshape(8, 4, 8, P, P) \
        .transpose(0, 1, 3, 2, 4)
    w2x = _hilo(w2f, 3).copy()
    # fold v bias into fc bias: fc(attn + bv) = fc(attn) + fc_w @ bv
    fcb_eff = (np.asarray(inputs["fc_b"], f32)
               + np.asarray(inputs["fc_w"], f32) @ np.asarray(inputs["bv"], f32))

    # pair-major t2 permutation: col c*512+par*256+n <-> token c*512+2n+par
    perm = np.empty(T, np.int64)
    for c in range(2):
        for par in range(2):
            base = c * 512 + par * 256
            perm[base:base + 256] = c * 512 + 2 * np.arange(256) + par

    common = dict(
        lng=np.broadcast_to(np.asarray(inputs["ln_noisy_g"], f32), (P, D)).copy(),
        lnb=np.broadcast_to(np.asarray(inputs["ln_noisy_b"], f32), (P, D)).copy(),
        clng=np.asarray(inputs["ln_clean_g"], f32).copy(),
        clnb=np.asarray(inputs["ln_clean_b"], f32).copy(),
        wql=wql, wkl=wkl, wvl=wvl,
        bq=np.asarray(inputs["bq"], f32).copy(),
        fcw=fcw, fcb=fcb_eff.copy(),
        w1x=w1x, fb1=np.asarray(inputs["ff_b1"], f32).copy(),
        w2x=w2x, fb2=np.asarray(inputs["ff_b2"], f32).copy(),
    )

    in_maps = []
    for i in range(8):
        b, half = i // 2, i % 2
        t0 = half * TQ
        noisyH = np.zeros((NHW, D), bf)
        lo, hi = t0 - P, t0 + 640
        clo, chi = max(lo, 0), min(hi, T)
        noisyH[clo - lo:chi - lo] = noisy[b, clo:chi].astype(bf)
        hm = np.zeros((NHW,), f32)
        hm[clo - lo:chi - lo] = 1.0
        maskb = np.where(np.arange(T) >= clean_len[b], NEG, 0.0).astype(f32)
        maskb_pm = maskb[perm][None, :].astype(bf).copy()
        mod = np.stack([sh_msa[b], 1.0 + sc_msa[b], g_msa[b],
                        sh_mlp[b], 1.0 + sc_mlp[b], g_mlp[b],
                        g_msa[b] / WSC, g_mlp[b] / WSC]).astype(f32)
        m = dict(common)
        m.update(noisyH=noisyH, clean=clean[b].astype(bf).copy(),
                 hm=np.broadcast_to(hm, (P, NHW)).astype(bf).copy(),
                 maskb=maskb_pm, mod=mod)
        in_maps.append(m)

    global _LAST_INMAPS
    _LAST_INMAPS = in_maps
    res = run_bass_kernel_spmd(nc, in_maps, core_ids=list(range(8)))
    out = np.empty((B, T, D), f32)
    for i in range(8):
        b, half = i // 2, i % 2
        out[b, half * TQ:(half + 1) * TQ] = res.results[i]["out"]
    return out


_LAST_INMAPS = None


def run_profiled(tmpdir=None):
    """Re-run the last kernel invocation with NTFF tracing; return exec ns."""
    if _NC_CACHE is None or _LAST_INMAPS is None:
        return None
    res = run_bass_kernel_spmd(_NC_CACHE, _LAST_INMAPS,
                               core_ids=list(range(8)), trace=True,
                               tmpdir=tmpdir)
    return res.exec_time_ns


if __name__ == "__main__":
    build_nc()
    print("build ok")


# revision 49
# speedup vs baseline: 1.1639x; 1.0134x over previous
# Trainium2 Bass kernel for nn_CrossAttention_noise (B=4, T1=T2=1024, D=1024,
# H=16, DK=64, K=13, FF=4096), SPMD over 8 NeuronCores.
#
# Sharding: core i handles batch b=i//2 and query-token half t0=(i%2)*512.
# Each core computes its 512 output tokens end-to-end (the K/V convolution
# over the full clean sequence is duplicated between the two cores of a
# batch; no collectives).
#
# Key structure:
#  - QKV grouped convs run "polyphase": even/odd output tokens are separate
#    PE-array columns, so each matmul streams N=256 with a full 128x128 array
#    (2x fewer PE cycles than the shifted-window form).  Conv weights+inputs
#    are fp8e4 (weights prescaled x32); K/V sbuf stores keep the x32 scale
#    and fold it into the softmax exp scale / the v65 ones-column.
#  - K/V token order is "pair-major" (evens then odds per 512-block), which
#    is softmax-invariant; the key-padding mask rides as row 64 of kT and a
#    ones row 64 of qT, so exp needs no per-chunk bias and batches 2 chunks.
#  - PV runs fp8 DoubleRow (contract 256/matmul): pT (exp output) and v65
#    are fp8e4.  fc runs fp8 DoubleRow too (attnT fp8, fcw fp8 x32).
#  - k bias is dropped (softmax-shift invariant), v bias is folded into
#    fc_b on the host (fc_b += fc_w @ bv), q bias stays on-device.
#  - FFN stays bf16 (fp8 there costs ~1.7e-2 rel err; over budget).
#  - Clean-attention path is emitted before the noisy path so PE work
#    (ct2 transposes, k/v convs) starts as soon as the 8 clean LNs finish.
import numpy as np
import ml_dtypes
from contextlib import ExitStack

import concourse.bass as bass
import concourse.mybir as mybir
import concourse.tile as tile
from concourse import bacc
from concourse.bass_utils import run_bass_kernel_spmd
from concourse.masks import make_identity

BF16 = mybir.dt.bfloat16
F32 = mybir.dt.float32
FP8 = mybir.dt.float8e4
AF = mybir.ActivationFunctionType
ALU = mybir.AluOpType
AX = mybir.AxisListType
DR = mybir.MatmulPerfMode.DoubleRow
WSC = 32.0            # fp8 weight prescale (conv weights, fc_w stored *32)

B, T, D, H, DK, KW, FF = 4, 1024, 1024, 16, 64, 13, 4096
TQ = 512          # query tokens per core
NHW = 768         # noisy halo window rows (zero-padded on host)
NT2W = 264        # nt2 pair-major width (valid cols 0..261)
CT2W = 520        # ct2 pair-major width (valid cols 3..514)
P = 128
EPS1, EPS2 = 1e-5, 1e-6
NEG = -1.0e30


def _ln_apply(nc, pool, x, out, eps_ap, affine=None, apply_eng="dve",
              reduce_eng="dve"):
    """out = (x - mean)/sqrt(var + eps) rowwise; x [p, D] in SBUF."""
    p = x.shape[0]
    s = pool.tile([P, 1], F32, tag="ln_s", name="ln_s")
    sq = pool.tile([P, 1], F32, tag="ln_sq", name="ln_sq")
    scr = pool.tile([P, D], BF16, tag="ln_scr", name="ln_scr", bufs=3)[:p]
    if reduce_eng == "dve":
        nc.vector.reduce_sum(s[:p], x, axis=AX.X)
    else:
        scr2 = pool.tile([P, D], BF16, tag="ln_scr", name="ln_scr2",
                         bufs=3)[:p]
        nc.scalar.activation(scr2, x, AF.Identity, accum_out=s[:p])
    nc.scalar.activation(scr, x, AF.Square, accum_out=sq[:p])
    mu = pool.tile([P, 1], F32, tag="ln_mu", name="ln_mu")[:p]
    nc.vector.tensor_scalar_mul(mu, s[:p], 1.0 / D)
    musq = pool.tile([P, 1], F32, tag="ln_musq", name="ln_musq")[:p]
    nc.vector.tensor_tensor(musq, mu, mu, ALU.mult)
    var = pool.tile([P, 1], F32, tag="ln_var", name="ln_var")[:p]
    nc.vector.tensor_scalar(var, sq[:p], 1.0 / D, musq, ALU.mult, ALU.subtract)
    std = pool.tile([P, 1], F32, tag="ln_std", name="ln_std")[:p]
    nc.scalar.activation(std, var, AF.Sqrt, bias=eps_ap[:p])
    rstd = pool.tile([P, 1], F32, tag="ln_rstd", name="ln_rstd")[:p]
    nc.vector.reciprocal(rstd, std)
    beta = pool.tile([P, 1], F32, tag="ln_beta", name="ln_beta")[:p]
    nc.vector.scalar_tensor_tensor(beta, mu, -1.0, rstd, ALU.mult, ALU.mult)
    if affine is not None:
        g, b = affine
        negmu = pool.tile([P, 1], F32, tag="ln_negmu", name="ln_negmu")[:p]
        nc.vector.tensor_scalar_mul(negmu, mu, -1.0)
        t1 = pool.tile([P, D], BF16, tag="ln_t1", name="ln_t1", bufs=2)[:p]
        nc.vector.scalar_tensor_tensor(t1, x, negmu, g, ALU.add, ALU.mult)
        nc.vector.scalar_tensor_tensor(out, t1, rstd, b, ALU.mult, ALU.add)
    elif apply_eng == "act":
        nc.scalar.activation(out, x, AF.Identity, bias=beta, scale=rstd)
    else:
        nc.vector.tensor_scalar(out, x, rstd, beta, ALU.mult, ALU.add)


def build_nc():
    nc = bacc.Bacc("TRN2", target_bir_lowering=False, debug=False,
                   num_devices=8)
    dt = {}

    def din(name, shape, dtype):
        dt[name] = nc.dram_tensor(name, list(shape), dtype,
                                  kind="ExternalInput").ap()

    din("noisyH", (NHW, D), BF16)          # rows [t0-128, t0+640), zero padded
    din("clean", (T, D), BF16)
    din("hm", (P, NHW), BF16)              # halo-token validity (rows equal)
    din("maskb", (1, T), BF16)            # 0 / -1e30 key mask, PAIR-MAJOR
    din("mod", (8, D), F32)               # sh/sc/g msa+mlp rows + g_msa/WSC
    din("lng", (P, D), BF16)              # ln_noisy_g broadcast to 128 rows
    din("lnb", (P, D), BF16)
    din("clng", (D,), F32)
    din("clnb", (D,), F32)
    din("wql", (P, H, 7, P), FP8)         # polyphase conv lhsT, *WSC
    din("wkl", (P, H, 7, P), FP8)
    din("wvl", (P, H, 7, P), FP8)
    din("bq", (D,), F32)
    din("fcw", (8, P, 8, P), FP8)         # fc_w.T*WSC tiles [mc][kp][ko][mj]
    din("fcb", (D,), F32)                 # fc_b + fc_w @ bv (host-folded)
    din("w1x", (32, P, 2, 8, P), FP8)     # ff_w1.T*WSC hi/lo [mc][kp][hl][ko][mj]
    din("fb1", (FF,), F32)
    din("w2x", (8, 4, P, 2, 8, P), FP8)   # ff_w2.T*WSC hi/lo tiles
    din("fb2", (D,), F32)
    out_ap = nc.dram_tensor("out", [TQ, D], F32, kind="ExternalOutput").ap()

    with tile.TileContext(nc) as tc:
        _emit(tc, dt, out_ap)
    nc.compile()
    return nc


def _emit(tc, dt, out_ap):
    nc = tc.nc
    with ExitStack() as ctx:
        const = ctx.enter_context(tc.tile_pool(name="const", bufs=1))
        small = ctx.enter_context(tc.tile_pool(name="small", bufs=3))
        lnio = ctx.enter_context(tc.tile_pool(name="lnio", bufs=3))
        big = ctx.enter_context(tc.tile_pool(name="bigsb", bufs=1))
        trans = ctx.enter_context(tc.tile_pool(name="trans", bufs=3))
        wpool = ctx.enter_context(tc.tile_pool(name="wstream", bufs=6))
        psc = ctx.enter_context(tc.tile_pool(name="psc", bufs=2, space="PSUM"))
        ptp = ctx.enter_context(tc.tile_pool(name="ptp", bufs=2, space="PSUM"))

        ident = const.tile([P, P], BF16)
        make_identity(nc, ident)
        eps1_t = const.tile([P, 1], F32)
        nc.vector.memset(eps1_t, EPS1)
        eps2_t = const.tile([P, 1], F32)
        nc.vector.memset(eps2_t, EPS2)

        def chanvec(name, w=8):
            t = const.tile([P, w], F32, tag=f"cv_{name}")
            nc.sync.dma_start(t, dt[name].rearrange("(m p) -> p m", p=P))
            return t

        bq_s = chanvec("bq")
        fcb_s, fb2_s = chanvec("fcb"), chanvec("fb2")
        clng_s, clnb_s = chanvec("clng"), chanvec("clnb")
        fb1_s = chanvec("fb1", 32)
        mod_s = const.tile([P, 8, 8], F32)
        for s in range(8):
            nc.sync.dma_start(mod_s[:, s, :],
                              dt["mod"][s].rearrange("(m p) -> p m", p=P))
        sh_msa, sc_msa, g_msa = mod_s[:, 0, :], mod_s[:, 1, :], mod_s[:, 2, :]
        sh_mlp, sc_mlp, g_mlp = mod_s[:, 3, :], mod_s[:, 4, :], mod_s[:, 5, :]
        g_msa_ds, g_mlp_ds = mod_s[:, 6, :], mod_s[:, 7, :]
        hm_s = const.tile([P, NHW], BF16)
        nc.sync.dma_start(hm_s, dt["hm"])
        lng_s = const.tile([P, D], BF16)
        nc.sync.dma_start(lng_s, dt["lng"])
        lnb_s = const.tile([P, D], BF16)
        nc.sync.dma_start(lnb_s, dt["lnb"])

        xres = big.tile([P, 4, D], F32)        # LN1 rows [t0, t0+512); later x
        attnT = big.tile([P, 8, TQ], FP8)      # concat_h(attn_h), chan-major

        with tc.tile_pool(name="bigc", bufs=1) as bigc:
            psm_cm = tc.tile_pool(name="psm", bufs=2, space="PSUM")
            psm = psm_cm.__enter__()

            def conv(h, wname, x2, nchunk, outT, bias=None, descale=None):
                """Polyphase grouped conv for head h into outT[0:64, :]."""
                hp, hc = h % 2, h // 2
                wsb = wpool.tile([P, 7, P], FP8, tag="convw", bufs=4,
                                 name=f"w_{wname}_{h}")
                nc.sync.dma_start(wsb, dt[wname][:, h])
                for c in range(nchunk):
                    ps = psm.tile([P, 256], F32, tag="conv")
                    for j in range(7):
                        nc.tensor.matmul(
                            ps, wsb[:, j, :],
                            x2[:, hp, c * 256 + j:c * 256 + j + 256],
                            start=(j == 0), stop=(j == 6))
                    if bias is not None:   # q: token-major interleave + bias
                        dst = outT[:DK, c * TQ:(c + 1) * TQ].rearrange(
                            "p (n two) -> p two n", two=2)
                        b = bias[hp * DK:(hp + 1) * DK, hc:hc + 1]
                        if h % 2 == 0:
                            nc.vector.tensor_scalar(dst[:, 0, :], ps[0:DK, :],
                                                    descale, b,
                                                    ALU.mult, ALU.add)
                            nc.scalar.activation(dst[:, 1, :], ps[DK:P, :],
                                                 AF.Identity, bias=b,
                                                 scale=descale)
                        else:
                            nc.scalar.activation(dst[:, 0, :], ps[0:DK, :],
                                                 AF.Identity, bias=b,
                                                 scale=descale)
                            nc.vector.tensor_scalar(dst[:, 1, :], ps[DK:P, :],
                                                    descale, b,
                                                    ALU.mult, ALU.add)
                    else:                  # k/v: pair-major contiguous
                        d0 = outT[:DK, c * TQ:c * TQ + 256]
                        d1 = outT[:DK, c * TQ + 256:(c + 1) * TQ]
                        nc.vector.tensor_copy(d0, ps[0:DK, :])
                        if c == 1 and wname == "wvl":
                            nc.scalar.activation(d1, ps[DK:P, :], AF.Identity)
                        else:
                            nc.vector.tensor_copy(d1, ps[DK:P, :])

            # ---- Phase N: noisy LNs -> nt2P builds -> q convs ---------------
            lnpN_cm = tc.tile_pool(name="lnpN", bufs=1)
            lnpN = lnpN_cm.__enter__()
            lnall = [lnpN.tile([P, D], BF16, name=f"lnall_{i}")
                     for i in range(6)]  # noisy ln2 tiles
            for r in range(6):
                xt = lnio.tile([P, D], BF16, tag="ln_in", bufs=2)
                nc.sync.dma_start(xt, dt["noisyH"][r * P:(r + 1) * P, :])
                if 1 <= r <= 4:
                    ln1 = xres[:, r - 1, :]
                else:
                    ln1 = lnio.tile([P, D], F32, tag="ln1_tmp", bufs=1)
                _ln_apply(nc, small, xt, ln1, eps1_t,
                          affine=(lng_s, lnb_s))
                _ln_apply(nc, small, ln1, lnall[r], eps2_t,
                          apply_eng=("act" if r % 2 else "dve"),
                          reduce_eng="act")

            nt2s = []
            for m in range(8):
                nt2m = bigc.tile([P, 2, NT2W], FP8, name=f"nt2_{m}")
                nt2s.append(nt2m)
                tmn = trans.tile([P, NHW], BF16, tag="tmn", bufs=2)
                for r in range(6):
                    pt = ptp.tile([P, P], BF16, tag="tpbf")
                    nc.tensor.transpose(pt, lnall[r][:, m * P:(m + 1) * P],
                                        ident)
                    dst = tmn[:, r * P:(r + 1) * P]
                    if r % 2 == 0:
                        nc.vector.tensor_scalar(dst, pt, sc_msa[:, m:m + 1],
                                                sh_msa[:, m:m + 1],
                                                ALU.mult, ALU.add)
                    else:
                        nc.scalar.activation(dst, pt, AF.Identity,
                                             bias=sh_msa[:, m:m + 1],
                                             scale=sc_msa[:, m:m + 1])
                for hh in range(2):
                    sl = slice(hh * DK, (hh + 1) * DK)
                    nc.gpsimd.tensor_tensor(nt2m[0:DK, hh, 0:262],
                                            tmn[sl, 122:646:2],
                                            hm_s[sl, 122:646:2], ALU.mult)
                    nc.gpsimd.tensor_tensor(nt2m[DK:P, hh, 0:262],
                                            tmn[sl, 123:647:2],
                                            hm_s[sl, 123:647:2], ALU.mult)
            lnpN_cm.__exit__(None, None, None)

            qTs = []
            for h in range(H):
                qT = bigc.tile([65, TQ], BF16, name=f"qT_{h}")
                nc.vector.memset(qT[64:65, :], 1.0)
                conv(h, "wql", nt2s[h // 2], 1, qT, bias=bq_s,
                     descale=1.0 / WSC)
                qTs.append(qT)

            # ---- Phase C: clean LNs -> ct2P builds (pair-major) -------------
            lnpC_cm = tc.tile_pool(name="lnpC", bufs=1)
            lnpC = lnpC_cm.__enter__()
            clnall = [lnpC.tile([P, D], BF16, name=f"clnall_{i}")
                      for i in range(8)]
            for r in range(8):
                xt = lnio.tile([P, D], BF16, tag="ln_in", bufs=2)
                nc.sync.dma_start(xt, dt["clean"][r * P:(r + 1) * P, :])
                _ln_apply(nc, small, xt, clnall[r], eps1_t,
                          apply_eng=("act" if r % 2 else "dve"),
                          reduce_eng=("act" if r % 2 else "dve"))
            ct2s = []
            for m in range(8):
                ct2m = bigc.tile([P, 2, CT2W], FP8, name=f"ct2_{m}")
                ct2s.append(ct2m)
                for hh in range(2):
                    nc.gpsimd.memset(ct2m[:, hh, 0:3], 0.0)
                    nc.gpsimd.memset(ct2m[:, hh, 515:CT2W], 0.0)
                tmc = trans.tile([P, T], BF16, tag="tmc", bufs=2)
                for r in range(8):
                    pt = ptp.tile([P, P], BF16, tag="tpbf")
                    nc.tensor.transpose(pt, clnall[r][:, m * P:(m + 1) * P],
                                        ident)
                    dst = tmc[:, r * P:(r + 1) * P]
                    if r % 2 == 0:
                        nc.vector.tensor_scalar(dst, pt, clng_s[:, m:m + 1],
                                                clnb_s[:, m:m + 1],
                                                ALU.mult, ALU.add)
                    else:
                        nc.scalar.activation(dst, pt, AF.Identity,
                                             bias=clnb_s[:, m:m + 1],
                                             scale=clng_s[:, m:m + 1])
                for hh in range(2):
                    sl = slice(hh * DK, (hh + 1) * DK)
                    i = (m * 2 + hh) % 4
                    if i == 0:
                        nc.gpsimd.tensor_copy(ct2m[0:DK, hh, 3:515],
                                              tmc[sl, 0::2])
                        nc.vector.tensor_copy(ct2m[DK:P, hh, 3:515],
                                              tmc[sl, 1::2])
                    elif i == 1:
                        nc.scalar.activation(ct2m[0:DK, hh, 3:515],
                                             tmc[sl, 0::2], AF.Identity)
                        nc.gpsimd.tensor_copy(ct2m[DK:P, hh, 3:515],
                                              tmc[sl, 1::2])
                    elif i == 2:
                        nc.vector.tensor_copy(ct2m[0:DK, hh, 3:515],
                                              tmc[sl, 0::2])
                        nc.gpsimd.tensor_copy(ct2m[DK:P, hh, 3:515],
                                              tmc[sl, 1::2])
                    else:
                        nc.gpsimd.tensor_copy(ct2m[0:DK, hh, 3:515],
                                              tmc[sl, 0::2])
                        nc.scalar.activation(ct2m[DK:P, hh, 3:515],
                                             tmc[sl, 1::2], AF.Identity)
            lnpC_cm.__exit__(None, None, None)

            qTs = []
            for h in range(H):
                qT = bigc.tile([65, TQ], BF16, name=f"qT_{h}")
                nc.vector.memset(qT[64:65, :], 1.0)
                conv(h, "wql", nt2s[h // 2], 1, qT, bias=bq_s,
                     descale=1.0 / WSC)
                qTs.append(qT)

            # ---- Phase K: per-head k/v conv + cross attention ---------------
            hpool_cm = tc.tile_pool(name="hpool", bufs=2)
            hpool = hpool_cm.__enter__()
            for h in range(H):
                hp, hc = h % 2, h // 2
                qT = qTs[h]
                kT = hpool.tile([65, T], BF16, tag="kT", bufs=2)
                nc.sync.dma_start(kT[64:65, :], dt["maskb"])
                conv(h, "wkl", ct2s[h // 2], 2, kT)
                vT = hpool.tile([DK, T], BF16, tag="vT", bufs=2)
                conv(h, "wvl", ct2s[h // 2], 2, vT)
                # v65: v tokens-on-partitions + 32.0 col (cancels x32 scale)
                v65 = hpool.tile([P, 8, 80], FP8, tag="v65", bufs=2)
                nc.vector.memset(v65[:, :, 64:65], WSC)
                nc.vector.memset(v65[:, :, 65:80], 0.0)
                for c2 in range(4):
                    pt = ptp.tile([P, P], BF16, tag="tpbf")
                    nc.tensor.transpose(pt[:, 0:DK],
                                        vT[:, 2 * c2 * P:(2 * c2 + 1) * P],
                                        ident[:DK, :DK])
                    nc.tensor.transpose(pt[:, DK:P],
                                        vT[:, (2 * c2 + 1) * P:(2 * c2 + 2) * P],
                                        ident[:DK, :DK])
                    nc.vector.tensor_copy(
                        v65[:, 2 * c2:2 * c2 + 2, 0:DK],
                        pt.rearrange("p (two n) -> p two n", two=2))

                # scores (x32): mask rides on kT row 64 * qT ones row;
                # fused scale/exp over 2 chunks at a time -> fp8 pT
                pT = hpool.tile([P, 8, TQ], FP8, tag="pT", bufs=2)
                for g in range(4):
                    ps2 = psc.tile([P, 2, TQ], F32, tag="sc")
                    for i in range(2):
                        cc = 2 * g + i
                        nc.tensor.matmul(ps2[:, i, :],
                                         kT[:, cc * P:(cc + 1) * P], qT,
                                         start=True, stop=True)
                    nc.scalar.activation(pT[:, 2 * g:2 * g + 2, :], ps2,
                                         AF.Exp, scale=0.125 / WSC)

                # PV fp8 DoubleRow: out rows 0:64 = 32*attn, row 64 = 32*denom
                pvt = psc.tile([P, 2, TQ], F32, tag="sc")
                pv = pvt[:, 0, :]
                for c2 in range(4):
                    nc.tensor.matmul(pv[:80, :],
                                     v65[:, 2 * c2:2 * c2 + 2, 0:80],
                                     pT[:, 2 * c2:2 * c2 + 2, :],
                                     start=(c2 == 0), stop=(c2 == 3),
                                     perf_mode=DR)
                linv = trans.tile([1, TQ], F32, tag="linv")
                nc.vector.reciprocal(linv, pv[64:65, :])
                bc_sb = trans.tile([DK, TQ], F32, tag="bcsb", bufs=2)
                nc.gpsimd.partition_broadcast(bc_sb, linv)
                nc.vector.tensor_tensor(attnT[hp * DK:(hp + 1) * DK, hc, :],
                                        pv[0:DK, :], bc_sb, ALU.mult)
            hpool_cm.__exit__(None, None, None)
            psm_cm.__exit__(None, None, None)

        # ---- Phase D: fc projection (fp8 DR) + gate + residual --------------
        fcgs = []
        for m in range(8):
            wt = wpool.tile([P, 8, P], FP8, tag="wt")
            nc.sync.dma_start(wt, dt["fcw"][m])
            ps = psc.tile([P, 2, TQ], F32, tag="sc")
            for k in range(4):
                nc.tensor.matmul(ps[:, 0, :], wt[:, 2 * k:2 * k + 2, :],
                                 attnT[:, 2 * k:2 * k + 2, :],
                                 start=(k == 0), stop=(k == 3), perf_mode=DR)
            fcg = trans.tile([P, TQ], BF16, tag="fcg", bufs=8,
                             name=f"fcg_{m}")
            fcbg = small.tile([P, 1], F32, tag="fcbg", name="fcbg")
            nc.vector.tensor_tensor(fcbg, fcb_s[:, m:m + 1],
                                    g_msa[:, m:m + 1], ALU.mult)
            nc.scalar.activation(fcg, ps[:, 0, :], AF.Identity, bias=fcbg,
                                 scale=g_msa_ds[:, m:m + 1])
            fcgs.append(fcg)
        for j in range(4):
            for m in range(8):
                pt = ptp.tile([P, P], BF16, tag="tpbf")
                nc.tensor.transpose(pt, fcgs[m][:, j * P:(j + 1) * P], ident)
                nc.vector.tensor_tensor(xres[:, j, m * P:(m + 1) * P], pt,
                                        xres[:, j, m * P:(m + 1) * P],
                                        ALU.add)

        # ---- Phase E: LN3 + mlp modulation -> n2T ---------------------------
        bigf_cm = tc.tile_pool(name="bigf", bufs=1)
        bigf = bigf_cm.__enter__()
        n2T = bigf.tile([P, 8, TQ], BF16)
        for s in range(4):
            l3 = lnio.tile([P, D], BF16, tag="ln2b")
            _ln_apply(nc, small, xres[:, s, :], l3, eps2_t,
                      apply_eng=("act" if s % 2 else "dve"),
                      reduce_eng="act")
            for m in range(8):
                pt = ptp.tile([P, P], BF16, tag="tpbf")
                nc.tensor.transpose(pt, l3[:, m * P:(m + 1) * P], ident)
                nc.vector.tensor_scalar(n2T[:, m, s * P:(s + 1) * P], pt,
                                        sc_mlp[:, m:m + 1], sh_mlp[:, m:m + 1],
                                        ALU.mult, ALU.add)

        # ---- Phase F: FFN (fp8 hi-lo DoubleRow, wl*al term dropped) ---------
        if True:
            n2h = bigf.tile([P, 8, TQ], FP8)
            n2l = bigf.tile([P, 8, TQ], FP8)
            for k2 in range(4):
                sl = slice(2 * k2, 2 * k2 + 2)
                nc.vector.tensor_copy(n2h[:, sl, :], n2T[:, sl, :])
                nc.vector.tensor_tensor(n2l[:, sl, :], n2T[:, sl, :],
                                        n2h[:, sl, :], ALU.subtract)
            ffah = bigf.tile([P, 32, TQ], FP8)
            ffal = bigf.tile([P, 32, TQ], FP8)
            for m in range(32):
                wt = wpool.tile([P, 2, 8, P], FP8, tag="wtf")
                nc.sync.dma_start(wt, dt["w1x"][m])
                ps = psc.tile([P, 2, TQ], F32, tag="sc")
                for k in range(4):
                    ksl = slice(2 * k, 2 * k + 2)
                    nc.tensor.matmul(ps[:, 0, :], wt[:, 0, ksl, :],
                                     n2h[:, ksl, :], start=(k == 0),
                                     stop=False, perf_mode=DR)
                for k in range(4):
                    ksl = slice(2 * k, 2 * k + 2)
                    nc.tensor.matmul(ps[:, 0, :], wt[:, 0, ksl, :],
                                     n2l[:, ksl, :], start=False,
                                     stop=False, perf_mode=DR)
                for k in range(4):
                    ksl = slice(2 * k, 2 * k + 2)
                    nc.tensor.matmul(ps[:, 0, :], wt[:, 1, ksl, :],
                                     n2h[:, ksl, :], start=False,
                                     stop=(k == 3), perf_mode=DR)
                ffg = trans.tile([P, TQ], BF16, tag="ffg", bufs=2)
                nc.scalar.activation(ffg, ps[:, 0, :], AF.Gelu_apprx_tanh,
                                     bias=fb1_s[:, m:m + 1], scale=1.0 / WSC)
                if m % 2 == 0:
                    nc.scalar.activation(ffah[:, m, :], ps[:, 0, :],
                                         AF.Gelu_apprx_tanh,
                                         bias=fb1_s[:, m:m + 1],
                                         scale=1.0 / WSC)
                else:
                    nc.vector.tensor_copy(ffah[:, m, :], ffg)
                nc.vector.tensor_tensor(ffal[:, m, :], ffg, ffah[:, m, :],
                                        ALU.subtract)
            for m in range(8):
                ps = psc.tile([P, 2, TQ], F32, tag="sc")
                for kq in range(4):
                    wt = wpool.tile([P, 2, 8, P], FP8, tag="wtf")
                    nc.sync.dma_start(wt, dt["w2x"][m, kq])
                    for i in range(4):
                        isl = slice(2 * i, 2 * i + 2)
                        kk = slice(kq * 8 + 2 * i, kq * 8 + 2 * i + 2)
                        nc.tensor.matmul(ps[:, 0, :], wt[:, 0, isl, :],
                                         ffah[:, kk, :],
                                         start=(kq == 0 and i == 0),
                                         stop=False, perf_mode=DR)
                        nc.tensor.matmul(ps[:, 0, :], wt[:, 0, isl, :],
                                         ffal[:, kk, :], start=False,
                                         stop=False, perf_mode=DR)
                        nc.tensor.matmul(ps[:, 0, :], wt[:, 1, isl, :],
                                         ffah[:, kk, :], start=False,
                                         stop=(kq == 3 and i == 3),
                                         perf_mode=DR)
                ffog = trans.tile([P, TQ], BF16, tag="ffog", bufs=2)
                fbg = small.tile([P, 1], F32, tag="fcbg", name="fbg")
                nc.vector.tensor_tensor(fbg, fb2_s[:, m:m + 1],
                                        g_mlp[:, m:m + 1], ALU.mult)
                nc.scalar.activation(ffog, ps[:, 0, :], AF.Identity, bias=fbg,
                                     scale=g_mlp_ds[:, m:m + 1])
                for j in range(4):
                    pt = ptp.tile([P, P], BF16, tag="tpbf")
                    nc.tensor.transpose(pt, ffog[:, j * P:(j + 1) * P], ident)
                    nc.vector.tensor_tensor(xres[:, j, m * P:(m + 1) * P], pt,
                                            xres[:, j, m * P:(m + 1) * P],
                                            ALU.add)
        bigf_cm.__exit__(None, None, None)

        for s in range(4):
            nc.sync.dma_start(out_ap[s * P:(s + 1) * P, :], xres[:, s, :])


# --------------------------- host side --------------------------------------
_NC_CACHE = None


def _prep_conv_w_poly(w):
    # w: (D, DK, KW) grouped conv -> [128, H, 7, 128] fp8 polyphase lhsT *WSC
    wr = (np.asarray(w, np.float32) * WSC).reshape(H, DK, DK, KW)  # h,o,c,tap
    arr = np.zeros((P, H, 7, P), np.float32)
    for j in range(7):
        t = lambda k: wr[:, :, :, k].transpose(2, 0, 1)    # -> [c, h, o]
        arr[0:DK, :, j, 0:DK] = t(2 * j)                   # even out, tap 2j
        if 2 * j + 1 <= 12:
            arr[DK:P, :, j, 0:DK] = t(2 * j + 1)           # even out, 2j+1
        if j >= 1:
            arr[0:DK, :, j, DK:P] = t(2 * j - 1)           # odd out, 2j-1
        arr[DK:P, :, j, DK:P] = t(2 * j)                   # odd out, 2j
    return arr.astype(ml_dtypes.float8_e4m3fn)


def kernel(**inputs):
    global _NC_CACHE
    if _NC_CACHE is None:
        _NC_CACHE = build_nc()
    nc = _NC_CACHE

    f32 = np.float32
    bf = ml_dtypes.bfloat16
    fp8 = ml_dtypes.float8_e4m3fn
    noisy = np.asarray(inputs["noisy_feats"], f32)
    clean = np.asarray(inputs["clean_feats"], f32)
    t = np.asarray(inputs["t"], f32)
    clean_len = np.asarray(inputs["clean_lengths"]).astype(np.int64)

    # AdaLayerNormZero on host (0.02% of FLOPs): emb = silu(t) @ ada_w.T + b
    st = t * (1.0 / (1.0 + np.exp(-t, dtype=f32)))
    emb = st @ np.asarray(inputs["ada_w"], f32).T + np.asarray(inputs["ada_b"], f32)
    sh_msa, sc_msa, g_msa, sh_mlp, sc_mlp, g_mlp = np.split(emb, 6, axis=1)

    wql = _prep_conv_w_poly(inputs["wq"])
    wkl = _prep_conv_w_poly(inputs["wk"])
    wvl = _prep_conv_w_poly(inputs["wv"])
    fcw = (np.asarray(inputs["fc_w"], f32).T * WSC).reshape(8, P, 8, P) \
        .transpose(2, 1, 0, 3).astype(fp8).copy()
    def _hilo(w, axis):
        hi = w.astype(fp8)
        lo = (w - hi.astype(f32)).astype(fp8)
        return np.stack([hi, lo], axis=axis)
    w1f = (np.asarray(inputs["ff_w1"], f32).T * WSC).reshape(8, P, 32, P) \
        .transpose(2, 1, 0, 3)
    w1x = _hilo(w1f, 2).copy()
    w2f = (np.asarray(inputs["ff_w2"], f32).T * WSC).reshape(32, P, 8, P) \
        .transpose(2, 0, 1, 3).reshape(8, 4, 8, P, P) \
        .transpose(0, 1, 3, 2, 4)
    w2x = _hilo(w2f, 3).copy()
    # fold v bias into fc bias: fc(attn + bv) = fc(attn) + fc_w @ bv
    fcb_eff = (np.asarray(inputs["fc_b"], f32)
               + np.asarray(inputs["fc_w"], f32) @ np.asarray(inputs["bv"], f32))

    # pair-major t2 permutation: col c*512+par*256+n <-> token c*512+2n+par
    perm = np.empty(T, np.int64)
    for c in range(2):
        for par in range(2):
            base = c * 512 + par * 256
            perm[base:base + 256] = c * 512 + 2 * np.arange(256) + par

    common = dict(
        lng=np.broadcast_to(np.asarray(inputs["ln_noisy_g"], f32), (P, D)).astype(bf).copy(),
        lnb=np.broadcast_to(np.asarray(inputs["ln_noisy_b"], f32), (P, D)).astype(bf).copy(),
        clng=np.asarray(inputs["ln_clean_g"], f32).copy(),
        clnb=np.asarray(inputs["ln_clean_b"], f32).copy(),
        wql=wql, wkl=wkl, wvl=wvl,
        bq=np.asarray(inputs["bq"], f32).copy(),
        fcw=fcw, fcb=fcb_eff.copy(),
        w1x=w1x, fb1=np.asarray(inputs["ff_b1"], f32).copy(),
        w2x=w2x, fb2=np.asarray(inputs["ff_b2"], f32).copy(),
    )

    in_maps = []
    for i in range(8):
        b, half = i // 2, i % 2
        t0 = half * TQ
        noisyH = np.zeros((NHW, D), bf)
        lo, hi = t0 - P, t0 + 640
        clo, chi = max(lo, 0), min(hi, T)
        noisyH[clo - lo:chi - lo] = noisy[b, clo:chi].astype(bf)
        hm = np.zeros((NHW,), f32)
        hm[clo - lo:chi - lo] = 1.0
        maskb = np.where(np.arange(T) >= clean_len[b], NEG, 0.0).astype(f32)
        maskb_pm = maskb[perm][None, :].astype(bf).copy()
        mod = np.stack([sh_msa[b], 1.0 + sc_msa[b], g_msa[b],
                        sh_mlp[b], 1.0 + sc_mlp[b], g_mlp[b],
                        g_msa[b] / WSC, g_mlp[b] / WSC]).astype(f32)
        m = dict(common)
        m.update(noisyH=noisyH, clean=clean[b].astype(bf).copy(),
                 hm=np.broadcast_to(hm, (P, NHW)).astype(bf).copy(),
                 maskb=maskb_pm, mod=mod)
        in_maps.append(m)

    global _LAST_INMAPS
    _LAST_INMAPS = in_maps
    res = run_bass_kernel_spmd(nc, in_maps, core_ids=list(range(8)))
    out = np.empty((B, T, D), f32)
    for i in range(8):
        b, half = i // 2, i % 2
        out[b, half * TQ:(half + 1) * TQ] = res.results[i]["out"]
    return out


_LAST_INMAPS = None


def run_profiled(tmpdir=None):
    """Re-run the last kernel invocation with NTFF tracing; return exec ns."""
    if _NC_CACHE is None or _LAST_INMAPS is None:
        return None
    res = run_bass_kernel_spmd(_NC_CACHE, _LAST_INMAPS,
                               core_ids=list(range(8)), trace=True,
                               tmpdir=tmpdir)
    return res.exec_time_ns


if __name__ == "__main__":
    build_nc()
    print("build ok")


# revision 50
# speedup vs baseline: 1.1664x; 1.0022x over previous
# Trainium2 Bass kernel for nn_CrossAttention_noise (B=4, T1=T2=1024, D=1024,
# H=16, DK=64, K=13, FF=4096), SPMD over 8 NeuronCores.
#
# Sharding: core i handles batch b=i//2 and query-token half t0=(i%2)*512.
# Each core computes its 512 output tokens end-to-end (the K/V convolution
# over the full clean sequence is duplicated between the two cores of a
# batch; no collectives).
#
# Key structure:
#  - QKV grouped convs run "polyphase": even/odd output tokens are separate
#    PE-array columns, so each matmul streams N=256 with a full 128x128 array
#    (2x fewer PE cycles than the shifted-window form).  Conv weights+inputs
#    are fp8e4 (weights prescaled x32); K/V sbuf stores keep the x32 scale
#    and fold it into the softmax exp scale / the v65 ones-column.
#  - K/V token order is "pair-major" (evens then odds per 512-block), which
#    is softmax-invariant; the key-padding mask rides as row 64 of kT and a
#    ones row 64 of qT, so exp needs no per-chunk bias and batches 2 chunks.
#  - PV runs fp8 DoubleRow (contract 256/matmul): pT (exp output) and v65
#    are fp8e4.  fc runs fp8 DoubleRow too (attnT fp8, fcw fp8 x32).
#  - k bias is dropped (softmax-shift invariant), v bias is folded into
#    fc_b on the host (fc_b += fc_w @ bv), q bias stays on-device.
#  - FFN stays bf16 (fp8 there costs ~1.7e-2 rel err; over budget).
#  - Clean-attention path is emitted before the noisy path so PE work
#    (ct2 transposes, k/v convs) starts as soon as the 8 clean LNs finish.
import numpy as np
import ml_dtypes
from contextlib import ExitStack

import concourse.bass as bass
import concourse.mybir as mybir
import concourse.tile as tile
from concourse import bacc
from concourse.bass_utils import run_bass_kernel_spmd
from concourse.masks import make_identity

BF16 = mybir.dt.bfloat16
F32 = mybir.dt.float32
FP8 = mybir.dt.float8e4
AF = mybir.ActivationFunctionType
ALU = mybir.AluOpType
AX = mybir.AxisListType
DR = mybir.MatmulPerfMode.DoubleRow
WSC = 32.0            # fp8 weight prescale (conv weights, fc_w stored *32)

B, T, D, H, DK, KW, FF = 4, 1024, 1024, 16, 64, 13, 4096
TQ = 512          # query tokens per core
NHW = 768         # noisy halo window rows (zero-padded on host)
NT2W = 264        # nt2 pair-major width (valid cols 0..261)
CT2W = 528        # ct2 pair-major width, 16B-aligned (valid cols 3..514)
P = 128
EPS1, EPS2 = 1e-5, 1e-6
NEG = -1.0e30


def _ln_apply(nc, pool, x, out, eps_ap, affine=None, apply_eng="dve",
              reduce_eng="dve"):
    """out = (x - mean)/sqrt(var + eps) rowwise; x [p, D] in SBUF."""
    p = x.shape[0]
    s = pool.tile([P, 1], F32, tag="ln_s", name="ln_s")
    sq = pool.tile([P, 1], F32, tag="ln_sq", name="ln_sq")
    scr = pool.tile([P, D], BF16, tag="ln_scr", name="ln_scr", bufs=3)[:p]
    if reduce_eng == "dve":
        nc.vector.reduce_sum(s[:p], x, axis=AX.X)
    else:
        scr2 = pool.tile([P, D], BF16, tag="ln_scr", name="ln_scr2",
                         bufs=3)[:p]
        nc.scalar.activation(scr2, x, AF.Identity, accum_out=s[:p])
    nc.scalar.activation(scr, x, AF.Square, accum_out=sq[:p])
    mu = pool.tile([P, 1], F32, tag="ln_mu", name="ln_mu")[:p]
    nc.vector.tensor_scalar_mul(mu, s[:p], 1.0 / D)
    musq = pool.tile([P, 1], F32, tag="ln_musq", name="ln_musq")[:p]
    nc.vector.tensor_tensor(musq, mu, mu, ALU.mult)
    var = pool.tile([P, 1], F32, tag="ln_var", name="ln_var")[:p]
    nc.vector.tensor_scalar(var, sq[:p], 1.0 / D, musq, ALU.mult, ALU.subtract)
    std = pool.tile([P, 1], F32, tag="ln_std", name="ln_std")[:p]
    nc.scalar.activation(std, var, AF.Sqrt, bias=eps_ap[:p])
    rstd = pool.tile([P, 1], F32, tag="ln_rstd", name="ln_rstd")[:p]
    nc.vector.reciprocal(rstd, std)
    beta = pool.tile([P, 1], F32, tag="ln_beta", name="ln_beta")[:p]
    nc.vector.scalar_tensor_tensor(beta, mu, -1.0, rstd, ALU.mult, ALU.mult)
    if affine is not None:
        g, b = affine
        negmu = pool.tile([P, 1], F32, tag="ln_negmu", name="ln_negmu")[:p]
        nc.vector.tensor_scalar_mul(negmu, mu, -1.0)
        t1 = pool.tile([P, D], BF16, tag="ln_t1", name="ln_t1", bufs=2)[:p]
        nc.vector.scalar_tensor_tensor(t1, x, negmu, g, ALU.add, ALU.mult)
        nc.vector.scalar_tensor_tensor(out, t1, rstd, b, ALU.mult, ALU.add)
    elif apply_eng == "act":
        nc.scalar.activation(out, x, AF.Identity, bias=beta, scale=rstd)
    else:
        nc.vector.tensor_scalar(out, x, rstd, beta, ALU.mult, ALU.add)


def build_nc():
    nc = bacc.Bacc("TRN2", target_bir_lowering=False, debug=False,
                   num_devices=8)
    dt = {}

    def din(name, shape, dtype):
        dt[name] = nc.dram_tensor(name, list(shape), dtype,
                                  kind="ExternalInput").ap()

    din("noisyH", (NHW, D), BF16)          # rows [t0-128, t0+640), zero padded
    din("clean", (T, D), BF16)
    din("hm", (P, NHW), BF16)              # halo-token validity (rows equal)
    din("maskb", (1, T), BF16)            # 0 / -1e30 key mask, PAIR-MAJOR
    din("mod", (8, D), F32)               # sh/sc/g msa+mlp rows + g_msa/WSC
    din("lng", (P, D), BF16)              # ln_noisy_g broadcast to 128 rows
    din("lnb", (P, D), BF16)
    din("clng", (D,), F32)
    din("clnb", (D,), F32)
    din("wql", (P, H, 7, P), FP8)         # polyphase conv lhsT, *WSC
    din("wkl", (P, H, 7, P), FP8)
    din("wvl", (P, H, 7, P), FP8)
    din("bq", (D,), F32)
    din("fcw", (8, P, 8, P), FP8)         # fc_w.T*WSC tiles [mc][kp][ko][mj]
    din("fcb", (D,), F32)                 # fc_b + fc_w @ bv (host-folded)
    din("w1x", (32, P, 2, 8, P), FP8)     # ff_w1.T*WSC hi/lo [mc][kp][hl][ko][mj]
    din("fb1", (FF,), F32)
    din("w2x", (8, 4, P, 2, 8, P), FP8)   # ff_w2.T*WSC hi/lo tiles
    din("fb2", (D,), F32)
    out_ap = nc.dram_tensor("out", [TQ, D], F32, kind="ExternalOutput").ap()

    with tile.TileContext(nc) as tc:
        _emit(tc, dt, out_ap)
    nc.compile()
    return nc


def _emit(tc, dt, out_ap):
    nc = tc.nc
    with ExitStack() as ctx:
        const = ctx.enter_context(tc.tile_pool(name="const", bufs=1))
        small = ctx.enter_context(tc.tile_pool(name="small", bufs=3))
        lnio = ctx.enter_context(tc.tile_pool(name="lnio", bufs=3))
        big = ctx.enter_context(tc.tile_pool(name="bigsb", bufs=1))
        trans = ctx.enter_context(tc.tile_pool(name="trans", bufs=3))
        wpool = ctx.enter_context(tc.tile_pool(name="wstream", bufs=6))
        psc = ctx.enter_context(tc.tile_pool(name="psc", bufs=2, space="PSUM"))
        ptp = ctx.enter_context(tc.tile_pool(name="ptp", bufs=2, space="PSUM"))

        ident = const.tile([P, P], BF16)
        make_identity(nc, ident)
        eps1_t = const.tile([P, 1], F32)
        nc.vector.memset(eps1_t, EPS1)
        eps2_t = const.tile([P, 1], F32)
        nc.vector.memset(eps2_t, EPS2)

        def chanvec(name, w=8):
            t = const.tile([P, w], F32, tag=f"cv_{name}")
            nc.sync.dma_start(t, dt[name].rearrange("(m p) -> p m", p=P))
            return t

        bq_s = chanvec("bq")
        fcb_s, fb2_s = chanvec("fcb"), chanvec("fb2")
        clng_s, clnb_s = chanvec("clng"), chanvec("clnb")
        fb1_s = chanvec("fb1", 32)
        mod_s = const.tile([P, 8, 8], F32)
        for s in range(8):
            nc.sync.dma_start(mod_s[:, s, :],
                              dt["mod"][s].rearrange("(m p) -> p m", p=P))
        sh_msa, sc_msa, g_msa = mod_s[:, 0, :], mod_s[:, 1, :], mod_s[:, 2, :]
        sh_mlp, sc_mlp, g_mlp = mod_s[:, 3, :], mod_s[:, 4, :], mod_s[:, 5, :]
        g_msa_ds, g_mlp_ds = mod_s[:, 6, :], mod_s[:, 7, :]
        hm_s = const.tile([P, NHW], BF16)
        nc.sync.dma_start(hm_s, dt["hm"])
        lng_s = const.tile([P, D], BF16)
        nc.sync.dma_start(lng_s, dt["lng"])
        lnb_s = const.tile([P, D], BF16)
        nc.sync.dma_start(lnb_s, dt["lnb"])

        xres = big.tile([P, 4, D], F32)        # LN1 rows [t0, t0+512); later x
        attnT = big.tile([P, 8, TQ], FP8)      # concat_h(attn_h), chan-major

        with tc.tile_pool(name="bigc", bufs=1) as bigc:
            psm_cm = tc.tile_pool(name="psm", bufs=2, space="PSUM")
            psm = psm_cm.__enter__()

            def conv(h, wname, x2, nchunk, outT, bias=None, descale=None):
                """Polyphase grouped conv for head h into outT[0:64, :]."""
                hp, hc = h % 2, h // 2
                wsb = wpool.tile([P, 7, P], FP8, tag="convw", bufs=4,
                                 name=f"w_{wname}_{h}")
                nc.sync.dma_start(wsb, dt[wname][:, h])
                for c in range(nchunk):
                    ps = psm.tile([P, 256], F32, tag="conv")
                    if len(x2.shape) == 4:   # dup-shifted tile: fp8 DoubleRow
                        for j2 in range(3):
                            nc.tensor.matmul(
                                ps, wsb[:, 2 * j2:2 * j2 + 2, :],
                                x2[:, hp, :, c * 256 + 2 * j2:
                                   c * 256 + 2 * j2 + 256],
                                start=(j2 == 0), stop=False, perf_mode=DR)
                        nc.tensor.matmul(
                            ps, wsb[:, 6, :],
                            x2[:, hp, 0, c * 256 + 6:c * 256 + 6 + 256],
                            start=False, stop=True)
                    else:
                        for j in range(7):
                            nc.tensor.matmul(
                                ps, wsb[:, j, :],
                                x2[:, hp, c * 256 + j:c * 256 + j + 256],
                                start=(j == 0), stop=(j == 6))
                    if bias is not None:   # q: token-major interleave + bias
                        dst = outT[:DK, c * TQ:(c + 1) * TQ].rearrange(
                            "p (n two) -> p two n", two=2)
                        b = bias[hp * DK:(hp + 1) * DK, hc:hc + 1]
                        if h % 2 == 0:
                            nc.vector.tensor_scalar(dst[:, 0, :], ps[0:DK, :],
                                                    descale, b,
                                                    ALU.mult, ALU.add)
                            nc.scalar.activation(dst[:, 1, :], ps[DK:P, :],
                                                 AF.Identity, bias=b,
                                                 scale=descale)
                        else:
                            nc.scalar.activation(dst[:, 0, :], ps[0:DK, :],
                                                 AF.Identity, bias=b,
                                                 scale=descale)
                            nc.vector.tensor_scalar(dst[:, 1, :], ps[DK:P, :],
                                                    descale, b,
                                                    ALU.mult, ALU.add)
                    else:                  # k/v: pair-major contiguous
                        d0 = outT[:DK, c * TQ:c * TQ + 256]
                        d1 = outT[:DK, c * TQ + 256:(c + 1) * TQ]
                        nc.vector.tensor_copy(d0, ps[0:DK, :])
                        if c == 1 and wname == "wvl":
                            nc.scalar.activation(d1, ps[DK:P, :], AF.Identity)
                        else:
                            nc.vector.tensor_copy(d1, ps[DK:P, :])

            # ---- Phase N: noisy LNs -> nt2P builds -> q convs ---------------
            lnpN_cm = tc.tile_pool(name="lnpN", bufs=1)
            lnpN = lnpN_cm.__enter__()
            lnall = [lnpN.tile([P, D], BF16, name=f"lnall_{i}")
                     for i in range(6)]  # noisy ln2 tiles
            for r in range(6):
                xt = lnio.tile([P, D], BF16, tag="ln_in", bufs=2)
                nc.sync.dma_start(xt, dt["noisyH"][r * P:(r + 1) * P, :])
                if 1 <= r <= 4:
                    ln1 = xres[:, r - 1, :]
                else:
                    ln1 = lnio.tile([P, D], F32, tag="ln1_tmp", bufs=1)
                _ln_apply(nc, small, xt, ln1, eps1_t,
                          affine=(lng_s, lnb_s))
                _ln_apply(nc, small, ln1, lnall[r], eps2_t,
                          apply_eng=("act" if r % 2 else "dve"),
                          reduce_eng="act")

            nt2s = []
            for m in range(8):
                nt2m = bigc.tile([P, 2, NT2W], FP8, name=f"nt2_{m}")
                nt2s.append(nt2m)
                tmn = trans.tile([P, NHW], BF16, tag="tmn", bufs=2)
                for r in range(6):
                    pt = ptp.tile([P, P], BF16, tag="tpbf")
                    nc.tensor.transpose(pt, lnall[r][:, m * P:(m + 1) * P],
                                        ident)
                    dst = tmn[:, r * P:(r + 1) * P]
                    if r % 2 == 0:
                        nc.vector.tensor_scalar(dst, pt, sc_msa[:, m:m + 1],
                                                sh_msa[:, m:m + 1],
                                                ALU.mult, ALU.add)
                    else:
                        nc.scalar.activation(dst, pt, AF.Identity,
                                             bias=sh_msa[:, m:m + 1],
                                             scale=sc_msa[:, m:m + 1])
                for hh in range(2):
                    sl = slice(hh * DK, (hh + 1) * DK)
                    nc.gpsimd.tensor_tensor(nt2m[0:DK, hh, 0:262],
                                            tmn[sl, 122:646:2],
                                            hm_s[sl, 122:646:2], ALU.mult)
                    nc.gpsimd.tensor_tensor(nt2m[DK:P, hh, 0:262],
                                            tmn[sl, 123:647:2],
                                            hm_s[sl, 123:647:2], ALU.mult)
            lnpN_cm.__exit__(None, None, None)

            qTs = []
            for h in range(H):
                qT = bigc.tile([65, TQ], BF16, name=f"qT_{h}")
                nc.vector.memset(qT[64:65, :], 1.0)
                conv(h, "wql", nt2s[h // 2], 1, qT, bias=bq_s,
                     descale=1.0 / WSC)
                qTs.append(qT)

            # ---- Phase C: clean LNs -> ct2P builds (pair-major) -------------
            lnpC_cm = tc.tile_pool(name="lnpC", bufs=1)
            lnpC = lnpC_cm.__enter__()
            clnall = [lnpC.tile([P, D], BF16, name=f"clnall_{i}")
                      for i in range(8)]
            for r in range(8):
                xt = lnio.tile([P, D], BF16, tag="ln_in", bufs=2)
                nc.sync.dma_start(xt, dt["clean"][r * P:(r + 1) * P, :])
                _ln_apply(nc, small, xt, clnall[r], eps1_t,
                          apply_eng=("act" if r % 2 else "dve"),
                          reduce_eng=("act" if r % 2 else "dve"))
            ct2s = []
            for m in range(8):
                ct2m = bigc.tile([P, 2, 2, CT2W], FP8, name=f"ct2_{m}")
                ct2s.append(ct2m)
                for hh in range(2):
                    nc.gpsimd.memset(ct2m[:, hh, 0, 0:3], 0.0)
                    nc.gpsimd.memset(ct2m[:, hh, 0, 515:CT2W], 0.0)
                    nc.gpsimd.memset(ct2m[:, hh, 1, 0:2], 0.0)
                    nc.gpsimd.memset(ct2m[:, hh, 1, 514:CT2W], 0.0)
                tmc = trans.tile([P, T], BF16, tag="tmc", bufs=2)
                for r in range(8):
                    pt = ptp.tile([P, P], BF16, tag="tpbf")
                    nc.tensor.transpose(pt, clnall[r][:, m * P:(m + 1) * P],
                                        ident)
                    dst = tmc[:, r * P:(r + 1) * P]
                    if r % 2 == 0:
                        nc.vector.tensor_scalar(dst, pt, clng_s[:, m:m + 1],
                                                clnb_s[:, m:m + 1],
                                                ALU.mult, ALU.add)
                    else:
                        nc.scalar.activation(dst, pt, AF.Identity,
                                             bias=clnb_s[:, m:m + 1],
                                             scale=clng_s[:, m:m + 1])
                for hh in range(2):
                    sl = slice(hh * DK, (hh + 1) * DK)
                    dsts = [ct2m[0:DK, hh, 0, 3:515], ct2m[DK:P, hh, 0, 3:515],
                            ct2m[0:DK, hh, 1, 2:514], ct2m[DK:P, hh, 1, 2:514]]
                    srcs = [tmc[sl, 0::2], tmc[sl, 1::2],
                            tmc[sl, 0::2], tmc[sl, 1::2]]
                    engs = ([nc.gpsimd, nc.vector, nc.gpsimd, None] if hh == 0
                            else [nc.vector, nc.gpsimd, None, nc.gpsimd])
                    for dd, ss, ee in zip(dsts, srcs, engs):
                        if ee is None:
                            nc.scalar.activation(dd, ss, AF.Identity)
                        else:
                            ee.tensor_copy(dd, ss)
            lnpC_cm.__exit__(None, None, None)

            qTs = []
            for h in range(H):
                qT = bigc.tile([65, TQ], BF16, name=f"qT_{h}")
                nc.vector.memset(qT[64:65, :], 1.0)
                conv(h, "wql", nt2s[h // 2], 1, qT, bias=bq_s,
                     descale=1.0 / WSC)
                qTs.append(qT)

            # ---- Phase K: per-head k/v conv + cross attention ---------------
            hpool_cm = tc.tile_pool(name="hpool", bufs=2)
            hpool = hpool_cm.__enter__()
            for h in range(H):
                hp, hc = h % 2, h // 2
                qT = qTs[h]
                kT = hpool.tile([65, T], BF16, tag="kT", bufs=2)
                nc.sync.dma_start(kT[64:65, :], dt["maskb"])
                conv(h, "wkl", ct2s[h // 2], 2, kT)
                vT = hpool.tile([DK, T], BF16, tag="vT", bufs=2)
                conv(h, "wvl", ct2s[h // 2], 2, vT)
                # v65: v tokens-on-partitions + 32.0 col (cancels x32 scale)
                v65 = hpool.tile([P, 8, 80], FP8, tag="v65", bufs=2)
                nc.vector.memset(v65[:, :, 64:65], WSC)
                nc.vector.memset(v65[:, :, 65:80], 0.0)
                for c2 in range(4):
                    pt = ptp.tile([P, P], BF16, tag="tpbf")
                    nc.tensor.transpose(pt[:, 0:DK],
                                        vT[:, 2 * c2 * P:(2 * c2 + 1) * P],
                                        ident[:DK, :DK])
                    nc.tensor.transpose(pt[:, DK:P],
                                        vT[:, (2 * c2 + 1) * P:(2 * c2 + 2) * P],
                                        ident[:DK, :DK])
                    nc.vector.tensor_copy(
                        v65[:, 2 * c2:2 * c2 + 2, 0:DK],
                        pt.rearrange("p (two n) -> p two n", two=2))

                # scores (x32): mask rides on kT row 64 * qT ones row;
                # fused scale/exp over 2 chunks at a time -> fp8 pT
                pT = hpool.tile([P, 8, TQ], FP8, tag="pT", bufs=2)
                for g in range(4):
                    ps2 = psc.tile([P, 2, TQ], F32, tag="sc")
                    for i in range(2):
                        cc = 2 * g + i
                        nc.tensor.matmul(ps2[:, i, :],
                                         kT[:, cc * P:(cc + 1) * P], qT,
                                         start=True, stop=True)
                    nc.scalar.activation(pT[:, 2 * g:2 * g + 2, :], ps2,
                                         AF.Exp, scale=0.125 / WSC)

                # PV fp8 DoubleRow: out rows 0:64 = 32*attn, row 64 = 32*denom
                pvt = psc.tile([P, 2, TQ], F32, tag="sc")
                pv = pvt[:, 0, :]
                for c2 in range(4):
                    nc.tensor.matmul(pv[:80, :],
                                     v65[:, 2 * c2:2 * c2 + 2, 0:80],
                                     pT[:, 2 * c2:2 * c2 + 2, :],
                                     start=(c2 == 0), stop=(c2 == 3),
                                     perf_mode=DR)
                linv = trans.tile([1, TQ], F32, tag="linv")
                nc.vector.reciprocal(linv, pv[64:65, :])
                bc_sb = trans.tile([DK, TQ], F32, tag="bcsb", bufs=2)
                nc.gpsimd.partition_broadcast(bc_sb, linv)
                nc.vector.tensor_tensor(attnT[hp * DK:(hp + 1) * DK, hc, :],
                                        pv[0:DK, :], bc_sb, ALU.mult)
            hpool_cm.__exit__(None, None, None)
            psm_cm.__exit__(None, None, None)

        # ---- Phase D: fc projection (fp8 DR) + gate + residual --------------
        fcgs = []
        for m in range(8):
            wt = wpool.tile([P, 8, P], FP8, tag="wt")
            nc.sync.dma_start(wt, dt["fcw"][m])
            ps = psc.tile([P, 2, TQ], F32, tag="sc")
            for k in range(4):
                nc.tensor.matmul(ps[:, 0, :], wt[:, 2 * k:2 * k + 2, :],
                                 attnT[:, 2 * k:2 * k + 2, :],
                                 start=(k == 0), stop=(k == 3), perf_mode=DR)
            fcg = trans.tile([P, TQ], BF16, tag="fcg", bufs=8,
                             name=f"fcg_{m}")
            fcbg = small.tile([P, 1], F32, tag="fcbg", name="fcbg")
            nc.vector.tensor_tensor(fcbg, fcb_s[:, m:m + 1],
                                    g_msa[:, m:m + 1], ALU.mult)
            nc.scalar.activation(fcg, ps[:, 0, :], AF.Identity, bias=fcbg,
                                 scale=g_msa_ds[:, m:m + 1])
            fcgs.append(fcg)
        for j in range(4):
            for m in range(8):
                pt = ptp.tile([P, P], BF16, tag="tpbf")
                nc.tensor.transpose(pt, fcgs[m][:, j * P:(j + 1) * P], ident)
                nc.vector.tensor_tensor(xres[:, j, m * P:(m + 1) * P], pt,
                                        xres[:, j, m * P:(m + 1) * P],
                                        ALU.add)

        # ---- Phase E: LN3 + mlp modulation -> n2T ---------------------------
        bigf_cm = tc.tile_pool(name="bigf", bufs=1)
        bigf = bigf_cm.__enter__()
        n2T = bigf.tile([P, 8, TQ], BF16)
        for s in range(4):
            l3 = lnio.tile([P, D], BF16, tag="ln2b")
            _ln_apply(nc, small, xres[:, s, :], l3, eps2_t,
                      apply_eng=("act" if s % 2 else "dve"),
                      reduce_eng="act")
            for m in range(8):
                pt = ptp.tile([P, P], BF16, tag="tpbf")
                nc.tensor.transpose(pt, l3[:, m * P:(m + 1) * P], ident)
                nc.vector.tensor_scalar(n2T[:, m, s * P:(s + 1) * P], pt,
                                        sc_mlp[:, m:m + 1], sh_mlp[:, m:m + 1],
                                        ALU.mult, ALU.add)

        # ---- Phase F: FFN (fp8 hi-lo DoubleRow, wl*al term dropped) ---------
        if True:
            n2h = bigf.tile([P, 8, TQ], FP8)
            n2l = bigf.tile([P, 8, TQ], FP8)
            for k2 in range(4):
                sl = slice(2 * k2, 2 * k2 + 2)
                nc.vector.tensor_copy(n2h[:, sl, :], n2T[:, sl, :])
                nc.vector.tensor_tensor(n2l[:, sl, :], n2T[:, sl, :],
                                        n2h[:, sl, :], ALU.subtract)
            ffah = bigf.tile([P, 32, TQ], FP8)
            ffal = bigf.tile([P, 32, TQ], FP8)
            for m in range(32):
                wt = wpool.tile([P, 2, 8, P], FP8, tag="wtf")
                nc.sync.dma_start(wt, dt["w1x"][m])
                ps = psc.tile([P, 2, TQ], F32, tag="sc")
                for k in range(4):
                    ksl = slice(2 * k, 2 * k + 2)
                    nc.tensor.matmul(ps[:, 0, :], wt[:, 0, ksl, :],
                                     n2h[:, ksl, :], start=(k == 0),
                                     stop=False, perf_mode=DR)
                for k in range(4):
                    ksl = slice(2 * k, 2 * k + 2)
                    nc.tensor.matmul(ps[:, 0, :], wt[:, 0, ksl, :],
                                     n2l[:, ksl, :], start=False,
                                     stop=False, perf_mode=DR)
                for k in range(4):
                    ksl = slice(2 * k, 2 * k + 2)
                    nc.tensor.matmul(ps[:, 0, :], wt[:, 1, ksl, :],
                                     n2h[:, ksl, :], start=False,
                                     stop=(k == 3), perf_mode=DR)
                ffg = trans.tile([P, TQ], BF16, tag="ffg", bufs=2)
                nc.scalar.activation(ffg, ps[:, 0, :], AF.Gelu_apprx_tanh,
                                     bias=fb1_s[:, m:m + 1], scale=1.0 / WSC)
                if m % 2 == 0:
                    nc.scalar.activation(ffah[:, m, :], ps[:, 0, :],
                                         AF.Gelu_apprx_tanh,
                                         bias=fb1_s[:, m:m + 1],
                                         scale=1.0 / WSC)
                else:
                    nc.vector.tensor_copy(ffah[:, m, :], ffg)
                nc.vector.tensor_tensor(ffal[:, m, :], ffg, ffah[:, m, :],
                                        ALU.subtract)
            for m in range(8):
                ps = psc.tile([P, 2, TQ], F32, tag="sc")
                for kq in range(4):
                    wt = wpool.tile([P, 2, 8, P], FP8, tag="wtf")
                    nc.sync.dma_start(wt, dt["w2x"][m, kq])
                    for i in range(4):
                        isl = slice(2 * i, 2 * i + 2)
                        kk = slice(kq * 8 + 2 * i, kq * 8 + 2 * i + 2)
                        nc.tensor.matmul(ps[:, 0, :], wt[:, 0, isl, :],
                                         ffah[:, kk, :],
                                         start=(kq == 0 and i == 0),
                                         stop=False, perf_mode=DR)
                        nc.tensor.matmul(ps[:, 0, :], wt[:, 0, isl, :],
                                         ffal[:, kk, :], start=False,
                                         stop=False, perf_mode=DR)
                        nc.tensor.matmul(ps[:, 0, :], wt[:, 1, isl, :],
                                         ffah[:, kk, :], start=False,
                                         stop=(kq == 3 and i == 3),
                                         perf_mode=DR)
                ffog = trans.tile([P, TQ], BF16, tag="ffog", bufs=2)
                fbg = small.tile([P, 1], F32, tag="fcbg", name="fbg")
                nc.vector.tensor_tensor(fbg, fb2_s[:, m:m + 1],
                                        g_mlp[:, m:m + 1], ALU.mult)
                nc.scalar.activation(ffog, ps[:, 0, :], AF.Identity, bias=fbg,
                                     scale=g_mlp_ds[:, m:m + 1])
                for j in range(4):
                    pt = ptp.tile([P, P], BF16, tag="tpbf")
                    nc.tensor.transpose(pt, ffog[:, j * P:(j + 1) * P], ident)
                    nc.vector.tensor_tensor(xres[:, j, m * P:(m + 1) * P], pt,
                                            xres[:, j, m * P:(m + 1) * P],
                                            ALU.add)
        bigf_cm.__exit__(None, None, None)

        for s in range(4):
            nc.sync.dma_start(out_ap[s * P:(s + 1) * P, :], xres[:, s, :])


# --------------------------- host side --------------------------------------
_NC_CACHE = None


def _prep_conv_w_poly(w):
    # w: (D, DK, KW) grouped conv -> [128, H, 7, 128] fp8 polyphase lhsT *WSC
    wr = (np.asarray(w, np.float32) * WSC).reshape(H, DK, DK, KW)  # h,o,c,tap
    arr = np.zeros((P, H, 7, P), np.float32)
    for j in range(7):
        t = lambda k: wr[:, :, :, k].transpose(2, 0, 1)    # -> [c, h, o]
        arr[0:DK, :, j, 0:DK] = t(2 * j)                   # even out, tap 2j
        if 2 * j + 1 <= 12:
            arr[DK:P, :, j, 0:DK] = t(2 * j + 1)           # even out, 2j+1
        if j >= 1:
            arr[0:DK, :, j, DK:P] = t(2 * j - 1)           # odd out, 2j-1
        arr[DK:P, :, j, DK:P] = t(2 * j)                   # odd out, 2j
    return arr.astype(ml_dtypes.float8_e4m3fn)


def kernel(**inputs):
    global _NC_CACHE
    if _NC_CACHE is None:
        _NC_CACHE = build_nc()
    nc = _NC_CACHE

    f32 = np.float32
    bf = ml_dtypes.bfloat16
    fp8 = ml_dtypes.float8_e4m3fn
    noisy = np.asarray(inputs["noisy_feats"], f32)
    clean = np.asarray(inputs["clean_feats"], f32)
    t = np.asarray(inputs["t"], f32)
    clean_len = np.asarray(inputs["clean_lengths"]).astype(np.int64)

    # AdaLayerNormZero on host (0.02% of FLOPs): emb = silu(t) @ ada_w.T + b
    st = t * (1.0 / (1.0 + np.exp(-t, dtype=f32)))
    emb = st @ np.asarray(inputs["ada_w"], f32).T + np.asarray(inputs["ada_b"], f32)
    sh_msa, sc_msa, g_msa, sh_mlp, sc_mlp, g_mlp = np.split(emb, 6, axis=1)

    wql = _prep_conv_w_poly(inputs["wq"])
    wkl = _prep_conv_w_poly(inputs["wk"])
    wvl = _prep_conv_w_poly(inputs["wv"])
    fcw = (np.asarray(inputs["fc_w"], f32).T * WSC).reshape(8, P, 8, P) \
        .transpose(2, 1, 0, 3).astype(fp8).copy()
    def _hilo(w, axis):
        hi = w.astype(fp8)
        lo = (w - hi.astype(f32)).astype(fp8)
        return np.stack([hi, lo], axis=axis)
    w1f = (np.asarray(inputs["ff_w1"], f32).T * WSC).reshape(8, P, 32, P) \
        .transpose(2, 1, 0, 3)
    w1x = _hilo(w1f, 2).copy()
    w2f = (np.asarray(inputs["ff_w2"], f32).T * WSC).reshape(32, P, 8, P) \
        .transpose(2, 0, 1, 3).reshape(8, 4, 8, P, P) \
        .transpose(0, 1, 3, 2, 4)
    w2x = _hilo(w2f, 3).copy()
    # fold v bias into fc bias: fc(attn + bv) = fc(attn) + fc_w @ bv
    fcb_eff = (np.asarray(inputs["fc_b"], f32)
               + np.asarray(inputs["fc_w"], f32) @ np.asarray(inputs["bv"], f32))

    # pair-major t2 permutation: col c*512+par*256+n <-> token c*512+2n+par
    perm = np.empty(T, np.int64)
    for c in range(2):
        for par in range(2):
            base = c * 512 + par * 256
            perm[base:base + 256] = c * 512 + 2 * np.arange(256) + par

    common = dict(
        lng=np.broadcast_to(np.asarray(inputs["ln_noisy_g"], f32), (P, D)).astype(bf).copy(),
        lnb=np.broadcast_to(np.asarray(inputs["ln_noisy_b"], f32), (P, D)).astype(bf).copy(),
        clng=np.asarray(inputs["ln_clean_g"], f32).copy(),
        clnb=np.asarray(inputs["ln_clean_b"], f32).copy(),
        wql=wql, wkl=wkl, wvl=wvl,
        bq=np.asarray(inputs["bq"], f32).copy(),
        fcw=fcw, fcb=fcb_eff.copy(),
        w1x=w1x, fb1=np.asarray(inputs["ff_b1"], f32).copy(),
        w2x=w2x, fb2=np.asarray(inputs["ff_b2"], f32).copy(),
    )

    in_maps = []
    for i in range(8):
        b, half = i // 2, i % 2
        t0 = half * TQ
        noisyH = np.zeros((NHW, D), bf)
        lo, hi = t0 - P, t0 + 640
        clo, chi = max(lo, 0), min(hi, T)
        noisyH[clo - lo:chi - lo] = noisy[b, clo:chi].astype(bf)
        hm = np.zeros((NHW,), f32)
        hm[clo - lo:chi - lo] = 1.0
        maskb = np.where(np.arange(T) >= clean_len[b], NEG, 0.0).astype(f32)
        maskb_pm = maskb[perm][None, :].astype(bf).copy()
        mod = np.stack([sh_msa[b], 1.0 + sc_msa[b], g_msa[b],
                        sh_mlp[b], 1.0 + sc_mlp[b], g_mlp[b],
                        g_msa[b] / WSC, g_mlp[b] / WSC]).astype(f32)
        m = dict(common)
        m.update(noisyH=noisyH, clean=clean[b].astype(bf).copy(),
                 hm=np.broadcast_to(hm, (P, NHW)).astype(bf).copy(),
                 maskb=maskb_pm, mod=mod)
        in_maps.append(m)

    global _LAST_INMAPS
    _LAST_INMAPS = in_maps
    res = run_bass_kernel_spmd(nc, in_maps, core_ids=list(range(8)))
    out = np.empty((B, T, D), f32)
    for i in range(8):
        b, half = i // 2, i % 2
        out[b, half * TQ:(half + 1) * TQ] = res.results[i]["out"]
    return out


_LAST_INMAPS = None


def run_profiled(tmpdir=None):
    """Re-run the last kernel invocation with NTFF tracing; return exec ns."""
    if _NC_CACHE is None or _LAST_INMAPS is None:
        return None
    res = run_bass_kernel_spmd(_NC_CACHE, _LAST_INMAPS,
                               core_ids=list(range(8)), trace=True,
                               tmpdir=tmpdir)
    return res.exec_time_ns


if __name__ == "__main__":
    build_nc()
    print("build ok")
